# revision 32
# baseline (speedup 1.0000x reference)
"""Trainium2 Bass kernel for an 8-batch image-conditioned decoder layer.

Strategy: pure data-parallel over the batch — core c computes batch element c
end-to-end (causal self-attention, cross-attention over the image tokens, both
layernorms, vocab projection). No collectives.

v2 schedule notes:
- Embedding gather + positional encoding are host-prepped (pure data movement);
  the device receives x0 in both seq-partition and d-partition layouts, so the
  TensorEngine starts on real work as soon as the first weights land.
- Dummy warmup matmuls run during the initial DMA window so the PE HAM clock
  gate is at full rate when QT starts.
- b1 is folded into bq2/bv2, and g2/b2 into Wp/bp (host-side), which trims the
  layernorm critical path; layer-2 LN output needs no affine at all.
- Layer 2 runs per-qt-pipelined (Q2T in two 256-col halves) and the first 4
  vocab chunks are computed early for qt {0,1} to keep the PE fed while the
  tail of attention drains; those chunks are re-streamed later for qt {2,3}.
- PSUM->SBUF copies, value-bias adds, and half of the element-wise work run on
  GpSimd; output DMAs go on the vector/gpsimd queues so sync/scalar stay free
  for Wp streaming.

All matmuls run in bf16 with fp32 PSUM accumulation.
"""

import os
import sys

for _p in ("/opt/trn_rl_repo", "/root/.axon_site/_ro/trn_rl_repo"):
    if os.path.isdir(_p) and _p not in sys.path:
        sys.path.append(_p)

import numpy as np
import ml_dtypes

BF16 = ml_dtypes.bfloat16

# Problem dims (hardcoded per spec)
V, D, DI, S, B, NI = 32000, 1024, 768, 512, 8, 197
EPS = 1e-5
P = 128
ST = S // P          # 4 seq tiles
DT = D // P          # 8 model-dim tiles
DIT = DI // P        # 6 image-dim tiles
NIT = 2              # image tokens: 197 -> 2 partition tiles (128 + 69)
NI_PAD = 256
VP = 32768           # vocab padded to 64 chunks of 512
CN = 512             # vocab chunk width
NCHUNK = VP // CN    # 64
GRP = 2              # chunks per output strip
NGRP = NCHUNK // GRP
NE = 4               # chunks computed early for qt {0,1} (re-streamed later)
N_CORES = 8
SCALE = 1.0 / float(np.sqrt(np.float32(D)))

_CACHE = {}
LAST_RESULTS = None


def _build_program():
    import concourse.bacc as bacc
    import concourse.bass as bass
    import concourse.mybir as mybir
    from concourse.masks import make_identity
    from concourse.tile import TileContext

    f32 = mybir.dt.float32
    bf16 = mybir.dt.bfloat16
    X = mybir.AxisListType.X
    ALU = mybir.AluOpType
    ACT_F = mybir.ActivationFunctionType

    nc = bacc.Bacc("TRN2", target_bir_lowering=False, debug=False,
                   num_devices=N_CORES)

    # ---- I/O ----
    h_x0b = nc.dram_tensor("x0b", [P, ST, D], bf16, kind="ExternalInput")
    h_x0T = nc.dram_tensor("x0t", [P, DT, S], bf16, kind="ExternalInput")
    h_img = nc.dram_tensor("img_t", [P, DIT, NI], bf16, kind="ExternalInput")
    h_wq1 = nc.dram_tensor("wq1c", [DT, P, DT, P], bf16, kind="ExternalInput")
    h_wk1 = nc.dram_tensor("wk1c", [DT, P, DT, P], bf16, kind="ExternalInput")
    h_wv1 = nc.dram_tensor("wv1", [P, DT, D], bf16, kind="ExternalInput")
    h_wq2 = nc.dram_tensor("wq2", [P, DT, D], bf16, kind="ExternalInput")
    h_wk2 = nc.dram_tensor("wk2c", [DT, P, DIT, P], bf16, kind="ExternalInput")
    h_wv2 = nc.dram_tensor("wv2", [P, DIT, D], bf16, kind="ExternalInput")
    h_wp = nc.dram_tensor("wp", [NCHUNK, P, DT, CN], bf16, kind="ExternalInput")
    h_bq1 = nc.dram_tensor("bq1", [P, DT], f32, kind="ExternalInput")
    h_bk1 = nc.dram_tensor("bk1", [P, DT], f32, kind="ExternalInput")
    h_bq2 = nc.dram_tensor("bq2", [P, DT], f32, kind="ExternalInput")
    h_bk2 = nc.dram_tensor("bk2", [P, DT], f32, kind="ExternalInput")
    h_bv2 = nc.dram_tensor("bv2", [D], f32, kind="ExternalInput")
    h_g1 = nc.dram_tensor("g1", [D], f32, kind="ExternalInput")
    h_bp = nc.dram_tensor("bp", [VP], bf16, kind="ExternalInput")
    h_out = nc.dram_tensor("out", [S, VP], bf16, kind="ExternalOutput")

    def bcast(handle, n, offset=0):
        ap = handle[:]
        return bass.AP(tensor=ap.tensor, offset=offset, ap=[[0, P], [1, n]])

    with TileContext(nc) as tc:
        import contextlib
        ctx = contextlib.ExitStack()
        with ctx:
            const = ctx.enter_context(tc.tile_pool(name="const", bufs=1))
            xb_p = ctx.enter_context(tc.tile_pool(name="xb", bufs=3))
            xt_p = ctx.enter_context(tc.tile_pool(name="xt", bufs=2))
            qk_p = ctx.enter_context(tc.tile_pool(name="qk", bufs=2))
            v_p = ctx.enter_context(tc.tile_pool(name="vp", bufs=2))
            k2t_p = ctx.enter_context(tc.tile_pool(name="k2t", bufs=1))
            pb_p = ctx.enter_context(tc.tile_pool(name="pb", bufs=4))
            pt_p = ctx.enter_context(tc.tile_pool(name="pt", bufs=1))
            xpre_p = ctx.enter_context(tc.tile_pool(name="xpre", bufs=2))
            stat_p = ctx.enter_context(tc.tile_pool(name="stat", bufs=4))
            wts_p = ctx.enter_context(tc.tile_pool(name="wts", bufs=2))
            wv2_p = ctx.enter_context(tc.tile_pool(name="wv2p", bufs=1))
            wqm_p = ctx.enter_context(tc.tile_pool(name="wqm", bufs=3))
            wp_p = ctx.enter_context(tc.tile_pool(name="wpp", bufs=3))
            bp_p = ctx.enter_context(tc.tile_pool(name="bpp", bufs=2))
            osb_p = ctx.enter_context(tc.tile_pool(name="osb", bufs=6))
            ps = ctx.enter_context(tc.tile_pool(name="ps", bufs=8, space="PSUM"))

            # ---- constants / warmup ----
            ident = const.tile([P, P], bf16)
            make_identity(nc, ident)
            trimask = const.tile([P, P], f32)
            nc.gpsimd.memset(trimask, 0.0)
            nc.gpsimd.affine_select(
                out=trimask, in_=trimask, compare_op=ALU.is_ge, fill=-1e10,
                base=0, pattern=[[-1, P]], channel_multiplier=1)
            warm_src = const.tile([P, 256], bf16)
            nc.vector.memset(warm_src, 0.0)
            epst = const.tile([P, 1], f32)
            nc.vector.memset(epst, EPS)
            neg1 = const.tile([P, 1], f32)
            nc.vector.memset(neg1, -1.0)

            # HAM warmup: keep the PE busy while the first weights stream in
            for w in range(20):
                pw = ps.tile([P, 512], f32, tag="ps", name=f"warm{w}")
                nc.tensor.matmul(pw[:, :256], lhsT=ident, rhs=warm_src,
                                 start=True, stop=True)

            # ---- early DMAs (per-queue order == emission order; the wqm
            # chunk-ring slot waits naturally pace later transfers behind
            # earlier consumption, keeping HBM free for the critical path) ----
            x0T = xt_p.tile([P, DT, S], bf16, tag="xt", name="x0t")
            nc.sync.dma_start(out=x0T, in_=h_x0T[:])
            bq1s = const.tile([P, DT], f32)
            bk1s = const.tile([P, DT], f32)
            bq2s = const.tile([P, DT], f32)
            bk2s = const.tile([P, DT], f32)
            for t, h in ((bq1s, h_bq1), (bk1s, h_bk1), (bq2s, h_bq2),
                         (bk2s, h_bk2)):
                nc.gpsimd.dma_start(out=t, in_=h[:])

            def chunk_dmas(h_src, kt, n, engs=(nc.sync, nc.scalar)):
                tiles = []
                for m in range(n):
                    w_m = wqm_p.tile([P, kt, P], bf16, tag="wqm",
                                     name=f"wc{m}")
                    engs[m % len(engs)].dma_start(out=w_m, in_=h_src[m])
                    tiles.append(w_m)
                return tiles

            def proj_chunked(w_tiles, b_sb, rhsT, name, kt=DT):
                """out[P, DT, S] bf16 = (W.T @ x.T) + b, d-partition."""
                o = qk_p.tile([P, DT, S], bf16, tag="qk", name=name)
                for m in range(DT):
                    pm = ps.tile([P, 512], f32, tag="ps", name="pm")
                    for k in range(kt):
                        nc.tensor.matmul(pm,
                                         lhsT=w_tiles[m][:, k, :],
                                         rhs=rhsT[:, k, :],
                                         start=(k == 0), stop=(k == kt - 1))
                    nc.scalar.activation(out=o[:, m, :], in_=pm,
                                         func=ACT_F.Identity,
                                         bias=b_sb[:, m:m + 1], scale=1.0)
                return o

            def proj_T_into(o, w_sb, b_sb, rhsT, c0, c1, kt=DT):
                w = c1 - c0
                for m in range(DT):
                    pm = ps.tile([P, 512], f32, tag="ps", name="pm")
                    for k in range(kt):
                        nc.tensor.matmul(pm[:, :w],
                                         lhsT=w_sb[:, k, m * P:(m + 1) * P],
                                         rhs=rhsT[:, k, c0:c1],
                                         start=(k == 0), stop=(k == kt - 1))
                    nc.scalar.activation(out=o[:, m, c0:c1], in_=pm[:, :w],
                                         func=ACT_F.Identity,
                                         bias=b_sb[:, m:m + 1], scale=1.0)

            wq1c = chunk_dmas(h_wq1, DT, DT)
            QT = proj_chunked(wq1c, bq1s, x0T, "qt")

            # gpsimd-gated DMAs: issue only once QT compute is underway so
            # they don't steal HBM bandwidth from the critical startup path
            gate_t = const.tile([P, 1], bf16)
            nc.gpsimd.tensor_copy(out=gate_t, in_=QT[:, 0, 0:1])
            img_sb = const.tile([P, DIT, NI], bf16)
            nc.gpsimd.dma_start(out=img_sb, in_=h_img[:])
            x0b = xb_p.tile([P, ST, D], bf16, tag="xb", name="x0b")
            nc.gpsimd.dma_start(out=x0b, in_=h_x0b[:])
            g1b = const.tile([P, D], f32)
            bv2b = const.tile([P, D], f32)
            for t, h in ((g1b, h_g1), (bv2b, h_bv2)):
                nc.gpsimd.dma_start(out=t, in_=bcast(h, D))
            wv2_sb = wv2_p.tile([P, DIT, D], bf16, tag="wv2")
            nc.gpsimd.dma_start(out=wv2_sb, in_=h_wv2[:])

            # wv1 fires from the scalar stream after QT's acts (~21us)
            wv1_sb = wts_p.tile([P, DT, D], bf16, tag="wts")
            nc.scalar.dma_start(out=wv1_sb, in_=h_wv1[:])

            wk1c = chunk_dmas(h_wk1, DT, DT)
            KT = proj_chunked(wk1c, bk1s, x0T, "kt")
            wk2c = chunk_dmas(h_wk2, DIT, DT, engs=(nc.gpsimd,))

            # value projection (bv1 folded into x0b host-side; attention rows
            # are convex combinations so the V-bias passes through unchanged)
            Vt = v_p.tile([P, ST, D], bf16, tag="v")
            for a in range(ST):
                for nh in range(2):
                    pm = ps.tile([P, 512], f32, tag="ps")
                    for k in range(DT):
                        nc.tensor.matmul(
                            pm, lhsT=x0T[:, k, a * P:(a + 1) * P],
                            rhs=wv1_sb[:, k, nh * 512:(nh + 1) * 512],
                            start=(k == 0), stop=(k == DT - 1))
                    nc.scalar.copy(out=Vt[:, a, nh * 512:(nh + 1) * 512],
                                   in_=pm)
                if a == 1:
                    # wq2 issued mid-Vt from the scalar stream (needed ~30us on)
                    wq2_sb = wts_p.tile([P, DT, D], bf16, tag="wts")
                    nc.scalar.dma_start(out=wq2_sb, in_=h_wq2[:])

            # ---- causal self-attention: scores + softmax (all qt) ----
            Pbs = []
            rinv1 = stat_p.tile([P, ST], f32, tag="rinv")
            for qt in range(ST):
                width = (qt + 1) * P
                pm = ps.tile([P, 512], f32, tag="ps")
                for k in range(DT):
                    nc.tensor.matmul(pm[:, :width],
                                     lhsT=QT[:, k, qt * P:(qt + 1) * P],
                                     rhs=KT[:, k, :width],
                                     start=(k == 0), stop=(k == DT - 1))
                # mask the diagonal block in place (PSUM RMW)
                nc.vector.tensor_tensor(out=pm[:, qt * P:width],
                                        in0=pm[:, qt * P:width], in1=trimask,
                                        op=ALU.add)
                nmax = stat_p.tile([P, 1], f32, tag="nmax")
                nc.vector.reduce_max(nmax, pm[:, :width], axis=X, negate=True)
                Pb = pb_p.tile([P, 512], bf16, tag="pb", name=f"pb{qt}")
                rsum = stat_p.tile([P, 1], f32, tag="rsum")
                nc.scalar.activation(out=Pb[:, :width], in_=pm[:, :width],
                                     func=ACT_F.Exp, bias=nmax, scale=1.0,
                                     accum_out=rsum)
                nc.vector.reciprocal(out=rinv1[:, qt:qt + 1], in_=rsum)
                Pbs.append(Pb)

            # ---- cross-attn K2 (fills the softmax1 pipeline shadow) ----
            K2T = k2t_p.tile([P, DT, NI_PAD], bf16, tag="k2t")
            for m in range(DT):
                pm = ps.tile([P, 512], f32, tag="ps")
                for k in range(DIT):
                    nc.tensor.matmul(pm[:, :NI],
                                     lhsT=wk2c[m][:, k, :],
                                     rhs=img_sb[:, k, :],
                                     start=(k == 0), stop=(k == DIT - 1))
                nc.scalar.activation(out=K2T[:, m, :NI], in_=pm[:, :NI],
                                     func=ACT_F.Identity,
                                     bias=bk2s[:, m:m + 1], scale=1.0)

            def layernorm(xpre, out_sl, gb):
                """xpre [P, D] f32 -> out_sl [P, D] bf16.

                Writes the normalized rows times gb (or raw normalized rows if
                gb is None — affine folded into the consumers)."""
                stats = stat_p.tile([P, 2, 6], f32, tag="bnst")
                for sg in range(2):
                    nc.vector.bn_stats(out=stats[:, sg, :],
                                       in_=xpre[:, sg * 512:(sg + 1) * 512])
                mv = stat_p.tile([P, 2], f32, tag="bnmv")
                nc.vector.bn_aggr(out=mv, in_=stats)
                rstd = stat_p.tile([P, 1], f32, tag="rstd")
                nc.scalar.activation(out=rstd, in_=mv[:, 1:2], func=ACT_F.Sqrt,
                                     bias=epst, scale=1.0)
                nc.vector.reciprocal(out=rstd, in_=rstd)
                nmr = stat_p.tile([P, 1], f32, tag="nmr")
                nc.vector.scalar_tensor_tensor(
                    out=nmr, in0=mv[:, 0:1], scalar=rstd, in1=neg1,
                    op0=ALU.mult, op1=ALU.mult)
                if gb is None:
                    nc.scalar.activation(out=out_sl, in_=xpre,
                                         func=ACT_F.Identity,
                                         bias=nmr, scale=rstd)
                else:
                    nc.scalar.activation(out=xpre, in_=xpre,
                                         func=ACT_F.Identity,
                                         bias=nmr, scale=rstd)
                    nc.gpsimd.tensor_tensor(out=out_sl, in0=xpre,
                                            in1=gb, op=ALU.mult)

            # ---- AV1 + residual + LN1 per qt ----
            PT = pt_p.tile([P, ST, S], bf16, tag="pt")
            x1b = xb_p.tile([P, ST, D], bf16, tag="xb", name="x1b")
            # cross-attn V2 blocks are interleaved below as PE filler while
            # the VEC-bound AV1/LN1 chain drains
            V2t = v_p.tile([P, NIT, D], bf16, tag="v")
            nc.gpsimd.memset(V2t, 0.0)

            def v2_block(a, nh):
                pa = P if a == 0 else NI - P
                pm = ps.tile([P, 512], f32, tag="ps")
                for k in range(DIT):
                    nc.tensor.matmul(
                        pm[:pa, :], lhsT=img_sb[:, k, a * P:a * P + pa],
                        rhs=wv2_sb[:, k, nh * 512:(nh + 1) * 512],
                        start=(k == 0), stop=(k == DIT - 1))
                nc.vector.tensor_tensor(
                    out=V2t[:pa, a, nh * 512:(nh + 1) * 512], in0=pm[:pa, :],
                    in1=bv2b[:pa, nh * 512:(nh + 1) * 512], op=ALU.add)

            for qt in range(ST):
                for kt in range(qt + 1):
                    tp = ps.tile([P, 512], bf16, tag="ps", name="tp")
                    nc.tensor.transpose(out=tp[:, :P],
                                        in_=Pbs[qt][:, kt * P:(kt + 1) * P],
                                        identity=ident)
                    nc.vector.tensor_copy(out=PT[:, kt, qt * P:(qt + 1) * P],
                                          in_=tp[:, :P])
                xpre = xpre_p.tile([P, D], f32, tag="xpre")
                for nh in range(2):
                    sl = slice(nh * 512, (nh + 1) * 512)
                    pm = ps.tile([P, 512], f32, tag="ps")
                    for kt in range(qt + 1):
                        nc.tensor.matmul(pm, lhsT=PT[:, kt, qt * P:(qt + 1) * P],
                                         rhs=Vt[:, kt, nh * 512:(nh + 1) * 512],
                                         start=(kt == 0), stop=(kt == qt))
                    # residual split: SCA rescale, then GPS add (keeps VEC free)
                    nc.scalar.activation(out=xpre[:, sl], in_=pm,
                                         func=ACT_F.Identity,
                                         scale=rinv1[:, qt:qt + 1])
                    nc.gpsimd.tensor_tensor(out=xpre[:, sl], in0=xpre[:, sl],
                                            in1=x0b[:, qt, sl], op=ALU.add)
                v2_block(qt % NIT, qt // NIT)  # PE filler during stt/LN1
                layernorm(xpre, x1b[:, qt, :], g1b)

            # ---- layer 2, pipelined in qt-pair halves ----
            def transpose_cols(dst, src_b, a_list, tag):
                """transpose x[P, a, db*P:(db+1)*P] -> dst[:, db, a*P:(a+1)*P]."""
                for a in a_list:
                    for db in range(DT):
                        tp = ps.tile([P, 512], bf16, tag="ps", name=tag)
                        nc.tensor.transpose(
                            out=tp[:, :P],
                            in_=src_b[:, a, db * P:(db + 1) * P],
                            identity=ident)
                        nc.scalar.copy(
                            out=dst[:, db, a * P:(a + 1) * P], in_=tp[:, :P])

            x1T = xt_p.tile([P, DT, S], bf16, tag="xt", name="x1t")
            Q2T = qk_p.tile([P, DT, S], bf16, tag="qk", name="q2t")
            P2bs = []
            rinv2 = stat_p.tile([P, ST], f32, tag="rinv2")

            def scores2_softmax(qt):
                pm = ps.tile([P, 512], f32, tag="ps")
                for k in range(DT):
                    nc.tensor.matmul(pm[:, :NI],
                                     lhsT=Q2T[:, k, qt * P:(qt + 1) * P],
                                     rhs=K2T[:, k, :NI],
                                     start=(k == 0), stop=(k == DT - 1))
                nmax = stat_p.tile([P, 1], f32, tag="nmax")
                nc.vector.reduce_max(nmax, pm[:, :NI], axis=X, negate=True)
                P2b = pb_p.tile([P, NI_PAD], bf16, tag="pb2", name=f"p2b{qt}")
                nc.gpsimd.memset(P2b[:, NI:], 0.0)
                rsum = stat_p.tile([P, 1], f32, tag="rsum")
                nc.scalar.activation(out=P2b[:, :NI], in_=pm[:, :NI],
                                     func=ACT_F.Exp, bias=nmax, scale=1.0,
                                     accum_out=rsum)
                nc.vector.reciprocal(out=rinv2[:, qt:qt + 1], in_=rsum)
                P2bs.append(P2b)

            # first half: qt 0,1
            transpose_cols(x1T, x1b, (0, 1), "x1t_tp")
            proj_T_into(Q2T, wq2_sb, bq2s, x1T, 0, 256)
            scores2_softmax(0)
            scores2_softmax(1)
            # second half: qt 2,3
            transpose_cols(x1T, x1b, (2, 3), "x1t_tp")
            proj_T_into(Q2T, wq2_sb, bq2s, x1T, 256, 512)
            scores2_softmax(2)
            scores2_softmax(3)

            # ---- AV2 + residual + LN2 per qt (normalized out; affine folded
            # into Wp/bp) ----
            PT2 = pt_p.tile([P, NIT, S], bf16, tag="pt2")
            x2b = xb_p.tile([P, ST, D], bf16, tag="xb", name="x2b")

            def av2_ln2(qt):
                for kt in range(NIT):
                    tp = ps.tile([P, 512], bf16, tag="ps", name="tp2")
                    nc.tensor.transpose(out=tp[:, :P],
                                        in_=P2bs[qt][:, kt * P:(kt + 1) * P],
                                        identity=ident)
                    nc.vector.tensor_copy(out=PT2[:, kt, qt * P:(qt + 1) * P],
                                          in_=tp[:, :P])
                xpre = xpre_p.tile([P, D], f32, tag="xpre")
                for nh in range(2):
                    sl = slice(nh * 512, (nh + 1) * 512)
                    pm = ps.tile([P, 512], f32, tag="ps")
                    for kt in range(NIT):
                        nc.tensor.matmul(pm, lhsT=PT2[:, kt, qt * P:(qt + 1) * P],
                                         rhs=V2t[:, kt, nh * 512:(nh + 1) * 512],
                                         start=(kt == 0), stop=(kt == NIT - 1))
                    nc.scalar.activation(out=xpre[:, sl], in_=pm,
                                         func=ACT_F.Identity,
                                         scale=rinv2[:, qt:qt + 1])
                    nc.gpsimd.tensor_tensor(out=xpre[:, sl], in0=xpre[:, sl],
                                            in1=x1b[:, qt, sl], op=ALU.add)
                layernorm(xpre, x2b[:, qt, :], None)

            x2T = xt_p.tile([P, DT, S], bf16, tag="xt", name="x2t")

            for qt in range(ST):
                av2_ln2(qt)
            transpose_cols(x2T, x2b, (0, 1), "x2t_tp")

            # ---- vocab projection, streamed in CN-column chunks ----
            def vocab_chunks(chunks, qts, dma_par):
                """Process wp[chunks] x qts; chunks must align to GRP groups."""
                for gi in range(0, len(chunks), GRP):
                    g = chunks[gi] // GRP
                    bp_bc = bp_p.tile([P, GRP * CN], bf16, tag="bp")
                    nc.gpsimd.dma_start(out=bp_bc,
                                        in_=bcast(h_bp, GRP * CN,
                                                  offset=g * GRP * CN))
                    osb = {q: osb_p.tile([P, GRP * CN], bf16, tag="osb",
                                         name=f"osb_{g}_{q}")
                           for q in qts}
                    for cc in range(GRP):
                        c = chunks[gi + cc]
                        wp_sb = wp_p.tile([P, DT, CN], bf16, tag="wp")
                        dma_eng = nc.sync if (c + dma_par) % 2 == 0 else nc.scalar
                        dma_eng.dma_start(out=wp_sb, in_=h_wp[c])
                        for qt in qts:
                            pm = ps.tile([P, 512], f32, tag="ps")
                            for k in range(DT):
                                nc.tensor.matmul(
                                    pm, lhsT=x2T[:, k, qt * P:(qt + 1) * P],
                                    rhs=wp_sb[:, k, :],
                                    start=(k == 0), stop=(k == DT - 1))
                            nc.vector.tensor_tensor(
                                out=osb[qt][:, cc * CN:(cc + 1) * CN], in0=pm,
                                in1=bp_bc[:, cc * CN:(cc + 1) * CN], op=ALU.add)
                            if cc == GRP - 1:
                                # fire each strip as soon as it completes
                                out_eng = nc.sync if qt < 2 else nc.scalar
                                out_eng.dma_start(
                                    out=h_out[qt * P:(qt + 1) * P,
                                              g * GRP * CN:(g + 1) * GRP * CN],
                                    in_=osb[qt])

            # early pass: first NE chunks for qt {0,1} while LN2(2,3) drains
            vocab_chunks(list(range(NE)), (0, 1), 0)
            transpose_cols(x2T, x2b, (2, 3), "x2t_tp")
            # late pass for those chunks' qt {2,3} (re-streamed), then the rest
            vocab_chunks(list(range(NE)), (2, 3), 1)
            vocab_chunks(list(range(NE, NCHUNK)), (0, 1, 2, 3), 0)

    nc.compile()
    return nc


def _tile_sq(w, kt):
    """[K, N] -> [128, K//128, N] contiguous."""
    k, n = w.shape
    assert k == kt * P
    return np.ascontiguousarray(
        w.reshape(kt, P, n).transpose(1, 0, 2)).astype(BF16)


def _pos_enc():
    posn = np.arange(S)[:, None].astype(np.float32)
    i = np.arange(0, D, 2).astype(np.float32)
    ang = posn / np.power(10000.0, i / D)
    pos = np.zeros((S, D), dtype=np.float32)
    pos[:, 0::2] = np.sin(ang)
    pos[:, 1::2] = np.cos(ang)
    return pos


def _prep_inputs(inputs):
    g = lambda name: np.asarray(inputs[name], dtype=np.float32)
    tokens = np.asarray(inputs["tokens"]).astype(np.int64)
    img = g("img_emb")
    table = g("emb_table")
    pos = _pos_enc()

    b1 = g("b1")
    g2 = g("g2")
    b2 = g("b2")
    wp = g("Wp") * g2[:, None]          # fold LN2 gamma
    wp_pad = np.zeros((D, VP), dtype=np.float32)
    wp_pad[:, :V] = wp
    wp_t = np.ascontiguousarray(
        wp_pad.reshape(DT, P, NCHUNK, CN).transpose(2, 1, 0, 3)).astype(BF16)
    bp_pad = np.zeros((VP,), dtype=np.float32)
    bp_pad[:V] = g("bp") + b2 @ g("Wp")  # fold LN2 beta
    bp_pad = bp_pad.astype(BF16)

    def bias_tiled(b):
        return np.ascontiguousarray(b.reshape(DT, P).T).astype(np.float32)

    def chunk_m(tiled):
        """[P, kt, D] -> [DT][P, kt, 128] contiguous chunks of output cols."""
        return np.ascontiguousarray(
            np.stack([tiled[:, :, m * P:(m + 1) * P] for m in range(DT)]))

    shared = {
        "wq1c": chunk_m(_tile_sq(g("Wq1") * SCALE, DT)),
        "wk1c": chunk_m(_tile_sq(g("Wk1"), DT)),
        "wv1": _tile_sq(g("Wv1"), DT),
        "wq2": _tile_sq(g("Wq2") * SCALE, DT),
        "wk2c": chunk_m(_tile_sq(g("Wk2"), DIT)),
        "wv2": _tile_sq(g("Wv2"), DIT),
        "wp": wp_t,
        "bq1": bias_tiled(g("bq1") * SCALE),
        "bk1": bias_tiled(g("bk1")),
        # fold LN1 beta into the cross-attn query bias and value bias
        "bq2": bias_tiled((g("bq2") + b1 @ g("Wq2")) * SCALE),
        "bk2": bias_tiled(g("bk2")),
        "bv2": g("bv2") + b1,
        "g1": g("g1"),
        "bp": bp_pad,
    }
    in_maps = []
    for c in range(N_CORES):
        m = dict(shared)
        x0 = table[tokens[c]] + pos                      # [S, D] f32
        # bv1 folded into the self-attn residual (A1 rows sum to 1)
        x0r = x0 + g("bv1")
        x0b = np.ascontiguousarray(
            x0r.reshape(ST, P, D).transpose(1, 0, 2)).astype(BF16)
        x0T = np.ascontiguousarray(
            x0.T.reshape(DT, P, S).transpose(1, 0, 2)).astype(BF16)
        m["x0b"] = x0b
        m["x0t"] = x0T
        m["img_t"] = np.ascontiguousarray(
            img[c].T.reshape(DIT, P, NI).transpose(1, 0, 2)).astype(BF16)
        in_maps.append(m)
    return in_maps


def _ensure_axon_hooks():
    """bass_utils imports antenv.axon_hooks when BASS_TRACE is set; stub it
    if the module is absent so tracing degrades instead of crashing."""
    try:
        import antenv.axon_hooks  # noqa: F401
    except ImportError:
        import types
        mod = types.ModuleType("antenv.axon_hooks")
        mod.get_axon_ntff_profile_hook = lambda: None
        mod.set_axon_ntff_profile_hook = lambda h: None
        sys.modules["antenv.axon_hooks"] = mod


def kernel(**inputs):
    global LAST_RESULTS
    _ensure_axon_hooks()
    from concourse.bass_utils import run_bass_kernel_spmd

    if "nc" not in _CACHE:
        _CACHE["nc"] = _build_program()
    nc = _CACHE["nc"]

    in_maps = _prep_inputs(inputs)
    res = run_bass_kernel_spmd(nc, in_maps, core_ids=list(range(N_CORES)))
    LAST_RESULTS = res
    out = np.stack([res.results[c]["out"][:, :V].astype(np.float32)
                    for c in range(N_CORES)])
    return out


# revision 41
# speedup vs baseline: 1.0061x; 1.0061x over previous
"""Trainium2 Bass kernel for an 8-batch image-conditioned decoder layer.

Strategy: pure data-parallel over the batch — core c computes batch element c
end-to-end (causal self-attention, cross-attention over the image tokens, both
layernorms, vocab projection). No collectives.

v2 schedule notes:
- Embedding gather + positional encoding are host-prepped (pure data movement);
  the device receives x0 in both seq-partition and d-partition layouts, so the
  TensorEngine starts on real work as soon as the first weights land.
- Dummy warmup matmuls run during the initial DMA window so the PE HAM clock
  gate is at full rate when QT starts.
- b1 is folded into bq2/bv2, and g2/b2 into Wp/bp (host-side), which trims the
  layernorm critical path; layer-2 LN output needs no affine at all.
- Layer 2 runs per-qt-pipelined (Q2T in two 256-col halves) and the first 4
  vocab chunks are computed early for qt {0,1} to keep the PE fed while the
  tail of attention drains; those chunks are re-streamed later for qt {2,3}.
- PSUM->SBUF copies, value-bias adds, and half of the element-wise work run on
  GpSimd; output DMAs go on the vector/gpsimd queues so sync/scalar stay free
  for Wp streaming.

All matmuls run in bf16 with fp32 PSUM accumulation.
"""

import os
import sys

for _p in ("/opt/trn_rl_repo", "/root/.axon_site/_ro/trn_rl_repo"):
    if os.path.isdir(_p) and _p not in sys.path:
        sys.path.append(_p)

import numpy as np
import ml_dtypes

BF16 = ml_dtypes.bfloat16

# Problem dims (hardcoded per spec)
V, D, DI, S, B, NI = 32000, 1024, 768, 512, 8, 197
EPS = 1e-5
P = 128
ST = S // P          # 4 seq tiles
DT = D // P          # 8 model-dim tiles
DIT = DI // P        # 6 image-dim tiles
NIT = 2              # image tokens: 197 -> 2 partition tiles (128 + 69)
NI_PAD = 256
VP = 32768           # vocab padded to 64 chunks of 512
CN = 512             # vocab chunk width
NCHUNK = VP // CN    # 64
GRP = 2              # chunks per output strip
NGRP = NCHUNK // GRP
NE = 4               # chunks computed early for qt {0,1} (re-streamed later)
N_CORES = 8
SCALE = 1.0 / float(np.sqrt(np.float32(D)))

_CACHE = {}
LAST_RESULTS = None


def _build_program():
    import concourse.bacc as bacc
    import concourse.bass as bass
    import concourse.mybir as mybir
    from concourse.masks import make_identity
    from concourse.tile import TileContext

    f32 = mybir.dt.float32
    bf16 = mybir.dt.bfloat16
    X = mybir.AxisListType.X
    ALU = mybir.AluOpType
    ACT_F = mybir.ActivationFunctionType

    nc = bacc.Bacc("TRN2", target_bir_lowering=False, debug=False,
                   num_devices=N_CORES)

    # ---- I/O ----
    h_x0b = nc.dram_tensor("x0b", [P, ST, D], bf16, kind="ExternalInput")
    h_x0T = nc.dram_tensor("x0t", [P, DT, S], bf16, kind="ExternalInput")
    h_img = nc.dram_tensor("img_t", [P, DIT, NI], bf16, kind="ExternalInput")
    h_wq1 = nc.dram_tensor("wq1c", [DT, P, DT, P], bf16, kind="ExternalInput")
    h_wk1 = nc.dram_tensor("wk1c", [DT, P, DT, P], bf16, kind="ExternalInput")
    h_wv1 = nc.dram_tensor("wv1", [P, DT, D], bf16, kind="ExternalInput")
    h_wq2 = nc.dram_tensor("wq2", [P, DT, D], bf16, kind="ExternalInput")
    h_wk2 = nc.dram_tensor("wk2c", [DT, P, DIT, P], bf16, kind="ExternalInput")
    h_wv2 = nc.dram_tensor("wv2", [P, DIT, D], bf16, kind="ExternalInput")
    h_wp = nc.dram_tensor("wp", [NCHUNK, P, DT, CN], bf16, kind="ExternalInput")
    h_bq1 = nc.dram_tensor("bq1", [P, DT], f32, kind="ExternalInput")
    h_bk1 = nc.dram_tensor("bk1", [P, DT], f32, kind="ExternalInput")
    h_bq2 = nc.dram_tensor("bq2", [P, DT], f32, kind="ExternalInput")
    h_bk2 = nc.dram_tensor("bk2", [P, DT], f32, kind="ExternalInput")
    h_bv2 = nc.dram_tensor("bv2", [D], f32, kind="ExternalInput")
    h_g1 = nc.dram_tensor("g1", [D], f32, kind="ExternalInput")
    h_bp = nc.dram_tensor("bp", [VP], bf16, kind="ExternalInput")
    h_out = nc.dram_tensor("out", [S, VP], bf16, kind="ExternalOutput")

    def bcast(handle, n, offset=0):
        ap = handle[:]
        return bass.AP(tensor=ap.tensor, offset=offset, ap=[[0, P], [1, n]])

    with TileContext(nc) as tc:
        import contextlib
        ctx = contextlib.ExitStack()
        with ctx:
            const = ctx.enter_context(tc.tile_pool(name="const", bufs=1))
            xb_p = ctx.enter_context(tc.tile_pool(name="xb", bufs=3))
            xt_p = ctx.enter_context(tc.tile_pool(name="xt", bufs=2))
            qk_p = ctx.enter_context(tc.tile_pool(name="qk", bufs=2))
            v_p = ctx.enter_context(tc.tile_pool(name="vp", bufs=2))
            k2t_p = ctx.enter_context(tc.tile_pool(name="k2t", bufs=1))
            pb_p = ctx.enter_context(tc.tile_pool(name="pb", bufs=4))
            pt_p = ctx.enter_context(tc.tile_pool(name="pt", bufs=1))
            xpre_p = ctx.enter_context(tc.tile_pool(name="xpre", bufs=2))
            stat_p = ctx.enter_context(tc.tile_pool(name="stat", bufs=4))
            wts_p = ctx.enter_context(tc.tile_pool(name="wts", bufs=2))
            wv2_p = ctx.enter_context(tc.tile_pool(name="wv2p", bufs=1))
            wqm_p = ctx.enter_context(tc.tile_pool(name="wqm", bufs=3))
            wp_p = ctx.enter_context(tc.tile_pool(name="wpp", bufs=3))
            bp_p = ctx.enter_context(tc.tile_pool(name="bpp", bufs=2))
            osb_p = ctx.enter_context(tc.tile_pool(name="osb", bufs=6))
            ps = ctx.enter_context(tc.tile_pool(name="ps", bufs=8, space="PSUM"))

            # ---- constants / warmup ----
            ident = const.tile([P, P], bf16)
            make_identity(nc, ident)
            trimask = const.tile([P, P], f32)
            nc.gpsimd.memset(trimask, 0.0)
            nc.gpsimd.affine_select(
                out=trimask, in_=trimask, compare_op=ALU.is_ge, fill=-1e10,
                base=0, pattern=[[-1, P]], channel_multiplier=1)
            warm_src = const.tile([P, 256], bf16)
            nc.vector.memset(warm_src, 0.0)
            epst = const.tile([P, 1], f32)
            nc.vector.memset(epst, EPS)
            neg1 = const.tile([P, 1], f32)
            nc.vector.memset(neg1, -1.0)

            # HAM warmup: keep the PE busy while the first weights stream in
            for w in range(28):
                pw = ps.tile([P, 512], f32, tag="ps", name=f"warm{w}")
                nc.tensor.matmul(pw[:, :256], lhsT=ident, rhs=warm_src,
                                 start=True, stop=True)

            # ---- early DMAs (per-queue order == emission order; the wqm
            # chunk-ring slot waits naturally pace later transfers behind
            # earlier consumption, keeping HBM free for the critical path) ----
            x0T = xt_p.tile([P, DT, S], bf16, tag="xt", name="x0t")
            nc.sync.dma_start(out=x0T, in_=h_x0T[:])
            bq1s = const.tile([P, DT], f32)
            bk1s = const.tile([P, DT], f32)
            bq2s = const.tile([P, DT], f32)
            bk2s = const.tile([P, DT], f32)
            for t, h in ((bq1s, h_bq1), (bk1s, h_bk1), (bq2s, h_bq2),
                         (bk2s, h_bk2)):
                nc.gpsimd.dma_start(out=t, in_=h[:])

            def chunk_dmas(h_src, kt, n, engs=(nc.sync, nc.scalar)):
                tiles = []
                for m in range(n):
                    w_m = wqm_p.tile([P, kt, P], bf16, tag="wqm",
                                     name=f"wc{m}")
                    engs[m % len(engs)].dma_start(out=w_m, in_=h_src[m])
                    tiles.append(w_m)
                return tiles

            def proj_stream(h_src, b_sb, rhsT, name, kt=DT, lead=3):
                """out[P, DT, S] bf16 = (W.T @ x.T) + b, d-partition.

                Weight chunks stream just-in-time from the scalar queue, paced
                by the act of the previous m-group, so early HBM bandwidth
                stays free for the critical path."""
                o = qk_p.tile([P, DT, S], bf16, tag="qk", name=name)
                tiles = {}

                def load(m):
                    if m < DT:
                        t = wqm_p.tile([P, kt, P], bf16, tag="wqm",
                                       name=f"{name}c{m}")
                        nc.scalar.dma_start(out=t, in_=h_src[m])
                        tiles[m] = t

                for m in range(lead):
                    load(m)
                for m in range(DT):
                    pm = ps.tile([P, 512], f32, tag="ps", name="pm")
                    for k in range(kt):
                        nc.tensor.matmul(pm,
                                         lhsT=tiles[m][:, k, :],
                                         rhs=rhsT[:, k, :],
                                         start=(k == 0), stop=(k == kt - 1))
                    load(m + lead)
                    nc.scalar.activation(out=o[:, m, :], in_=pm,
                                         func=ACT_F.Identity,
                                         bias=b_sb[:, m:m + 1], scale=1.0)
                return o

            def proj_T_into(o, w_sb, b_sb, rhsT, c0, c1, kt=DT):
                w = c1 - c0
                for m in range(DT):
                    pm = ps.tile([P, 512], f32, tag="ps", name="pm")
                    for k in range(kt):
                        nc.tensor.matmul(pm[:, :w],
                                         lhsT=w_sb[:, k, m * P:(m + 1) * P],
                                         rhs=rhsT[:, k, c0:c1],
                                         start=(k == 0), stop=(k == kt - 1))
                    nc.scalar.activation(out=o[:, m, c0:c1], in_=pm[:, :w],
                                         func=ACT_F.Identity,
                                         bias=b_sb[:, m:m + 1], scale=1.0)

            QT = proj_stream(h_wq1, bq1s, x0T, "qt")

            # gpsimd-gated DMAs: issue only once QT compute is underway so
            # they don't steal HBM bandwidth from the critical startup path
            gate_t = const.tile([P, 1], bf16)
            nc.gpsimd.tensor_copy(out=gate_t, in_=QT[:, 0, 0:1])
            img_sb = const.tile([P, DIT, NI], bf16)
            nc.gpsimd.dma_start(out=img_sb, in_=h_img[:])
            x0b = xb_p.tile([P, ST, D], bf16, tag="xb", name="x0b")
            nc.gpsimd.dma_start(out=x0b, in_=h_x0b[:])
            g1b = const.tile([P, D], f32)
            bv2b = const.tile([P, D], f32)
            for t, h in ((g1b, h_g1), (bv2b, h_bv2)):
                nc.gpsimd.dma_start(out=t, in_=bcast(h, D))
            wv2_sb = wv2_p.tile([P, DIT, D], bf16, tag="wv2")
            nc.gpsimd.dma_start(out=wv2_sb, in_=h_wv2[:])

            # wv1 fires from the scalar stream after QT's acts (~21us)
            wv1_sb = wts_p.tile([P, DT, D], bf16, tag="wts")
            nc.scalar.dma_start(out=wv1_sb, in_=h_wv1[:])

            KT = proj_stream(h_wk1, bk1s, x0T, "kt")
            wk2c = chunk_dmas(h_wk2, DIT, DT, engs=(nc.gpsimd,))

            # value projection (bv1 folded into x0b host-side; attention rows
            # are convex combinations so the V-bias passes through unchanged)
            Vt = v_p.tile([P, ST, D], bf16, tag="v")
            for a in range(ST):
                for nh in range(2):
                    pm = ps.tile([P, 512], f32, tag="ps")
                    for k in range(DT):
                        nc.tensor.matmul(
                            pm, lhsT=x0T[:, k, a * P:(a + 1) * P],
                            rhs=wv1_sb[:, k, nh * 512:(nh + 1) * 512],
                            start=(k == 0), stop=(k == DT - 1))
                    nc.scalar.copy(out=Vt[:, a, nh * 512:(nh + 1) * 512],
                                   in_=pm)
                if a == 1:
                    # wq2 issued mid-Vt from the scalar stream (needed ~30us on)
                    wq2_sb = wts_p.tile([P, DT, D], bf16, tag="wts")
                    nc.scalar.dma_start(out=wq2_sb, in_=h_wq2[:])

            # ---- causal self-attention: scores + softmax (all qt) ----
            Pbs = []
            rinv1 = stat_p.tile([P, ST], f32, tag="rinv")
            for qt in range(ST):
                width = (qt + 1) * P
                pm = ps.tile([P, 512], f32, tag="ps")
                for k in range(DT):
                    nc.tensor.matmul(pm[:, :width],
                                     lhsT=QT[:, k, qt * P:(qt + 1) * P],
                                     rhs=KT[:, k, :width],
                                     start=(k == 0), stop=(k == DT - 1))
                # mask the diagonal block in place (PSUM RMW)
                nc.vector.tensor_tensor(out=pm[:, qt * P:width],
                                        in0=pm[:, qt * P:width], in1=trimask,
                                        op=ALU.add)
                nmax = stat_p.tile([P, 1], f32, tag="nmax")
                nc.vector.reduce_max(nmax, pm[:, :width], axis=X, negate=True)
                Pb = pb_p.tile([P, 512], bf16, tag="pb", name=f"pb{qt}")
                rsum = stat_p.tile([P, 1], f32, tag="rsum")
                nc.scalar.activation(out=Pb[:, :width], in_=pm[:, :width],
                                     func=ACT_F.Exp, bias=nmax, scale=1.0,
                                     accum_out=rsum)
                nc.vector.reciprocal(out=rinv1[:, qt:qt + 1], in_=rsum)
                Pbs.append(Pb)

            # ---- cross-attn K2 (fills the softmax1 pipeline shadow) ----
            K2T = k2t_p.tile([P, DT, NI_PAD], bf16, tag="k2t")
            for m in range(DT):
                pm = ps.tile([P, 512], f32, tag="ps")
                for k in range(DIT):
                    nc.tensor.matmul(pm[:, :NI],
                                     lhsT=wk2c[m][:, k, :],
                                     rhs=img_sb[:, k, :],
                                     start=(k == 0), stop=(k == DIT - 1))
                nc.scalar.activation(out=K2T[:, m, :NI], in_=pm[:, :NI],
                                     func=ACT_F.Identity,
                                     bias=bk2s[:, m:m + 1], scale=1.0)

            def layernorm(xpre, out_sl, gb):
                """xpre [P, D] f32 -> out_sl [P, D] bf16.

                Writes the normalized rows times gb (or raw normalized rows if
                gb is None — affine folded into the consumers)."""
                stats = stat_p.tile([P, 2, 6], f32, tag="bnst")
                for sg in range(2):
                    nc.vector.bn_stats(out=stats[:, sg, :],
                                       in_=xpre[:, sg * 512:(sg + 1) * 512])
                mv = stat_p.tile([P, 2], f32, tag="bnmv")
                nc.vector.bn_aggr(out=mv, in_=stats)
                rstd = stat_p.tile([P, 1], f32, tag="rstd")
                nc.scalar.activation(out=rstd, in_=mv[:, 1:2], func=ACT_F.Sqrt,
                                     bias=epst, scale=1.0)
                nc.vector.reciprocal(out=rstd, in_=rstd)
                nmr = stat_p.tile([P, 1], f32, tag="nmr")
                nc.vector.scalar_tensor_tensor(
                    out=nmr, in0=mv[:, 0:1], scalar=rstd, in1=neg1,
                    op0=ALU.mult, op1=ALU.mult)
                if gb is None:
                    nc.scalar.activation(out=out_sl, in_=xpre,
                                         func=ACT_F.Identity,
                                         bias=nmr, scale=rstd)
                else:
                    nc.scalar.activation(out=xpre, in_=xpre,
                                         func=ACT_F.Identity,
                                         bias=nmr, scale=rstd)
                    nc.gpsimd.tensor_tensor(out=out_sl, in0=xpre,
                                            in1=gb, op=ALU.mult)

            # ---- AV1 + residual + LN1 per qt ----
            PT = pt_p.tile([P, ST, S], bf16, tag="pt")
            x1b = xb_p.tile([P, ST, D], bf16, tag="xb", name="x1b")
            wp_pre = {}
            bp_pre = {}
            # cross-attn V2 blocks are interleaved below as PE filler while
            # the VEC-bound AV1/LN1 chain drains
            V2t = v_p.tile([P, NIT, D], bf16, tag="v")
            nc.gpsimd.memset(V2t, 0.0)

            def v2_block(a, nh):
                pa = P if a == 0 else NI - P
                pm = ps.tile([P, 512], f32, tag="ps")
                for k in range(DIT):
                    nc.tensor.matmul(
                        pm[:pa, :], lhsT=img_sb[:, k, a * P:a * P + pa],
                        rhs=wv2_sb[:, k, nh * 512:(nh + 1) * 512],
                        start=(k == 0), stop=(k == DIT - 1))
                nc.vector.tensor_tensor(
                    out=V2t[:pa, a, nh * 512:(nh + 1) * 512], in0=pm[:pa, :],
                    in1=bv2b[:pa, nh * 512:(nh + 1) * 512], op=ALU.add)

            for qt in range(ST):
                for kt in range(qt + 1):
                    tp = ps.tile([P, 512], bf16, tag="ps", name="tp")
                    nc.tensor.transpose(out=tp[:, :P],
                                        in_=Pbs[qt][:, kt * P:(kt + 1) * P],
                                        identity=ident)
                    nc.vector.tensor_copy(out=PT[:, kt, qt * P:(qt + 1) * P],
                                          in_=tp[:, :P])
                xpre = xpre_p.tile([P, D], f32, tag="xpre")
                for nh in range(2):
                    sl = slice(nh * 512, (nh + 1) * 512)
                    pm = ps.tile([P, 512], f32, tag="ps")
                    for kt in range(qt + 1):
                        nc.tensor.matmul(pm, lhsT=PT[:, kt, qt * P:(qt + 1) * P],
                                         rhs=Vt[:, kt, nh * 512:(nh + 1) * 512],
                                         start=(kt == 0), stop=(kt == qt))
                    # residual split: SCA rescale, then GPS add (keeps VEC free)
                    nc.scalar.activation(out=xpre[:, sl], in_=pm,
                                         func=ACT_F.Identity,
                                         scale=rinv1[:, qt:qt + 1])
                    nc.gpsimd.tensor_tensor(out=xpre[:, sl], in0=xpre[:, sl],
                                            in1=x0b[:, qt, sl], op=ALU.add)
                v2_block(qt % NIT, qt // NIT)  # PE filler during stt/LN1
                layernorm(xpre, x1b[:, qt, :], g1b)
                if qt == 0:
                    # prefetch the first vocab chunks + bias strip now that
                    # the startup DMA window has drained (gpsimd stream pos)
                    for c in range(3):
                        t = wp_p.tile([P, DT, CN], bf16, tag="wp",
                                      name=f"wp_pre{c}")
                        nc.gpsimd.dma_start(out=t, in_=h_wp[c])
                        wp_pre[c] = t
                    bp0 = bp_p.tile([P, GRP * CN], bf16, tag="bp",
                                    name="bp_pre0")
                    nc.gpsimd.dma_start(out=bp0, in_=bcast(h_bp, GRP * CN))
                    bp_pre[0] = bp0

            # ---- layer 2, pipelined in qt-pair halves ----
            def transpose_cols(dst, src_b, a_list, tag):
                """transpose x[P, a, db*P:(db+1)*P] -> dst[:, db, a*P:(a+1)*P]."""
                for a in a_list:
                    for db in range(DT):
                        tp = ps.tile([P, 512], bf16, tag="ps", name=tag)
                        nc.tensor.transpose(
                            out=tp[:, :P],
                            in_=src_b[:, a, db * P:(db + 1) * P],
                            identity=ident)
                        nc.scalar.copy(
                            out=dst[:, db, a * P:(a + 1) * P], in_=tp[:, :P])

            x1T = xt_p.tile([P, DT, S], bf16, tag="xt", name="x1t")
            Q2T = qk_p.tile([P, DT, S], bf16, tag="qk", name="q2t")
            P2bs = []
            rinv2 = stat_p.tile([P, ST], f32, tag="rinv2")

            def scores2_softmax(qt):
                pm = ps.tile([P, 512], f32, tag="ps")
                for k in range(DT):
                    nc.tensor.matmul(pm[:, :NI],
                                     lhsT=Q2T[:, k, qt * P:(qt + 1) * P],
                                     rhs=K2T[:, k, :NI],
                                     start=(k == 0), stop=(k == DT - 1))
                nmax = stat_p.tile([P, 1], f32, tag="nmax")
                nc.vector.reduce_max(nmax, pm[:, :NI], axis=X, negate=True)
                P2b = pb_p.tile([P, NI_PAD], bf16, tag="pb2", name=f"p2b{qt}")
                nc.gpsimd.memset(P2b[:, NI:], 0.0)
                rsum = stat_p.tile([P, 1], f32, tag="rsum")
                nc.scalar.activation(out=P2b[:, :NI], in_=pm[:, :NI],
                                     func=ACT_F.Exp, bias=nmax, scale=1.0,
                                     accum_out=rsum)
                nc.vector.reciprocal(out=rinv2[:, qt:qt + 1], in_=rsum)
                P2bs.append(P2b)

            # first half: qt 0,1
            transpose_cols(x1T, x1b, (0, 1), "x1t_tp")
            proj_T_into(Q2T, wq2_sb, bq2s, x1T, 0, 256)
            scores2_softmax(0)
            scores2_softmax(1)
            # second half: qt 2,3
            transpose_cols(x1T, x1b, (2, 3), "x1t_tp")
            proj_T_into(Q2T, wq2_sb, bq2s, x1T, 256, 512)
            scores2_softmax(2)
            scores2_softmax(3)

            # ---- AV2 + residual + LN2 per qt (normalized out; affine folded
            # into Wp/bp) ----
            PT2 = pt_p.tile([P, NIT, S], bf16, tag="pt2")
            x2b = xb_p.tile([P, ST, D], bf16, tag="xb", name="x2b")

            def av2_ln2(qt):
                for kt in range(NIT):
                    tp = ps.tile([P, 512], bf16, tag="ps", name="tp2")
                    nc.tensor.transpose(out=tp[:, :P],
                                        in_=P2bs[qt][:, kt * P:(kt + 1) * P],
                                        identity=ident)
                    nc.vector.tensor_copy(out=PT2[:, kt, qt * P:(qt + 1) * P],
                                          in_=tp[:, :P])
                xpre = xpre_p.tile([P, D], f32, tag="xpre")
                for nh in range(2):
                    sl = slice(nh * 512, (nh + 1) * 512)
                    pm = ps.tile([P, 512], f32, tag="ps")
                    for kt in range(NIT):
                        nc.tensor.matmul(pm, lhsT=PT2[:, kt, qt * P:(qt + 1) * P],
                                         rhs=V2t[:, kt, nh * 512:(nh + 1) * 512],
                                         start=(kt == 0), stop=(kt == NIT - 1))
                    nc.scalar.activation(out=xpre[:, sl], in_=pm,
                                         func=ACT_F.Identity,
                                         scale=rinv2[:, qt:qt + 1])
                    nc.gpsimd.tensor_tensor(out=xpre[:, sl], in0=xpre[:, sl],
                                            in1=x1b[:, qt, sl], op=ALU.add)
                layernorm(xpre, x2b[:, qt, :], None)

            x2T = xt_p.tile([P, DT, S], bf16, tag="xt", name="x2t")

            for qt in range(ST):
                av2_ln2(qt)
            transpose_cols(x2T, x2b, (0, 1), "x2t_tp")

            # ---- vocab projection, streamed in CN-column chunks ----
            def vocab_chunks(chunks, qts, dma_par, pre=()):
                """Process wp[chunks] x qts; chunks must align to GRP groups."""
                for gi in range(0, len(chunks), GRP):
                    g = chunks[gi] // GRP
                    if g in bp_pre:
                        bp_bc = bp_pre.pop(g)
                    else:
                        bp_bc = bp_p.tile([P, GRP * CN], bf16, tag="bp")
                        nc.gpsimd.dma_start(out=bp_bc,
                                            in_=bcast(h_bp, GRP * CN,
                                                      offset=g * GRP * CN))
                    osb = {q: osb_p.tile([P, GRP * CN], bf16, tag="osb",
                                         name=f"osb_{g}_{q}")
                           for q in qts}
                    for cc in range(GRP):
                        c = chunks[gi + cc]
                        if c in pre and c in wp_pre:
                            wp_sb = wp_pre.pop(c)
                        else:
                            wp_sb = wp_p.tile([P, DT, CN], bf16, tag="wp")
                            dma_eng = (nc.sync if (c + dma_par) % 2 == 0
                                       else nc.scalar)
                            dma_eng.dma_start(out=wp_sb, in_=h_wp[c])
                        for qt in qts:
                            pm = ps.tile([P, 512], f32, tag="ps")
                            for k in range(DT):
                                nc.tensor.matmul(
                                    pm, lhsT=x2T[:, k, qt * P:(qt + 1) * P],
                                    rhs=wp_sb[:, k, :],
                                    start=(k == 0), stop=(k == DT - 1))
                            nc.vector.tensor_tensor(
                                out=osb[qt][:, cc * CN:(cc + 1) * CN], in0=pm,
                                in1=bp_bc[:, cc * CN:(cc + 1) * CN], op=ALU.add)
                            if cc == GRP - 1:
                                # fire each strip as soon as it completes
                                out_eng = nc.sync if qt < 2 else nc.scalar
                                out_eng.dma_start(
                                    out=h_out[qt * P:(qt + 1) * P,
                                              g * GRP * CN:(g + 1) * GRP * CN],
                                    in_=osb[qt])

            # early pass: first NE chunks for qt {0,1} while LN2(2,3) drains
            vocab_chunks(list(range(NE)), (0, 1), 0, pre=(0, 1, 2))
            transpose_cols(x2T, x2b, (2, 3), "x2t_tp")
            # late pass for those chunks' qt {2,3} (re-streamed), then the rest
            vocab_chunks(list(range(NE)), (2, 3), 1)
            vocab_chunks(list(range(NE, NCHUNK)), (0, 1, 2, 3), 0)

    nc.compile()
    return nc


def _tile_sq(w, kt):
    """[K, N] -> [128, K//128, N] contiguous."""
    k, n = w.shape
    assert k == kt * P
    return np.ascontiguousarray(
        w.reshape(kt, P, n).transpose(1, 0, 2)).astype(BF16)


def _pos_enc():
    posn = np.arange(S)[:, None].astype(np.float32)
    i = np.arange(0, D, 2).astype(np.float32)
    ang = posn / np.power(10000.0, i / D)
    pos = np.zeros((S, D), dtype=np.float32)
    pos[:, 0::2] = np.sin(ang)
    pos[:, 1::2] = np.cos(ang)
    return pos


def _prep_inputs(inputs):
    g = lambda name: np.asarray(inputs[name], dtype=np.float32)
    tokens = np.asarray(inputs["tokens"]).astype(np.int64)
    img = g("img_emb")
    table = g("emb_table")
    pos = _pos_enc()

    b1 = g("b1")
    g2 = g("g2")
    b2 = g("b2")
    wp = g("Wp") * g2[:, None]          # fold LN2 gamma
    wp_pad = np.zeros((D, VP), dtype=np.float32)
    wp_pad[:, :V] = wp
    wp_t = np.ascontiguousarray(
        wp_pad.reshape(DT, P, NCHUNK, CN).transpose(2, 1, 0, 3)).astype(BF16)
    bp_pad = np.zeros((VP,), dtype=np.float32)
    bp_pad[:V] = g("bp") + b2 @ g("Wp")  # fold LN2 beta
    bp_pad = bp_pad.astype(BF16)

    def bias_tiled(b):
        return np.ascontiguousarray(b.reshape(DT, P).T).astype(np.float32)

    def chunk_m(tiled):
        """[P, kt, D] -> [DT][P, kt, 128] contiguous chunks of output cols."""
        return np.ascontiguousarray(
            np.stack([tiled[:, :, m * P:(m + 1) * P] for m in range(DT)]))

    shared = {
        "wq1c": chunk_m(_tile_sq(g("Wq1") * SCALE, DT)),
        "wk1c": chunk_m(_tile_sq(g("Wk1"), DT)),
        "wv1": _tile_sq(g("Wv1"), DT),
        "wq2": _tile_sq(g("Wq2") * SCALE, DT),
        "wk2c": chunk_m(_tile_sq(g("Wk2"), DIT)),
        "wv2": _tile_sq(g("Wv2"), DIT),
        "wp": wp_t,
        "bq1": bias_tiled(g("bq1") * SCALE),
        "bk1": bias_tiled(g("bk1")),
        # fold LN1 beta into the cross-attn query bias and value bias
        "bq2": bias_tiled((g("bq2") + b1 @ g("Wq2")) * SCALE),
        "bk2": bias_tiled(g("bk2")),
        "bv2": g("bv2") + b1,
        "g1": g("g1"),
        "bp": bp_pad,
    }
    in_maps = []
    for c in range(N_CORES):
        m = dict(shared)
        x0 = table[tokens[c]] + pos                      # [S, D] f32
        # bv1 folded into the self-attn residual (A1 rows sum to 1)
        x0r = x0 + g("bv1")
        x0b = np.ascontiguousarray(
            x0r.reshape(ST, P, D).transpose(1, 0, 2)).astype(BF16)
        x0T = np.ascontiguousarray(
            x0.T.reshape(DT, P, S).transpose(1, 0, 2)).astype(BF16)
        m["x0b"] = x0b
        m["x0t"] = x0T
        m["img_t"] = np.ascontiguousarray(
            img[c].T.reshape(DIT, P, NI).transpose(1, 0, 2)).astype(BF16)
        in_maps.append(m)
    return in_maps


def _ensure_axon_hooks():
    """bass_utils imports antenv.axon_hooks when BASS_TRACE is set; stub it
    if the module is absent so tracing degrades instead of crashing."""
    try:
        import antenv.axon_hooks  # noqa: F401
    except ImportError:
        import types
        mod = types.ModuleType("antenv.axon_hooks")
        mod.get_axon_ntff_profile_hook = lambda: None
        mod.set_axon_ntff_profile_hook = lambda h: None
        sys.modules["antenv.axon_hooks"] = mod


def kernel(**inputs):
    global LAST_RESULTS
    _ensure_axon_hooks()
    from concourse.bass_utils import run_bass_kernel_spmd

    if "nc" not in _CACHE:
        _CACHE["nc"] = _build_program()
    nc = _CACHE["nc"]

    in_maps = _prep_inputs(inputs)
    res = run_bass_kernel_spmd(nc, in_maps, core_ids=list(range(N_CORES)))
    LAST_RESULTS = res
    out = np.stack([res.results[c]["out"][:, :V].astype(np.float32)
                    for c in range(N_CORES)])
    return out


# revision 44
# speedup vs baseline: 1.0221x; 1.0159x over previous
"""Trainium2 Bass kernel for an 8-batch image-conditioned decoder layer.

Strategy: pure data-parallel over the batch — core c computes batch element c
end-to-end (causal self-attention, cross-attention over the image tokens, both
layernorms, vocab projection). No collectives.

v2 schedule notes:
- Embedding gather + positional encoding are host-prepped (pure data movement);
  the device receives x0 in both seq-partition and d-partition layouts, so the
  TensorEngine starts on real work as soon as the first weights land.
- Dummy warmup matmuls run during the initial DMA window so the PE HAM clock
  gate is at full rate when QT starts.
- b1 is folded into bq2/bv2, and g2/b2 into Wp/bp (host-side), which trims the
  layernorm critical path; layer-2 LN output needs no affine at all.
- Layer 2 runs per-qt-pipelined (Q2T in two 256-col halves) and the first 4
  vocab chunks are computed early for qt {0,1} to keep the PE fed while the
  tail of attention drains; those chunks are re-streamed later for qt {2,3}.
- PSUM->SBUF copies, value-bias adds, and half of the element-wise work run on
  GpSimd; output DMAs go on the vector/gpsimd queues so sync/scalar stay free
  for Wp streaming.

All matmuls run in bf16 with fp32 PSUM accumulation.
"""

import os
import sys

for _p in ("/opt/trn_rl_repo", "/root/.axon_site/_ro/trn_rl_repo"):
    if os.path.isdir(_p) and _p not in sys.path:
        sys.path.append(_p)

import numpy as np
import ml_dtypes

BF16 = ml_dtypes.bfloat16

# Problem dims (hardcoded per spec)
V, D, DI, S, B, NI = 32000, 1024, 768, 512, 8, 197
EPS = 1e-5
P = 128
ST = S // P          # 4 seq tiles
DT = D // P          # 8 model-dim tiles
DIT = DI // P        # 6 image-dim tiles
NIT = 2              # image tokens: 197 -> 2 partition tiles (128 + 69)
NI_PAD = 256
VP = 32768           # vocab padded to 64 chunks of 512
CN = 512             # vocab chunk width
NCHUNK = VP // CN    # 64
GRP = 2              # chunks per output strip
NGRP = NCHUNK // GRP
NE = 4               # chunks computed early for qt {0,1} (re-streamed later)
N_CORES = 8
SCALE = 1.0 / float(np.sqrt(np.float32(D)))

_CACHE = {}
LAST_RESULTS = None


def _build_program():
    import concourse.bacc as bacc
    import concourse.bass as bass
    import concourse.mybir as mybir
    from concourse.masks import make_identity
    from concourse.tile import TileContext

    f32 = mybir.dt.float32
    bf16 = mybir.dt.bfloat16
    X = mybir.AxisListType.X
    ALU = mybir.AluOpType
    ACT_F = mybir.ActivationFunctionType

    nc = bacc.Bacc("TRN2", target_bir_lowering=False, debug=False,
                   num_devices=N_CORES)

    # ---- I/O ----
    h_x0b = nc.dram_tensor("x0b", [P, ST, D], bf16, kind="ExternalInput")
    h_x0T = nc.dram_tensor("x0t", [P, DT, S], bf16, kind="ExternalInput")
    h_img = nc.dram_tensor("img_t", [P, DIT, NI], bf16, kind="ExternalInput")
    h_wq1 = nc.dram_tensor("wq1c", [DT, P, DT, P], bf16, kind="ExternalInput")
    h_wk1 = nc.dram_tensor("wk1c", [DT, P, DT, P], bf16, kind="ExternalInput")
    h_wv1 = nc.dram_tensor("wv1", [P, DT, D], bf16, kind="ExternalInput")
    h_wq2 = nc.dram_tensor("wq2", [P, DT, D], bf16, kind="ExternalInput")
    h_wk2 = nc.dram_tensor("wk2c", [DT, P, DIT, P], bf16, kind="ExternalInput")
    h_wv2 = nc.dram_tensor("wv2", [P, DIT, D], bf16, kind="ExternalInput")
    h_wp = nc.dram_tensor("wp", [NCHUNK, P, DT, CN], bf16, kind="ExternalInput")
    h_bq1 = nc.dram_tensor("bq1", [P, DT], f32, kind="ExternalInput")
    h_bk1 = nc.dram_tensor("bk1", [P, DT], f32, kind="ExternalInput")
    h_bq2 = nc.dram_tensor("bq2", [P, DT], f32, kind="ExternalInput")
    h_bk2 = nc.dram_tensor("bk2", [P, DT], f32, kind="ExternalInput")
    h_bv2 = nc.dram_tensor("bv2", [D], f32, kind="ExternalInput")
    h_g1 = nc.dram_tensor("g1", [D], f32, kind="ExternalInput")
    h_bp = nc.dram_tensor("bp", [VP], bf16, kind="ExternalInput")
    h_out = nc.dram_tensor("out", [S, VP], bf16, kind="ExternalOutput")

    def bcast(handle, n, offset=0):
        ap = handle[:]
        return bass.AP(tensor=ap.tensor, offset=offset, ap=[[0, P], [1, n]])

    with TileContext(nc) as tc:
        import contextlib
        ctx = contextlib.ExitStack()
        with ctx:
            const = ctx.enter_context(tc.tile_pool(name="const", bufs=1))
            xb_p = ctx.enter_context(tc.tile_pool(name="xb", bufs=3))
            xt_p = ctx.enter_context(tc.tile_pool(name="xt", bufs=2))
            qk_p = ctx.enter_context(tc.tile_pool(name="qk", bufs=2))
            v_p = ctx.enter_context(tc.tile_pool(name="vp", bufs=2))
            k2t_p = ctx.enter_context(tc.tile_pool(name="k2t", bufs=1))
            pb_p = ctx.enter_context(tc.tile_pool(name="pb", bufs=4))
            pt_p = ctx.enter_context(tc.tile_pool(name="pt", bufs=1))
            xpre_p = ctx.enter_context(tc.tile_pool(name="xpre", bufs=2))
            stat_p = ctx.enter_context(tc.tile_pool(name="stat", bufs=4))
            wts_p = ctx.enter_context(tc.tile_pool(name="wts", bufs=2))
            wv2_p = ctx.enter_context(tc.tile_pool(name="wv2p", bufs=1))
            wqm_p = ctx.enter_context(tc.tile_pool(name="wqm", bufs=4))
            wp_p = ctx.enter_context(tc.tile_pool(name="wpp", bufs=3))
            bp_p = ctx.enter_context(tc.tile_pool(name="bpp", bufs=2))
            osb_p = ctx.enter_context(tc.tile_pool(name="osb", bufs=6))
            ps = ctx.enter_context(tc.tile_pool(name="ps", bufs=8, space="PSUM"))

            # ---- constants / warmup ----
            ident = const.tile([P, P], bf16)
            make_identity(nc, ident)
            trimask = const.tile([P, P], f32)
            nc.gpsimd.memset(trimask, 0.0)
            nc.gpsimd.affine_select(
                out=trimask, in_=trimask, compare_op=ALU.is_ge, fill=-1e10,
                base=0, pattern=[[-1, P]], channel_multiplier=1)
            warm_src = const.tile([P, 256], bf16)
            nc.vector.memset(warm_src, 0.0)
            epst = const.tile([P, 1], f32)
            nc.vector.memset(epst, EPS)
            neg1 = const.tile([P, 1], f32)
            nc.vector.memset(neg1, -1.0)

            # HAM warmup: keep the PE busy while the first weights stream in
            for w in range(28):
                pw = ps.tile([P, 512], f32, tag="ps", name=f"warm{w}")
                nc.tensor.matmul(pw[:, :256], lhsT=ident, rhs=warm_src,
                                 start=True, stop=True)

            # ---- early DMAs (per-queue order == emission order; the wqm
            # chunk-ring slot waits naturally pace later transfers behind
            # earlier consumption, keeping HBM free for the critical path) ----
            x0T = xt_p.tile([P, DT, S], bf16, tag="xt", name="x0t")
            nc.sync.dma_start(out=x0T, in_=h_x0T[:])
            bq1s = const.tile([P, DT], f32)
            bk1s = const.tile([P, DT], f32)
            bq2s = const.tile([P, DT], f32)
            bk2s = const.tile([P, DT], f32)
            for t, h in ((bq1s, h_bq1), (bk1s, h_bk1), (bq2s, h_bq2),
                         (bk2s, h_bk2)):
                nc.gpsimd.dma_start(out=t, in_=h[:])

            def chunk_dmas(h_src, kt, n, engs=(nc.sync, nc.scalar)):
                tiles = []
                for m in range(n):
                    w_m = wqm_p.tile([P, kt, P], bf16, tag="wqm",
                                     name=f"wc{m}")
                    engs[m % len(engs)].dma_start(out=w_m, in_=h_src[m])
                    tiles.append(w_m)
                return tiles

            def proj_stream(h_src, b_sb, rhsT, name, kt=DT, lead=4):
                """out[P, DT, S] bf16 = (W.T @ x.T) + b, d-partition.

                Weight chunks stream just-in-time from the scalar queue, paced
                by the act of the previous m-group, so early HBM bandwidth
                stays free for the critical path."""
                o = qk_p.tile([P, DT, S], bf16, tag="qk", name=name)
                tiles = {}

                def load(m):
                    if m < DT:
                        t = wqm_p.tile([P, kt, P], bf16, tag="wqm",
                                       name=f"{name}c{m}")
                        nc.scalar.dma_start(out=t, in_=h_src[m])
                        tiles[m] = t

                for m in range(lead):
                    load(m)
                for m in range(DT):
                    pm = ps.tile([P, 512], f32, tag="ps", name="pm")
                    for k in range(kt):
                        nc.tensor.matmul(pm,
                                         lhsT=tiles[m][:, k, :],
                                         rhs=rhsT[:, k, :],
                                         start=(k == 0), stop=(k == kt - 1))
                    load(m + lead)
                    nc.scalar.activation(out=o[:, m, :], in_=pm,
                                         func=ACT_F.Identity,
                                         bias=b_sb[:, m:m + 1], scale=1.0)
                return o

            def proj_T_into(o, w_sb, b_sb, rhsT, c0, c1, kt=DT):
                w = c1 - c0
                for m in range(DT):
                    pm = ps.tile([P, 512], f32, tag="ps", name="pm")
                    for k in range(kt):
                        nc.tensor.matmul(pm[:, :w],
                                         lhsT=w_sb[:, k, m * P:(m + 1) * P],
                                         rhs=rhsT[:, k, c0:c1],
                                         start=(k == 0), stop=(k == kt - 1))
                    nc.scalar.activation(out=o[:, m, c0:c1], in_=pm[:, :w],
                                         func=ACT_F.Identity,
                                         bias=b_sb[:, m:m + 1], scale=1.0)

            QT = proj_stream(h_wq1, bq1s, x0T, "qt")

            # gpsimd-gated DMAs: staged across two gates so they never starve
            # the just-in-time weight chunks feeding the PE
            gate_t = const.tile([P, 1], bf16)
            nc.gpsimd.tensor_copy(out=gate_t, in_=QT[:, 0, 0:1])
            img_sb = const.tile([P, DIT, NI], bf16)
            nc.gpsimd.dma_start(out=img_sb, in_=h_img[:])

            # wv1 fires from the scalar stream after QT's acts (~21us)
            wv1_sb = wts_p.tile([P, DT, D], bf16, tag="wts")
            nc.scalar.dma_start(out=wv1_sb, in_=h_wv1[:])

            KT = proj_stream(h_wk1, bk1s, x0T, "kt")

            gate_t2 = const.tile([P, 1], bf16)
            nc.gpsimd.tensor_copy(out=gate_t2, in_=KT[:, 0, 0:1])
            x0b = xb_p.tile([P, ST, D], bf16, tag="xb", name="x0b")
            nc.gpsimd.dma_start(out=x0b, in_=h_x0b[:])
            wv2_sb = wv2_p.tile([P, DIT, D], bf16, tag="wv2")
            nc.gpsimd.dma_start(out=wv2_sb, in_=h_wv2[:])
            wk2c = chunk_dmas(h_wk2, DIT, DT, engs=(nc.gpsimd,))
            g1b = const.tile([P, D], f32)
            bv2b = const.tile([P, D], f32)
            for t, h in ((g1b, h_g1), (bv2b, h_bv2)):
                nc.gpsimd.dma_start(out=t, in_=bcast(h, D))

            # value projection (bv1 folded into x0b host-side; attention rows
            # are convex combinations so the V-bias passes through unchanged)
            Vt = v_p.tile([P, ST, D], bf16, tag="v")
            for a in range(ST):
                for nh in range(2):
                    pm = ps.tile([P, 512], f32, tag="ps")
                    for k in range(DT):
                        nc.tensor.matmul(
                            pm, lhsT=x0T[:, k, a * P:(a + 1) * P],
                            rhs=wv1_sb[:, k, nh * 512:(nh + 1) * 512],
                            start=(k == 0), stop=(k == DT - 1))
                    nc.scalar.copy(out=Vt[:, a, nh * 512:(nh + 1) * 512],
                                   in_=pm)
                if a == 1:
                    # wq2 issued mid-Vt from the scalar stream (needed ~30us on)
                    wq2_sb = wts_p.tile([P, DT, D], bf16, tag="wts")
                    nc.scalar.dma_start(out=wq2_sb, in_=h_wq2[:])

            # ---- causal self-attention: scores + softmax (all qt) ----
            Pbs = []
            rinv1 = stat_p.tile([P, ST], f32, tag="rinv")
            for qt in range(ST):
                width = (qt + 1) * P
                pm = ps.tile([P, 512], f32, tag="ps")
                for k in range(DT):
                    nc.tensor.matmul(pm[:, :width],
                                     lhsT=QT[:, k, qt * P:(qt + 1) * P],
                                     rhs=KT[:, k, :width],
                                     start=(k == 0), stop=(k == DT - 1))
                # mask the diagonal block in place (PSUM RMW)
                nc.vector.tensor_tensor(out=pm[:, qt * P:width],
                                        in0=pm[:, qt * P:width], in1=trimask,
                                        op=ALU.add)
                nmax = stat_p.tile([P, 1], f32, tag="nmax")
                nc.vector.reduce_max(nmax, pm[:, :width], axis=X, negate=True)
                Pb = pb_p.tile([P, 512], bf16, tag="pb", name=f"pb{qt}")
                rsum = stat_p.tile([P, 1], f32, tag="rsum")
                nc.scalar.activation(out=Pb[:, :width], in_=pm[:, :width],
                                     func=ACT_F.Exp, bias=nmax, scale=1.0,
                                     accum_out=rsum)
                nc.vector.reciprocal(out=rinv1[:, qt:qt + 1], in_=rsum)
                Pbs.append(Pb)

            # ---- cross-attn K2 (fills the softmax1 pipeline shadow) ----
            K2T = k2t_p.tile([P, DT, NI_PAD], bf16, tag="k2t")
            for m in range(DT):
                pm = ps.tile([P, 512], f32, tag="ps")
                for k in range(DIT):
                    nc.tensor.matmul(pm[:, :NI],
                                     lhsT=wk2c[m][:, k, :],
                                     rhs=img_sb[:, k, :],
                                     start=(k == 0), stop=(k == DIT - 1))
                nc.scalar.activation(out=K2T[:, m, :NI], in_=pm[:, :NI],
                                     func=ACT_F.Identity,
                                     bias=bk2s[:, m:m + 1], scale=1.0)

            def layernorm(xpre, out_sl, gb):
                """xpre [P, D] f32 -> out_sl [P, D] bf16.

                Writes the normalized rows times gb (or raw normalized rows if
                gb is None — affine folded into the consumers)."""
                stats = stat_p.tile([P, 2, 6], f32, tag="bnst")
                for sg in range(2):
                    nc.vector.bn_stats(out=stats[:, sg, :],
                                       in_=xpre[:, sg * 512:(sg + 1) * 512])
                mv = stat_p.tile([P, 2], f32, tag="bnmv")
                nc.vector.bn_aggr(out=mv, in_=stats)
                rstd = stat_p.tile([P, 1], f32, tag="rstd")
                nc.scalar.activation(out=rstd, in_=mv[:, 1:2], func=ACT_F.Sqrt,
                                     bias=epst, scale=1.0)
                nc.vector.reciprocal(out=rstd, in_=rstd)
                nmr = stat_p.tile([P, 1], f32, tag="nmr")
                nc.vector.scalar_tensor_tensor(
                    out=nmr, in0=mv[:, 0:1], scalar=rstd, in1=neg1,
                    op0=ALU.mult, op1=ALU.mult)
                if gb is None:
                    nc.scalar.activation(out=out_sl, in_=xpre,
                                         func=ACT_F.Identity,
                                         bias=nmr, scale=rstd)
                else:
                    nc.scalar.activation(out=xpre, in_=xpre,
                                         func=ACT_F.Identity,
                                         bias=nmr, scale=rstd)
                    nc.gpsimd.tensor_tensor(out=out_sl, in0=xpre,
                                            in1=gb, op=ALU.mult)

            # ---- AV1 + residual + LN1 per qt ----
            PT = pt_p.tile([P, ST, S], bf16, tag="pt")
            x1b = xb_p.tile([P, ST, D], bf16, tag="xb", name="x1b")
            wp_pre = {}
            bp_pre = {}
            # cross-attn V2 blocks are interleaved below as PE filler while
            # the VEC-bound AV1/LN1 chain drains
            V2t = v_p.tile([P, NIT, D], bf16, tag="v")
            nc.gpsimd.memset(V2t, 0.0)

            def v2_block(a, nh):
                pa = P if a == 0 else NI - P
                pm = ps.tile([P, 512], f32, tag="ps")
                for k in range(DIT):
                    nc.tensor.matmul(
                        pm[:pa, :], lhsT=img_sb[:, k, a * P:a * P + pa],
                        rhs=wv2_sb[:, k, nh * 512:(nh + 1) * 512],
                        start=(k == 0), stop=(k == DIT - 1))
                nc.vector.tensor_tensor(
                    out=V2t[:pa, a, nh * 512:(nh + 1) * 512], in0=pm[:pa, :],
                    in1=bv2b[:pa, nh * 512:(nh + 1) * 512], op=ALU.add)

            for qt in range(ST):
                for kt in range(qt + 1):
                    tp = ps.tile([P, 512], bf16, tag="ps", name="tp")
                    nc.tensor.transpose(out=tp[:, :P],
                                        in_=Pbs[qt][:, kt * P:(kt + 1) * P],
                                        identity=ident)
                    nc.vector.tensor_copy(out=PT[:, kt, qt * P:(qt + 1) * P],
                                          in_=tp[:, :P])
                xpre = xpre_p.tile([P, D], f32, tag="xpre")
                for nh in range(2):
                    sl = slice(nh * 512, (nh + 1) * 512)
                    pm = ps.tile([P, 512], f32, tag="ps")
                    for kt in range(qt + 1):
                        nc.tensor.matmul(pm, lhsT=PT[:, kt, qt * P:(qt + 1) * P],
                                         rhs=Vt[:, kt, nh * 512:(nh + 1) * 512],
                                         start=(kt == 0), stop=(kt == qt))
                    # residual split: SCA rescale, then GPS add (keeps VEC free)
                    nc.scalar.activation(out=xpre[:, sl], in_=pm,
                                         func=ACT_F.Identity,
                                         scale=rinv1[:, qt:qt + 1])
                    nc.gpsimd.tensor_tensor(out=xpre[:, sl], in0=xpre[:, sl],
                                            in1=x0b[:, qt, sl], op=ALU.add)
                v2_block(qt % NIT, qt // NIT)  # PE filler during stt/LN1
                layernorm(xpre, x1b[:, qt, :], g1b)
                if qt == 0:
                    # prefetch the first vocab chunks + bias strip now that
                    # the startup DMA window has drained (gpsimd stream pos)
                    for c in range(3):
                        t = wp_p.tile([P, DT, CN], bf16, tag="wp",
                                      name=f"wp_pre{c}")
                        nc.gpsimd.dma_start(out=t, in_=h_wp[c])
                        wp_pre[c] = t
                    bp0 = bp_p.tile([P, GRP * CN], bf16, tag="bp",
                                    name="bp_pre0")
                    nc.gpsimd.dma_start(out=bp0, in_=bcast(h_bp, GRP * CN))
                    bp_pre[0] = bp0

            # ---- layer 2, pipelined in qt-pair halves ----
            def transpose_cols(dst, src_b, a_list, tag):
                """transpose x[P, a, db*P:(db+1)*P] -> dst[:, db, a*P:(a+1)*P]."""
                for a in a_list:
                    for db in range(DT):
                        tp = ps.tile([P, 512], bf16, tag="ps", name=tag)
                        nc.tensor.transpose(
                            out=tp[:, :P],
                            in_=src_b[:, a, db * P:(db + 1) * P],
                            identity=ident)
                        nc.scalar.copy(
                            out=dst[:, db, a * P:(a + 1) * P], in_=tp[:, :P])

            x1T = xt_p.tile([P, DT, S], bf16, tag="xt", name="x1t")
            Q2T = qk_p.tile([P, DT, S], bf16, tag="qk", name="q2t")
            P2bs = []
            rinv2 = stat_p.tile([P, ST], f32, tag="rinv2")

            def scores2_softmax(qt):
                pm = ps.tile([P, 512], f32, tag="ps")
                for k in range(DT):
                    nc.tensor.matmul(pm[:, :NI],
                                     lhsT=Q2T[:, k, qt * P:(qt + 1) * P],
                                     rhs=K2T[:, k, :NI],
                                     start=(k == 0), stop=(k == DT - 1))
                nmax = stat_p.tile([P, 1], f32, tag="nmax")
                nc.vector.reduce_max(nmax, pm[:, :NI], axis=X, negate=True)
                P2b = pb_p.tile([P, NI_PAD], bf16, tag="pb2", name=f"p2b{qt}")
                nc.gpsimd.memset(P2b[:, NI:], 0.0)
                rsum = stat_p.tile([P, 1], f32, tag="rsum")
                nc.scalar.activation(out=P2b[:, :NI], in_=pm[:, :NI],
                                     func=ACT_F.Exp, bias=nmax, scale=1.0,
                                     accum_out=rsum)
                nc.vector.reciprocal(out=rinv2[:, qt:qt + 1], in_=rsum)
                P2bs.append(P2b)

            # first half: qt 0,1
            transpose_cols(x1T, x1b, (0, 1), "x1t_tp")
            proj_T_into(Q2T, wq2_sb, bq2s, x1T, 0, 256)
            scores2_softmax(0)
            scores2_softmax(1)
            # second half: qt 2,3
            transpose_cols(x1T, x1b, (2, 3), "x1t_tp")
            proj_T_into(Q2T, wq2_sb, bq2s, x1T, 256, 512)
            scores2_softmax(2)
            scores2_softmax(3)

            # ---- AV2 + residual + LN2 per qt (normalized out; affine folded
            # into Wp/bp) ----
            PT2 = pt_p.tile([P, NIT, S], bf16, tag="pt2")
            x2b = xb_p.tile([P, ST, D], bf16, tag="xb", name="x2b")

            def av2_ln2(qt):
                for kt in range(NIT):
                    tp = ps.tile([P, 512], bf16, tag="ps", name="tp2")
                    nc.tensor.transpose(out=tp[:, :P],
                                        in_=P2bs[qt][:, kt * P:(kt + 1) * P],
                                        identity=ident)
                    nc.vector.tensor_copy(out=PT2[:, kt, qt * P:(qt + 1) * P],
                                          in_=tp[:, :P])
                xpre = xpre_p.tile([P, D], f32, tag="xpre")
                for nh in range(2):
                    sl = slice(nh * 512, (nh + 1) * 512)
                    pm = ps.tile([P, 512], f32, tag="ps")
                    for kt in range(NIT):
                        nc.tensor.matmul(pm, lhsT=PT2[:, kt, qt * P:(qt + 1) * P],
                                         rhs=V2t[:, kt, nh * 512:(nh + 1) * 512],
                                         start=(kt == 0), stop=(kt == NIT - 1))
                    nc.scalar.activation(out=xpre[:, sl], in_=pm,
                                         func=ACT_F.Identity,
                                         scale=rinv2[:, qt:qt + 1])
                    nc.gpsimd.tensor_tensor(out=xpre[:, sl], in0=xpre[:, sl],
                                            in1=x1b[:, qt, sl], op=ALU.add)
                layernorm(xpre, x2b[:, qt, :], None)

            x2T = xt_p.tile([P, DT, S], bf16, tag="xt", name="x2t")

            for qt in range(ST):
                av2_ln2(qt)
            transpose_cols(x2T, x2b, (0, 1), "x2t_tp")

            # ---- vocab projection, streamed in CN-column chunks ----
            def vocab_chunks(chunks, qts, dma_par, pre=()):
                """Process wp[chunks] x qts; chunks must align to GRP groups."""
                for gi in range(0, len(chunks), GRP):
                    g = chunks[gi] // GRP
                    if g in bp_pre:
                        bp_bc = bp_pre.pop(g)
                    else:
                        bp_bc = bp_p.tile([P, GRP * CN], bf16, tag="bp")
                        nc.gpsimd.dma_start(out=bp_bc,
                                            in_=bcast(h_bp, GRP * CN,
                                                      offset=g * GRP * CN))
                    osb = {q: osb_p.tile([P, GRP * CN], bf16, tag="osb",
                                         name=f"osb_{g}_{q}")
                           for q in qts}
                    for cc in range(GRP):
                        c = chunks[gi + cc]
                        if c in pre and c in wp_pre:
                            wp_sb = wp_pre.pop(c)
                        else:
                            wp_sb = wp_p.tile([P, DT, CN], bf16, tag="wp")
                            dma_eng = (nc.sync if (c + dma_par) % 2 == 0
                                       else nc.scalar)
                            dma_eng.dma_start(out=wp_sb, in_=h_wp[c])
                        for qt in qts:
                            pm = ps.tile([P, 512], f32, tag="ps")
                            for k in range(DT):
                                nc.tensor.matmul(
                                    pm, lhsT=x2T[:, k, qt * P:(qt + 1) * P],
                                    rhs=wp_sb[:, k, :],
                                    start=(k == 0), stop=(k == DT - 1))
                            nc.vector.tensor_tensor(
                                out=osb[qt][:, cc * CN:(cc + 1) * CN], in0=pm,
                                in1=bp_bc[:, cc * CN:(cc + 1) * CN], op=ALU.add)
                            if cc == GRP - 1:
                                # fire each strip as soon as it completes
                                out_eng = nc.sync if qt < 2 else nc.scalar
                                out_eng.dma_start(
                                    out=h_out[qt * P:(qt + 1) * P,
                                              g * GRP * CN:(g + 1) * GRP * CN],
                                    in_=osb[qt])

            # early pass: first NE chunks for qt {0,1} while LN2(2,3) drains
            vocab_chunks(list(range(NE)), (0, 1), 0, pre=(0, 1, 2))
            transpose_cols(x2T, x2b, (2, 3), "x2t_tp")
            # late pass for those chunks' qt {2,3} (re-streamed), then the rest
            vocab_chunks(list(range(NE)), (2, 3), 1)
            vocab_chunks(list(range(NE, NCHUNK)), (0, 1, 2, 3), 0)

    nc.compile()
    return nc


def _tile_sq(w, kt):
    """[K, N] -> [128, K//128, N] contiguous."""
    k, n = w.shape
    assert k == kt * P
    return np.ascontiguousarray(
        w.reshape(kt, P, n).transpose(1, 0, 2)).astype(BF16)


def _pos_enc():
    posn = np.arange(S)[:, None].astype(np.float32)
    i = np.arange(0, D, 2).astype(np.float32)
    ang = posn / np.power(10000.0, i / D)
    pos = np.zeros((S, D), dtype=np.float32)
    pos[:, 0::2] = np.sin(ang)
    pos[:, 1::2] = np.cos(ang)
    return pos


def _prep_inputs(inputs):
    g = lambda name: np.asarray(inputs[name], dtype=np.float32)
    tokens = np.asarray(inputs["tokens"]).astype(np.int64)
    img = g("img_emb")
    table = g("emb_table")
    pos = _pos_enc()

    b1 = g("b1")
    g2 = g("g2")
    b2 = g("b2")
    wp = g("Wp") * g2[:, None]          # fold LN2 gamma
    wp_pad = np.zeros((D, VP), dtype=np.float32)
    wp_pad[:, :V] = wp
    wp_t = np.ascontiguousarray(
        wp_pad.reshape(DT, P, NCHUNK, CN).transpose(2, 1, 0, 3)).astype(BF16)
    bp_pad = np.zeros((VP,), dtype=np.float32)
    bp_pad[:V] = g("bp") + b2 @ g("Wp")  # fold LN2 beta
    bp_pad = bp_pad.astype(BF16)

    def bias_tiled(b):
        return np.ascontiguousarray(b.reshape(DT, P).T).astype(np.float32)

    def chunk_m(tiled):
        """[P, kt, D] -> [DT][P, kt, 128] contiguous chunks of output cols."""
        return np.ascontiguousarray(
            np.stack([tiled[:, :, m * P:(m + 1) * P] for m in range(DT)]))

    shared = {
        "wq1c": chunk_m(_tile_sq(g("Wq1") * SCALE, DT)),
        "wk1c": chunk_m(_tile_sq(g("Wk1"), DT)),
        "wv1": _tile_sq(g("Wv1"), DT),
        "wq2": _tile_sq(g("Wq2") * SCALE, DT),
        "wk2c": chunk_m(_tile_sq(g("Wk2"), DIT)),
        "wv2": _tile_sq(g("Wv2"), DIT),
        "wp": wp_t,
        "bq1": bias_tiled(g("bq1") * SCALE),
        "bk1": bias_tiled(g("bk1")),
        # fold LN1 beta into the cross-attn query bias and value bias
        "bq2": bias_tiled((g("bq2") + b1 @ g("Wq2")) * SCALE),
        "bk2": bias_tiled(g("bk2")),
        "bv2": g("bv2") + b1,
        "g1": g("g1"),
        "bp": bp_pad,
    }
    in_maps = []
    for c in range(N_CORES):
        m = dict(shared)
        x0 = table[tokens[c]] + pos                      # [S, D] f32
        # bv1 folded into the self-attn residual (A1 rows sum to 1)
        x0r = x0 + g("bv1")
        x0b = np.ascontiguousarray(
            x0r.reshape(ST, P, D).transpose(1, 0, 2)).astype(BF16)
        x0T = np.ascontiguousarray(
            x0.T.reshape(DT, P, S).transpose(1, 0, 2)).astype(BF16)
        m["x0b"] = x0b
        m["x0t"] = x0T
        m["img_t"] = np.ascontiguousarray(
            img[c].T.reshape(DIT, P, NI).transpose(1, 0, 2)).astype(BF16)
        in_maps.append(m)
    return in_maps


def _ensure_axon_hooks():
    """bass_utils imports antenv.axon_hooks when BASS_TRACE is set; stub it
    if the module is absent so tracing degrades instead of crashing."""
    try:
        import antenv.axon_hooks  # noqa: F401
    except ImportError:
        import types
        mod = types.ModuleType("antenv.axon_hooks")
        mod.get_axon_ntff_profile_hook = lambda: None
        mod.set_axon_ntff_profile_hook = lambda h: None
        sys.modules["antenv.axon_hooks"] = mod


def kernel(**inputs):
    global LAST_RESULTS
    _ensure_axon_hooks()
    from concourse.bass_utils import run_bass_kernel_spmd

    if "nc" not in _CACHE:
        _CACHE["nc"] = _build_program()
    nc = _CACHE["nc"]

    in_maps = _prep_inputs(inputs)
    res = run_bass_kernel_spmd(nc, in_maps, core_ids=list(range(N_CORES)))
    LAST_RESULTS = res
    out = np.stack([res.results[c]["out"][:, :V].astype(np.float32)
                    for c in range(N_CORES)])
    return out


# revision 45
# speedup vs baseline: 1.0266x; 1.0044x over previous
"""Trainium2 Bass kernel for an 8-batch image-conditioned decoder layer.

Strategy: pure data-parallel over the batch — core c computes batch element c
end-to-end (causal self-attention, cross-attention over the image tokens, both
layernorms, vocab projection). No collectives.

Schedule notes:
- Embedding gather + positional encoding are host-prepped (pure data movement);
  the device receives x0 in both seq-partition and d-partition layouts.
- Every DMA-touched tensor is laid out 2D ([P, free]) so each transfer lowers
  to a single DIRECT2D descriptor (3D APs cost one issue slot per outer index,
  ~0.6us of engine time each).
- Dummy warmup matmuls run during the initial DMA window so the PE HAM clock
  gate is at full rate when QT starts.
- Q/K weights stream as per-m-group chunks just-in-time, paced by the compute
  stream, so early HBM bandwidth stays focused on the critical path; larger
  secondary tensors are released behind gates keyed on QT/KT progress.
- b1 is folded into bq2/bv2, bv1 into the residual copy of x0, and g2/b2 into
  Wp/bp, which trims the layernorm critical path.
- Layer 2 runs per-qt-pipelined (Q2T in two 256-col halves) and the first
  vocab chunks are computed early for qt {0,1}; those chunks are re-streamed
  later for qt {2,3}.
- PSUM->SBUF moves run on Scalar; SBUF-only elementwise work on GpSimd; VEC
  keeps softmax/bn_stats and the vocab bias adds.

All matmuls run in bf16 with fp32 PSUM accumulation.
"""

import os
import sys

for _p in ("/opt/trn_rl_repo", "/root/.axon_site/_ro/trn_rl_repo"):
    if os.path.isdir(_p) and _p not in sys.path:
        sys.path.append(_p)

import numpy as np
import ml_dtypes

BF16 = ml_dtypes.bfloat16

# Problem dims (hardcoded per spec)
V, D, DI, S, B, NI = 32000, 1024, 768, 512, 8, 197
EPS = 1e-5
P = 128
ST = S // P          # 4 seq tiles
DT = D // P          # 8 model-dim tiles
DIT = DI // P        # 6 image-dim tiles
NIT = 2              # image tokens: 197 -> 2 partition tiles (128 + 69)
NI_PAD = 256
VP = 32768           # vocab padded to 64 chunks of 512
CN = 512             # vocab chunk width
NCHUNK = VP // CN    # 64
GRP = 2              # chunks per output strip
NGRP = NCHUNK // GRP
NE = 4               # chunks computed early for qt {0,1} (re-streamed later)
N_CORES = 8
SCALE = 1.0 / float(np.sqrt(np.float32(D)))

_CACHE = {}
LAST_RESULTS = None


def _build_program():
    import concourse.bacc as bacc
    import concourse.bass as bass
    import concourse.mybir as mybir
    from concourse.masks import make_identity
    from concourse.tile import TileContext

    f32 = mybir.dt.float32
    bf16 = mybir.dt.bfloat16
    X = mybir.AxisListType.X
    ALU = mybir.AluOpType
    ACT_F = mybir.ActivationFunctionType

    nc = bacc.Bacc("TRN2", target_bir_lowering=False, debug=False,
                   num_devices=N_CORES)

    # ---- I/O (all 2D so every DMA is a single DIRECT2D) ----
    h_x0b = nc.dram_tensor("x0b", [P, ST * D], bf16, kind="ExternalInput")
    h_x0T = nc.dram_tensor("x0t", [P, DT * S], bf16, kind="ExternalInput")
    h_img = nc.dram_tensor("img_t", [P, DIT * NI], bf16, kind="ExternalInput")
    h_wq1 = nc.dram_tensor("wq1c", [DT, P, DT * P], bf16, kind="ExternalInput")
    h_wk1 = nc.dram_tensor("wk1c", [DT, P, DT * P], bf16, kind="ExternalInput")
    h_wv1 = nc.dram_tensor("wv1", [P, DT * D], bf16, kind="ExternalInput")
    h_wq2 = nc.dram_tensor("wq2", [P, DT * D], bf16, kind="ExternalInput")
    h_wk2 = nc.dram_tensor("wk2c", [DT, P, DIT * P], bf16,
                           kind="ExternalInput")
    h_wv2 = nc.dram_tensor("wv2", [P, DIT * D], bf16, kind="ExternalInput")
    h_wp = nc.dram_tensor("wp", [NCHUNK, P, DT * CN], bf16,
                          kind="ExternalInput")
    h_bq1 = nc.dram_tensor("bq1", [P, DT], f32, kind="ExternalInput")
    h_bk1 = nc.dram_tensor("bk1", [P, DT], f32, kind="ExternalInput")
    h_bq2 = nc.dram_tensor("bq2", [P, DT], f32, kind="ExternalInput")
    h_bk2 = nc.dram_tensor("bk2", [P, DT], f32, kind="ExternalInput")
    h_bv2 = nc.dram_tensor("bv2", [D], f32, kind="ExternalInput")
    h_g1 = nc.dram_tensor("g1", [D], f32, kind="ExternalInput")
    h_bp = nc.dram_tensor("bp", [VP], bf16, kind="ExternalInput")
    h_out = nc.dram_tensor("out", [S, VP], bf16, kind="ExternalOutput")

    def bcast(handle, n, offset=0):
        ap = handle[:]
        return bass.AP(tensor=ap.tensor, offset=offset, ap=[[0, P], [1, n]])

    with TileContext(nc) as tc:
        import contextlib
        ctx = contextlib.ExitStack()
        with ctx:
            const = ctx.enter_context(tc.tile_pool(name="const", bufs=1))
            xb_p = ctx.enter_context(tc.tile_pool(name="xb", bufs=3))
            xt_p = ctx.enter_context(tc.tile_pool(name="xt", bufs=2))
            qk_p = ctx.enter_context(tc.tile_pool(name="qk", bufs=2))
            v_p = ctx.enter_context(tc.tile_pool(name="vp", bufs=2))
            k2t_p = ctx.enter_context(tc.tile_pool(name="k2t", bufs=1))
            pb_p = ctx.enter_context(tc.tile_pool(name="pb", bufs=4))
            pt_p = ctx.enter_context(tc.tile_pool(name="pt", bufs=1))
            xpre_p = ctx.enter_context(tc.tile_pool(name="xpre", bufs=2))
            stat_p = ctx.enter_context(tc.tile_pool(name="stat", bufs=4))
            wts_p = ctx.enter_context(tc.tile_pool(name="wts", bufs=2))
            wv2_p = ctx.enter_context(tc.tile_pool(name="wv2p", bufs=1))
            wqm_p = ctx.enter_context(tc.tile_pool(name="wqm", bufs=4))
            wp_p = ctx.enter_context(tc.tile_pool(name="wpp", bufs=3))
            bp_p = ctx.enter_context(tc.tile_pool(name="bpp", bufs=2))
            osb_p = ctx.enter_context(tc.tile_pool(name="osb", bufs=6))
            ps = ctx.enter_context(tc.tile_pool(name="ps", bufs=8, space="PSUM"))

            # ---- constants / warmup ----
            ident = const.tile([P, P], bf16)
            make_identity(nc, ident)
            trimask = const.tile([P, P], f32)
            nc.gpsimd.memset(trimask, 0.0)
            nc.gpsimd.affine_select(
                out=trimask, in_=trimask, compare_op=ALU.is_ge, fill=-1e10,
                base=0, pattern=[[-1, P]], channel_multiplier=1)
            warm_src = const.tile([P, 256], bf16)
            nc.vector.memset(warm_src, 0.0)
            epst = const.tile([P, 1], f32)
            nc.vector.memset(epst, EPS)
            neg1 = const.tile([P, 1], f32)
            nc.vector.memset(neg1, -1.0)

            # HAM warmup: keep the PE busy while the first weights stream in
            for w in range(28):
                pw = ps.tile([P, 512], f32, tag="ps", name=f"warm{w}")
                nc.tensor.matmul(pw[:, :256], lhsT=ident, rhs=warm_src,
                                 start=True, stop=True)

            # ---- early DMAs ----
            x0T = xt_p.tile([P, DT * S], bf16, tag="xt", name="x0t")
            nc.sync.dma_start(out=x0T, in_=h_x0T[:])
            bq1s = const.tile([P, DT], f32)
            bk1s = const.tile([P, DT], f32)
            bq2s = const.tile([P, DT], f32)
            bk2s = const.tile([P, DT], f32)
            for t, h in ((bq1s, h_bq1), (bk1s, h_bk1), (bq2s, h_bq2),
                         (bk2s, h_bk2)):
                nc.gpsimd.dma_start(out=t, in_=h[:])

            def proj_stream(h_src, b_sb, name, kt=DT, lead=4):
                """out[P, DT, S] bf16 = (W.T @ x0.T) + b, d-partition.

                Weight chunks stream just-in-time from the scalar queue, paced
                by the compute stream, so early HBM bandwidth stays free for
                the critical path."""
                o = qk_p.tile([P, DT, S], bf16, tag="qk", name=name)
                tiles = {}

                def load(m):
                    if m < DT:
                        t = wqm_p.tile([P, kt * P], bf16, tag="wqm",
                                       name=f"{name}c{m}")
                        nc.scalar.dma_start(out=t, in_=h_src[m])
                        tiles[m] = t

                for m in range(lead):
                    load(m)
                for m in range(DT):
                    pm = ps.tile([P, 512], f32, tag="ps", name="pm")
                    for k in range(kt):
                        nc.tensor.matmul(pm,
                                         lhsT=tiles[m][:, k * P:(k + 1) * P],
                                         rhs=x0T[:, k * S:(k + 1) * S],
                                         start=(k == 0), stop=(k == kt - 1))
                    load(m + lead)
                    nc.scalar.activation(out=o[:, m, :], in_=pm,
                                         func=ACT_F.Identity,
                                         bias=b_sb[:, m:m + 1], scale=1.0)
                return o

            QT = proj_stream(h_wq1, bq1s, "qt")

            # gpsimd-gated DMAs: staged across two gates so they never starve
            # the just-in-time weight chunks feeding the PE
            gate_t = const.tile([P, 1], bf16)
            nc.gpsimd.tensor_copy(out=gate_t, in_=QT[:, 0, 0:1])
            img_sb = const.tile([P, DIT * NI], bf16)
            nc.gpsimd.dma_start(out=img_sb, in_=h_img[:])

            # wv1 fires from the scalar stream after QT's acts (~21us)
            wv1_sb = wts_p.tile([P, DT * D], bf16, tag="wts")
            nc.scalar.dma_start(out=wv1_sb, in_=h_wv1[:])

            KT = proj_stream(h_wk1, bk1s, "kt")

            gate_t2 = const.tile([P, 1], bf16)
            nc.gpsimd.tensor_copy(out=gate_t2, in_=KT[:, 0, 0:1])
            x0b = xb_p.tile([P, ST * D], bf16, tag="xb", name="x0b")
            nc.gpsimd.dma_start(out=x0b, in_=h_x0b[:])
            wv2_sb = wv2_p.tile([P, DIT * D], bf16, tag="wv2")
            nc.gpsimd.dma_start(out=wv2_sb, in_=h_wv2[:])
            wk2c = []
            for m in range(DT):
                t = wqm_p.tile([P, DIT * P], bf16, tag="wqm", name=f"k2c{m}")
                nc.gpsimd.dma_start(out=t, in_=h_wk2[m])
                wk2c.append(t)
            g1b = const.tile([P, D], f32)
            bv2b = const.tile([P, D], f32)
            for t, h in ((g1b, h_g1), (bv2b, h_bv2)):
                nc.gpsimd.dma_start(out=t, in_=bcast(h, D))

            # value projection (bv1 folded into x0b host-side; attention rows
            # are convex combinations so the V-bias passes through unchanged)
            Vt = v_p.tile([P, ST, D], bf16, tag="v")
            for a in range(ST):
                for nh in range(2):
                    pm = ps.tile([P, 512], f32, tag="ps")
                    for k in range(DT):
                        nc.tensor.matmul(
                            pm,
                            lhsT=x0T[:, k * S + a * P:k * S + (a + 1) * P],
                            rhs=wv1_sb[:, k * D + nh * 512:
                                       k * D + (nh + 1) * 512],
                            start=(k == 0), stop=(k == DT - 1))
                    nc.scalar.copy(out=Vt[:, a, nh * 512:(nh + 1) * 512],
                                   in_=pm)
                if a == 1:
                    # wq2 issued mid-Vt from the scalar stream
                    wq2_sb = wts_p.tile([P, DT * D], bf16, tag="wts")
                    nc.scalar.dma_start(out=wq2_sb, in_=h_wq2[:])

            # ---- causal self-attention: scores + softmax (all qt) ----
            Pbs = []
            rinv1 = stat_p.tile([P, ST], f32, tag="rinv")
            for qt in range(ST):
                width = (qt + 1) * P
                pm = ps.tile([P, 512], f32, tag="ps")
                for k in range(DT):
                    nc.tensor.matmul(pm[:, :width],
                                     lhsT=QT[:, k, qt * P:(qt + 1) * P],
                                     rhs=KT[:, k, :width],
                                     start=(k == 0), stop=(k == DT - 1))
                # mask the diagonal block in place (PSUM RMW)
                nc.vector.tensor_tensor(out=pm[:, qt * P:width],
                                        in0=pm[:, qt * P:width], in1=trimask,
                                        op=ALU.add)
                nmax = stat_p.tile([P, 1], f32, tag="nmax")
                nc.vector.reduce_max(nmax, pm[:, :width], axis=X, negate=True)
                Pb = pb_p.tile([P, 512], bf16, tag="pb", name=f"pb{qt}")
                rsum = stat_p.tile([P, 1], f32, tag="rsum")
                nc.scalar.activation(out=Pb[:, :width], in_=pm[:, :width],
                                     func=ACT_F.Exp, bias=nmax, scale=1.0,
                                     accum_out=rsum)
                nc.vector.reciprocal(out=rinv1[:, qt:qt + 1], in_=rsum)
                Pbs.append(Pb)

            # ---- cross-attn K2 (fills the softmax1 pipeline shadow) ----
            K2T = k2t_p.tile([P, DT, NI_PAD], bf16, tag="k2t")
            for m in range(DT):
                pm = ps.tile([P, 512], f32, tag="ps")
                for k in range(DIT):
                    nc.tensor.matmul(pm[:, :NI],
                                     lhsT=wk2c[m][:, k * P:(k + 1) * P],
                                     rhs=img_sb[:, k * NI:(k + 1) * NI],
                                     start=(k == 0), stop=(k == DIT - 1))
                nc.scalar.activation(out=K2T[:, m, :NI], in_=pm[:, :NI],
                                     func=ACT_F.Identity,
                                     bias=bk2s[:, m:m + 1], scale=1.0)

            def layernorm(xpre, out_sl, gb):
                """xpre [P, D] f32 -> out_sl [P, D] bf16.

                Writes the normalized rows times gb (or raw normalized rows if
                gb is None — affine folded into the consumers)."""
                stats = stat_p.tile([P, 2, 6], f32, tag="bnst")
                for sg in range(2):
                    nc.vector.bn_stats(out=stats[:, sg, :],
                                       in_=xpre[:, sg * 512:(sg + 1) * 512])
                mv = stat_p.tile([P, 2], f32, tag="bnmv")
                nc.vector.bn_aggr(out=mv, in_=stats)
                rstd = stat_p.tile([P, 1], f32, tag="rstd")
                nc.scalar.activation(out=rstd, in_=mv[:, 1:2], func=ACT_F.Sqrt,
                                     bias=epst, scale=1.0)
                nc.vector.reciprocal(out=rstd, in_=rstd)
                nmr = stat_p.tile([P, 1], f32, tag="nmr")
                nc.vector.scalar_tensor_tensor(
                    out=nmr, in0=mv[:, 0:1], scalar=rstd, in1=neg1,
                    op0=ALU.mult, op1=ALU.mult)
                if gb is None:
                    nc.scalar.activation(out=out_sl, in_=xpre,
                                         func=ACT_F.Identity,
                                         bias=nmr, scale=rstd)
                else:
                    nc.scalar.activation(out=xpre, in_=xpre,
                                         func=ACT_F.Identity,
                                         bias=nmr, scale=rstd)
                    nc.gpsimd.tensor_tensor(out=out_sl, in0=xpre,
                                            in1=gb, op=ALU.mult)

            # ---- AV1 + residual + LN1 per qt ----
            PT = pt_p.tile([P, ST, S], bf16, tag="pt")
            x1b = xb_p.tile([P, ST, D], bf16, tag="xb", name="x1b")
            wp_pre = {}
            bp_pre = {}
            # cross-attn V2 blocks are interleaved below as PE filler while
            # the VEC-bound AV1/LN1 chain drains
            V2t = v_p.tile([P, NIT, D], bf16, tag="v")
            nc.gpsimd.memset(V2t, 0.0)

            def v2_block(a, nh):
                pa = P if a == 0 else NI - P
                pm = ps.tile([P, 512], f32, tag="ps")
                for k in range(DIT):
                    nc.tensor.matmul(
                        pm[:pa, :],
                        lhsT=img_sb[:, k * NI + a * P:k * NI + a * P + pa],
                        rhs=wv2_sb[:, k * D + nh * 512:k * D + (nh + 1) * 512],
                        start=(k == 0), stop=(k == DIT - 1))
                nc.vector.tensor_tensor(
                    out=V2t[:pa, a, nh * 512:(nh + 1) * 512], in0=pm[:pa, :],
                    in1=bv2b[:pa, nh * 512:(nh + 1) * 512], op=ALU.add)

            for qt in range(ST):
                for kt in range(qt + 1):
                    tp = ps.tile([P, 512], bf16, tag="ps", name="tp")
                    nc.tensor.transpose(out=tp[:, :P],
                                        in_=Pbs[qt][:, kt * P:(kt + 1) * P],
                                        identity=ident)
                    nc.vector.tensor_copy(out=PT[:, kt, qt * P:(qt + 1) * P],
                                          in_=tp[:, :P])
                xpre = xpre_p.tile([P, D], f32, tag="xpre")
                for nh in range(2):
                    sl = slice(nh * 512, (nh + 1) * 512)
                    pm = ps.tile([P, 512], f32, tag="ps")
                    for kt in range(qt + 1):
                        nc.tensor.matmul(pm, lhsT=PT[:, kt, qt * P:(qt + 1) * P],
                                         rhs=Vt[:, kt, nh * 512:(nh + 1) * 512],
                                         start=(kt == 0), stop=(kt == qt))
                    # residual split: SCA rescale, then GPS add (keeps VEC free)
                    nc.scalar.activation(out=xpre[:, sl], in_=pm,
                                         func=ACT_F.Identity,
                                         scale=rinv1[:, qt:qt + 1])
                    nc.gpsimd.tensor_tensor(
                        out=xpre[:, sl], in0=xpre[:, sl],
                        in1=x0b[:, qt * D + nh * 512:qt * D + (nh + 1) * 512],
                        op=ALU.add)
                v2_block(qt % NIT, qt // NIT)  # PE filler during stt/LN1
                layernorm(xpre, x1b[:, qt, :], g1b)
                if qt == 0:
                    # prefetch the first vocab chunks + bias strip now that
                    # the startup DMA window has drained (gpsimd stream pos)
                    for c in range(3):
                        t = wp_p.tile([P, DT * CN], bf16, tag="wp",
                                      name=f"wp_pre{c}")
                        nc.gpsimd.dma_start(out=t, in_=h_wp[c])
                        wp_pre[c] = t
                    bp0 = bp_p.tile([P, GRP * CN], bf16, tag="bp",
                                    name="bp_pre0")
                    nc.gpsimd.dma_start(out=bp0, in_=bcast(h_bp, GRP * CN))
                    bp_pre[0] = bp0

            # ---- layer 2, pipelined in qt-pair halves ----
            def transpose_cols(dst, src_b, a_list, tag):
                """transpose x[P, a, db*P:(db+1)*P] -> dst[:, db, a*P:(a+1)*P]."""
                for a in a_list:
                    for db in range(DT):
                        tp = ps.tile([P, 512], bf16, tag="ps", name=tag)
                        nc.tensor.transpose(
                            out=tp[:, :P],
                            in_=src_b[:, a, db * P:(db + 1) * P],
                            identity=ident)
                        nc.scalar.copy(
                            out=dst[:, db, a * P:(a + 1) * P], in_=tp[:, :P])

            def proj_T_into(o, w_sb, b_sb, rhsT, c0, c1, kt=DT):
                w = c1 - c0
                for m in range(DT):
                    pm = ps.tile([P, 512], f32, tag="ps", name="pm")
                    for k in range(kt):
                        nc.tensor.matmul(
                            pm[:, :w],
                            lhsT=w_sb[:, k * D + m * P:k * D + (m + 1) * P],
                            rhs=rhsT[:, k, c0:c1],
                            start=(k == 0), stop=(k == kt - 1))
                    nc.scalar.activation(out=o[:, m, c0:c1], in_=pm[:, :w],
                                         func=ACT_F.Identity,
                                         bias=b_sb[:, m:m + 1], scale=1.0)

            x1T = xt_p.tile([P, DT, S], bf16, tag="xt", name="x1t")
            Q2T = qk_p.tile([P, DT, S], bf16, tag="qk", name="q2t")
            P2bs = []
            rinv2 = stat_p.tile([P, ST], f32, tag="rinv2")

            def scores2_softmax(qt):
                pm = ps.tile([P, 512], f32, tag="ps")
                for k in range(DT):
                    nc.tensor.matmul(pm[:, :NI],
                                     lhsT=Q2T[:, k, qt * P:(qt + 1) * P],
                                     rhs=K2T[:, k, :NI],
                                     start=(k == 0), stop=(k == DT - 1))
                nmax = stat_p.tile([P, 1], f32, tag="nmax")
                nc.vector.reduce_max(nmax, pm[:, :NI], axis=X, negate=True)
                P2b = pb_p.tile([P, NI_PAD], bf16, tag="pb2", name=f"p2b{qt}")
                nc.gpsimd.memset(P2b[:, NI:], 0.0)
                rsum = stat_p.tile([P, 1], f32, tag="rsum")
                nc.scalar.activation(out=P2b[:, :NI], in_=pm[:, :NI],
                                     func=ACT_F.Exp, bias=nmax, scale=1.0,
                                     accum_out=rsum)
                nc.vector.reciprocal(out=rinv2[:, qt:qt + 1], in_=rsum)
                P2bs.append(P2b)

            # first half: qt 0,1
            transpose_cols(x1T, x1b, (0, 1), "x1t_tp")
            proj_T_into(Q2T, wq2_sb, bq2s, x1T, 0, 256)
            scores2_softmax(0)
            scores2_softmax(1)
            # second half: qt 2,3
            transpose_cols(x1T, x1b, (2, 3), "x1t_tp")
            proj_T_into(Q2T, wq2_sb, bq2s, x1T, 256, 512)
            scores2_softmax(2)
            scores2_softmax(3)

            # ---- AV2 + residual + LN2 per qt (normalized out; affine folded
            # into Wp/bp) ----
            PT2 = pt_p.tile([P, NIT, S], bf16, tag="pt2")
            x2b = xb_p.tile([P, ST, D], bf16, tag="xb", name="x2b")

            def av2_ln2(qt):
                for kt in range(NIT):
                    tp = ps.tile([P, 512], bf16, tag="ps", name="tp2")
                    nc.tensor.transpose(out=tp[:, :P],
                                        in_=P2bs[qt][:, kt * P:(kt + 1) * P],
                                        identity=ident)
                    nc.vector.tensor_copy(out=PT2[:, kt, qt * P:(qt + 1) * P],
                                          in_=tp[:, :P])
                xpre = xpre_p.tile([P, D], f32, tag="xpre")
                for nh in range(2):
                    sl = slice(nh * 512, (nh + 1) * 512)
                    pm = ps.tile([P, 512], f32, tag="ps")
                    for kt in range(NIT):
                        nc.tensor.matmul(pm, lhsT=PT2[:, kt, qt * P:(qt + 1) * P],
                                         rhs=V2t[:, kt, nh * 512:(nh + 1) * 512],
                                         start=(kt == 0), stop=(kt == NIT - 1))
                    nc.scalar.activation(out=xpre[:, sl], in_=pm,
                                         func=ACT_F.Identity,
                                         scale=rinv2[:, qt:qt + 1])
                    nc.gpsimd.tensor_tensor(out=xpre[:, sl], in0=xpre[:, sl],
                                            in1=x1b[:, qt, sl], op=ALU.add)
                layernorm(xpre, x2b[:, qt, :], None)

            x2T = xt_p.tile([P, DT, S], bf16, tag="xt", name="x2t")

            for qt in range(ST):
                av2_ln2(qt)
            transpose_cols(x2T, x2b, (0, 1), "x2t_tp")

            # ---- vocab projection, streamed in CN-column chunks ----
            def vocab_chunks(chunks, qts, dma_par, pre=()):
                """Process wp[chunks] x qts; chunks must align to GRP groups."""
                for gi in range(0, len(chunks), GRP):
                    g = chunks[gi] // GRP
                    if g in bp_pre:
                        bp_bc = bp_pre.pop(g)
                    else:
                        bp_bc = bp_p.tile([P, GRP * CN], bf16, tag="bp")
                        nc.gpsimd.dma_start(out=bp_bc,
                                            in_=bcast(h_bp, GRP * CN,
                                                      offset=g * GRP * CN))
                    osb = {q: osb_p.tile([P, GRP * CN], bf16, tag="osb",
                                         name=f"osb_{g}_{q}")
                           for q in qts}
                    for cc in range(GRP):
                        c = chunks[gi + cc]
                        if c in pre and c in wp_pre:
                            wp_sb = wp_pre.pop(c)
                        else:
                            wp_sb = wp_p.tile([P, DT * CN], bf16, tag="wp")
                            dma_eng = (nc.sync if (c + dma_par) % 2 == 0
                                       else nc.scalar)
                            dma_eng.dma_start(out=wp_sb, in_=h_wp[c])
                        for qt in qts:
                            pm = ps.tile([P, 512], f32, tag="ps")
                            for k in range(DT):
                                nc.tensor.matmul(
                                    pm, lhsT=x2T[:, k, qt * P:(qt + 1) * P],
                                    rhs=wp_sb[:, k * CN:(k + 1) * CN],
                                    start=(k == 0), stop=(k == DT - 1))
                            nc.vector.tensor_tensor(
                                out=osb[qt][:, cc * CN:(cc + 1) * CN], in0=pm,
                                in1=bp_bc[:, cc * CN:(cc + 1) * CN], op=ALU.add)
                            if cc == GRP - 1:
                                # fire each strip as soon as it completes
                                out_eng = nc.sync if qt < 2 else nc.scalar
                                out_eng.dma_start(
                                    out=h_out[qt * P:(qt + 1) * P,
                                              g * GRP * CN:(g + 1) * GRP * CN],
                                    in_=osb[qt])

            # early pass: first NE chunks for qt {0,1} while LN2(2,3) drains
            vocab_chunks(list(range(NE)), (0, 1), 0, pre=(0, 1, 2))
            transpose_cols(x2T, x2b, (2, 3), "x2t_tp")
            # late pass for those chunks' qt {2,3} (re-streamed), then the rest
            vocab_chunks(list(range(NE)), (2, 3), 1)
            vocab_chunks(list(range(NE, NCHUNK)), (0, 1, 2, 3), 0)

    nc.compile()
    return nc


def _tile_sq(w, kt):
    """[K, N] -> [128, K//128, N] contiguous."""
    k, n = w.shape
    assert k == kt * P
    return np.ascontiguousarray(
        w.reshape(kt, P, n).transpose(1, 0, 2)).astype(BF16)


def _pos_enc():
    posn = np.arange(S)[:, None].astype(np.float32)
    i = np.arange(0, D, 2).astype(np.float32)
    ang = posn / np.power(10000.0, i / D)
    pos = np.zeros((S, D), dtype=np.float32)
    pos[:, 0::2] = np.sin(ang)
    pos[:, 1::2] = np.cos(ang)
    return pos


def _prep_inputs(inputs):
    g = lambda name: np.asarray(inputs[name], dtype=np.float32)
    tokens = np.asarray(inputs["tokens"]).astype(np.int64)
    img = g("img_emb")
    table = g("emb_table")
    pos = _pos_enc()

    b1 = g("b1")
    g2 = g("g2")
    b2 = g("b2")
    wp = g("Wp") * g2[:, None]          # fold LN2 gamma
    wp_pad = np.zeros((D, VP), dtype=np.float32)
    wp_pad[:, :V] = wp
    wp_t = np.ascontiguousarray(
        wp_pad.reshape(DT, P, NCHUNK, CN).transpose(2, 1, 0, 3)).astype(BF16)
    bp_pad = np.zeros((VP,), dtype=np.float32)
    bp_pad[:V] = g("bp") + b2 @ g("Wp")  # fold LN2 beta
    bp_pad = bp_pad.astype(BF16)

    def bias_tiled(b):
        return np.ascontiguousarray(b.reshape(DT, P).T).astype(np.float32)

    def chunk_m(tiled):
        """[P, kt, D] -> [DT, P, kt*128] contiguous chunks of output cols."""
        return np.ascontiguousarray(
            np.stack([tiled[:, :, m * P:(m + 1) * P].reshape(P, -1)
                      for m in range(DT)]))

    shared = {
        "wq1c": chunk_m(_tile_sq(g("Wq1") * SCALE, DT)),
        "wk1c": chunk_m(_tile_sq(g("Wk1"), DT)),
        "wv1": _tile_sq(g("Wv1"), DT).reshape(P, -1),
        "wq2": _tile_sq(g("Wq2") * SCALE, DT).reshape(P, -1),
        "wk2c": chunk_m(_tile_sq(g("Wk2"), DIT)),
        "wv2": _tile_sq(g("Wv2"), DIT).reshape(P, -1),
        "wp": wp_t.reshape(NCHUNK, P, -1),
        "bq1": bias_tiled(g("bq1") * SCALE),
        "bk1": bias_tiled(g("bk1")),
        # fold LN1 beta into the cross-attn query bias and value bias
        "bq2": bias_tiled((g("bq2") + b1 @ g("Wq2")) * SCALE),
        "bk2": bias_tiled(g("bk2")),
        "bv2": g("bv2") + b1,
        "g1": g("g1"),
        "bp": bp_pad,
    }
    in_maps = []
    for c in range(N_CORES):
        m = dict(shared)
        x0 = table[tokens[c]] + pos                      # [S, D] f32
        # bv1 folded into the self-attn residual (A1 rows sum to 1)
        x0r = x0 + g("bv1")
        x0b = np.ascontiguousarray(
            x0r.reshape(ST, P, D).transpose(1, 0, 2)).astype(BF16)
        x0T = np.ascontiguousarray(
            x0.T.reshape(DT, P, S).transpose(1, 0, 2)).astype(BF16)
        m["x0b"] = x0b.reshape(P, -1)
        m["x0t"] = x0T.reshape(P, -1)
        m["img_t"] = np.ascontiguousarray(
            img[c].T.reshape(DIT, P, NI).transpose(1, 0, 2)).astype(
                BF16).reshape(P, -1)
        in_maps.append(m)
    return in_maps


def _ensure_axon_hooks():
    """bass_utils imports antenv.axon_hooks when BASS_TRACE is set; stub it
    if the module is absent so tracing degrades instead of crashing."""
    try:
        import antenv.axon_hooks  # noqa: F401
    except ImportError:
        import types
        mod = types.ModuleType("antenv.axon_hooks")
        mod.get_axon_ntff_profile_hook = lambda: None
        mod.set_axon_ntff_profile_hook = lambda h: None
        sys.modules["antenv.axon_hooks"] = mod


def kernel(**inputs):
    global LAST_RESULTS
    _ensure_axon_hooks()
    from concourse.bass_utils import run_bass_kernel_spmd

    if "nc" not in _CACHE:
        _CACHE["nc"] = _build_program()
    nc = _CACHE["nc"]

    in_maps = _prep_inputs(inputs)
    res = run_bass_kernel_spmd(nc, in_maps, core_ids=list(range(N_CORES)))
    LAST_RESULTS = res
    out = np.stack([res.results[c]["out"][:, :V].astype(np.float32)
                    for c in range(N_CORES)])
    return out


# revision 50
# speedup vs baseline: 1.0572x; 1.0298x over previous
"""Trainium2 Bass kernel for an 8-batch image-conditioned decoder layer.

Strategy: pure data-parallel over the batch — core c computes batch element c
end-to-end (causal self-attention, cross-attention over the image tokens, both
layernorms, vocab projection). No collectives.

Schedule notes:
- Embedding gather + positional encoding are host-prepped (pure data movement);
  the device receives x0 in both seq-partition and d-partition layouts.
- Every DMA-touched tensor is laid out 2D ([P, free]) so each transfer lowers
  to a single DIRECT2D descriptor (3D APs cost one issue slot per outer index,
  ~0.6us of engine time each).
- Dummy warmup matmuls run during the initial DMA window so the PE HAM clock
  gate is at full rate when QT starts.
- Q/K weights stream as per-m-group chunks just-in-time, paced by the compute
  stream, so early HBM bandwidth stays focused on the critical path; larger
  secondary tensors are released behind gates keyed on QT/KT progress.
- b1 is folded into bq2/bv2, bv1 into the residual copy of x0, and g2/b2 into
  Wp/bp, which trims the layernorm critical path.
- Layer 2 runs per-qt-pipelined (Q2T in two 256-col halves) and the first
  vocab chunks are computed early for qt {0,1}; those chunks are re-streamed
  later for qt {2,3}.
- PSUM->SBUF moves run on Scalar; SBUF-only elementwise work on GpSimd; VEC
  keeps softmax/bn_stats and the vocab bias adds.

All matmuls run in bf16 with fp32 PSUM accumulation.
"""

import os
import sys

for _p in ("/opt/trn_rl_repo", "/root/.axon_site/_ro/trn_rl_repo"):
    if os.path.isdir(_p) and _p not in sys.path:
        sys.path.append(_p)

import numpy as np
import ml_dtypes

BF16 = ml_dtypes.bfloat16

# Problem dims (hardcoded per spec)
V, D, DI, S, B, NI = 32000, 1024, 768, 512, 8, 197
EPS = 1e-5
P = 128
ST = S // P          # 4 seq tiles
DT = D // P          # 8 model-dim tiles
DIT = DI // P        # 6 image-dim tiles
NIT = 2              # image tokens: 197 -> 2 partition tiles (128 + 69)
NI_PAD = 256
VP = 32768           # vocab padded to 64 chunks of 512
CN = 512             # vocab chunk width
NCHUNK = VP // CN    # 64
GRP = 2              # chunks per output strip
NGRP = NCHUNK // GRP
NE = 4               # chunks computed early for qt {0,1} (re-streamed later)
N_CORES = 8
SCALE = 1.0 / float(np.sqrt(np.float32(D)))

_CACHE = {}
LAST_RESULTS = None


def _build_program():
    import concourse.bacc as bacc
    import concourse.bass as bass
    import concourse.mybir as mybir
    from concourse.masks import make_identity
    from concourse.tile import TileContext

    f32 = mybir.dt.float32
    bf16 = mybir.dt.bfloat16
    X = mybir.AxisListType.X
    ALU = mybir.AluOpType
    ACT_F = mybir.ActivationFunctionType

    nc = bacc.Bacc("TRN2", target_bir_lowering=False, debug=False,
                   num_devices=N_CORES)

    # ---- I/O (all 2D so every DMA is a single DIRECT2D) ----
    h_x0b = nc.dram_tensor("x0b", [P, ST * D], bf16, kind="ExternalInput")
    h_x0T = nc.dram_tensor("x0t", [P, DT * S], bf16, kind="ExternalInput")
    h_img = nc.dram_tensor("img_t", [P, DIT * NI], bf16, kind="ExternalInput")
    h_wq1 = nc.dram_tensor("wq1", [P, DT * D], bf16, kind="ExternalInput")
    h_wk1 = nc.dram_tensor("wk1", [P, DT * D], bf16, kind="ExternalInput")
    h_wv1 = nc.dram_tensor("wv1", [P, DT * D], bf16, kind="ExternalInput")
    h_wq2 = nc.dram_tensor("wq2", [P, DT * D], bf16, kind="ExternalInput")
    h_wk2 = nc.dram_tensor("wk2", [P, DIT * D], bf16, kind="ExternalInput")
    h_wv2 = nc.dram_tensor("wv2", [P, DIT * D], bf16, kind="ExternalInput")
    h_wp = nc.dram_tensor("wp", [NCHUNK, P, DT * CN], bf16,
                          kind="ExternalInput")
    h_bq1 = nc.dram_tensor("bq1", [P, DT], f32, kind="ExternalInput")
    h_bk1 = nc.dram_tensor("bk1", [P, DT], f32, kind="ExternalInput")
    h_bq2 = nc.dram_tensor("bq2", [P, DT], f32, kind="ExternalInput")
    h_bk2 = nc.dram_tensor("bk2", [P, DT], f32, kind="ExternalInput")
    h_bv2 = nc.dram_tensor("bv2", [D], f32, kind="ExternalInput")
    h_g1 = nc.dram_tensor("g1", [D], f32, kind="ExternalInput")
    h_bp = nc.dram_tensor("bp", [VP], bf16, kind="ExternalInput")
    h_out = nc.dram_tensor("out", [S, VP], bf16, kind="ExternalOutput")

    def bcast(handle, n, offset=0):
        ap = handle[:]
        return bass.AP(tensor=ap.tensor, offset=offset, ap=[[0, P], [1, n]])

    with TileContext(nc) as tc:
        import contextlib
        ctx = contextlib.ExitStack()
        with ctx:
            const = ctx.enter_context(tc.tile_pool(name="const", bufs=1))
            xb_p = ctx.enter_context(tc.tile_pool(name="xb", bufs=3))
            xt_p = ctx.enter_context(tc.tile_pool(name="xt", bufs=2))
            qk_p = ctx.enter_context(tc.tile_pool(name="qk", bufs=2))
            v_p = ctx.enter_context(tc.tile_pool(name="vp", bufs=2))
            k2t_p = ctx.enter_context(tc.tile_pool(name="k2t", bufs=1))
            pb_p = ctx.enter_context(tc.tile_pool(name="pb", bufs=4))
            pt_p = ctx.enter_context(tc.tile_pool(name="pt", bufs=1))
            xpre_p = ctx.enter_context(tc.tile_pool(name="xpre", bufs=2))
            stat_p = ctx.enter_context(tc.tile_pool(name="stat", bufs=4))
            wts_p = ctx.enter_context(tc.tile_pool(name="wts", bufs=2))
            wv2_p = ctx.enter_context(tc.tile_pool(name="wv2p", bufs=1))
            wp_p = ctx.enter_context(tc.tile_pool(name="wpp", bufs=3))
            bp_p = ctx.enter_context(tc.tile_pool(name="bpp", bufs=2))
            osb_p = ctx.enter_context(tc.tile_pool(name="osb", bufs=6))
            ps = ctx.enter_context(tc.tile_pool(name="ps", bufs=8, space="PSUM"))

            # ---- constants / warmup ----
            ident = const.tile([P, P], bf16)
            make_identity(nc, ident)
            trimask = const.tile([P, P], f32)
            nc.gpsimd.memset(trimask, 0.0)
            nc.gpsimd.affine_select(
                out=trimask, in_=trimask, compare_op=ALU.is_ge, fill=-1e10,
                base=0, pattern=[[-1, P]], channel_multiplier=1)
            warm_src = const.tile([P, 256], bf16)
            nc.vector.memset(warm_src, 0.0)
            epst = const.tile([P, 1], f32)
            nc.vector.memset(epst, EPS)
            neg1 = const.tile([P, 1], f32)
            nc.vector.memset(neg1, -1.0)

            # HAM warmup: keep the PE busy while the first weights stream in
            for w in range(28):
                pw = ps.tile([P, 512], f32, tag="ps", name=f"warm{w}")
                nc.tensor.matmul(pw[:, :256], lhsT=ident, rhs=warm_src,
                                 start=True, stop=True)

            # ---- early DMAs ----
            x0T = xt_p.tile([P, DT * S], bf16, tag="xt", name="x0t")
            nc.sync.dma_start(out=x0T, in_=h_x0T[:])
            bq1s = const.tile([P, DT], f32)
            bk1s = const.tile([P, DT], f32)
            bq2s = const.tile([P, DT], f32)
            bk2s = const.tile([P, DT], f32)
            for t, h in ((bq1s, h_bq1), (bk1s, h_bk1), (bq2s, h_bq2),
                         (bk2s, h_bk2)):
                nc.gpsimd.dma_start(out=t, in_=h[:])

            # Weight loads: whole tensors through a 2-slot ring. The ring-slot
            # waits stagger the transfers automatically (wv1 fires only once
            # QT has consumed wq1, wq2 once KT is done), so the early HBM
            # window belongs to x0T+wq1+wk1 alone. The blocked DMAs sit on the
            # otherwise-idle sync stream.
            wq1_sb = wts_p.tile([P, DT * D], bf16, tag="wts")
            nc.scalar.dma_start(out=wq1_sb, in_=h_wq1[:])
            wk1_sb = wts_p.tile([P, DT * D], bf16, tag="wts")
            nc.sync.dma_start(out=wk1_sb, in_=h_wk1[:])
            wv1_sb = wts_p.tile([P, DT * D], bf16, tag="wts")
            nc.sync.dma_start(out=wv1_sb, in_=h_wv1[:])
            wq2_sb = wts_p.tile([P, DT * D], bf16, tag="wts")
            nc.sync.dma_start(out=wq2_sb, in_=h_wq2[:])

            def proj_T(w_sb, b_sb, name):
                """out[P, DT, S] bf16 = (W.T @ x0.T) + b, d-partition."""
                o = qk_p.tile([P, DT, S], bf16, tag="qk", name=name)
                for m in range(DT):
                    pm = ps.tile([P, 512], f32, tag="ps", name="pm")
                    for k in range(DT):
                        nc.tensor.matmul(
                            pm,
                            lhsT=w_sb[:, k * D + m * P:k * D + (m + 1) * P],
                            rhs=x0T[:, k * S:(k + 1) * S],
                            start=(k == 0), stop=(k == DT - 1))
                    nc.scalar.activation(out=o[:, m, :], in_=pm,
                                         func=ACT_F.Identity,
                                         bias=b_sb[:, m:m + 1], scale=1.0)
                return o

            QT = proj_T(wq1_sb, bq1s, "qt")

            # gpsimd-gated DMAs: released once QT compute is underway so they
            # don't steal HBM bandwidth from the critical startup path
            gate_t = const.tile([P, 1], bf16)
            nc.gpsimd.tensor_copy(out=gate_t, in_=QT[:, 0, 0:1])
            img_sb = const.tile([P, DIT * NI], bf16)
            nc.gpsimd.dma_start(out=img_sb, in_=h_img[:])
            wk2_sb = wv2_p.tile([P, DIT * D], bf16, tag="wk2")
            nc.gpsimd.dma_start(out=wk2_sb, in_=h_wk2[:])

            KT = proj_T(wk1_sb, bk1s, "kt")

            gate_t2 = const.tile([P, 1], bf16)
            nc.gpsimd.tensor_copy(out=gate_t2, in_=KT[:, 0, 0:1])
            x0b = xb_p.tile([P, ST * D], bf16, tag="xb", name="x0b")
            nc.gpsimd.dma_start(out=x0b, in_=h_x0b[:])
            wv2_sb = wv2_p.tile([P, DIT * D], bf16, tag="wv2")
            nc.gpsimd.dma_start(out=wv2_sb, in_=h_wv2[:])
            g1b = const.tile([P, D], f32)
            bv2b = const.tile([P, D], f32)
            for t, h in ((g1b, h_g1), (bv2b, h_bv2)):
                nc.gpsimd.dma_start(out=t, in_=bcast(h, D))

            # value projection (bv1 folded into x0b host-side; attention rows
            # are convex combinations so the V-bias passes through unchanged)
            Vt = v_p.tile([P, ST, D], bf16, tag="v")
            for a in range(ST):
                for nh in range(2):
                    pm = ps.tile([P, 512], f32, tag="ps")
                    for k in range(DT):
                        nc.tensor.matmul(
                            pm,
                            lhsT=x0T[:, k * S + a * P:k * S + (a + 1) * P],
                            rhs=wv1_sb[:, k * D + nh * 512:
                                       k * D + (nh + 1) * 512],
                            start=(k == 0), stop=(k == DT - 1))
                    nc.scalar.copy(out=Vt[:, a, nh * 512:(nh + 1) * 512],
                                   in_=pm)

            # ---- causal self-attention: scores + softmax (all qt) ----
            Pbs = []
            rinv1 = stat_p.tile([P, ST], f32, tag="rinv")
            for qt in range(ST):
                width = (qt + 1) * P
                pm = ps.tile([P, 512], f32, tag="ps")
                for k in range(DT):
                    nc.tensor.matmul(pm[:, :width],
                                     lhsT=QT[:, k, qt * P:(qt + 1) * P],
                                     rhs=KT[:, k, :width],
                                     start=(k == 0), stop=(k == DT - 1))
                # mask the diagonal block in place (PSUM RMW)
                nc.vector.tensor_tensor(out=pm[:, qt * P:width],
                                        in0=pm[:, qt * P:width], in1=trimask,
                                        op=ALU.add)
                nmax = stat_p.tile([P, 1], f32, tag="nmax")
                nc.vector.reduce_max(nmax, pm[:, :width], axis=X, negate=True)
                Pb = pb_p.tile([P, 512], bf16, tag="pb", name=f"pb{qt}")
                rsum = stat_p.tile([P, 1], f32, tag="rsum")
                nc.scalar.activation(out=Pb[:, :width], in_=pm[:, :width],
                                     func=ACT_F.Exp, bias=nmax, scale=1.0,
                                     accum_out=rsum)
                nc.vector.reciprocal(out=rinv1[:, qt:qt + 1], in_=rsum)
                Pbs.append(Pb)

            # ---- cross-attn K2 (fills the softmax1 pipeline shadow) ----
            K2T = k2t_p.tile([P, DT, NI_PAD], bf16, tag="k2t")
            for m in range(DT):
                pm = ps.tile([P, 512], f32, tag="ps")
                for k in range(DIT):
                    nc.tensor.matmul(
                        pm[:, :NI],
                        lhsT=wk2_sb[:, k * D + m * P:k * D + (m + 1) * P],
                        rhs=img_sb[:, k * NI:(k + 1) * NI],
                        start=(k == 0), stop=(k == DIT - 1))
                nc.scalar.activation(out=K2T[:, m, :NI], in_=pm[:, :NI],
                                     func=ACT_F.Identity,
                                     bias=bk2s[:, m:m + 1], scale=1.0)

            def layernorm(xpre, out_sl, gb):
                """xpre [P, D] f32 -> out_sl [P, D] bf16.

                Writes the normalized rows times gb (or raw normalized rows if
                gb is None — affine folded into the consumers)."""
                stats = stat_p.tile([P, 2, 6], f32, tag="bnst")
                for sg in range(2):
                    nc.vector.bn_stats(out=stats[:, sg, :],
                                       in_=xpre[:, sg * 512:(sg + 1) * 512])
                mv = stat_p.tile([P, 2], f32, tag="bnmv")
                nc.vector.bn_aggr(out=mv, in_=stats)
                rstd = stat_p.tile([P, 1], f32, tag="rstd")
                nc.scalar.activation(out=rstd, in_=mv[:, 1:2], func=ACT_F.Sqrt,
                                     bias=epst, scale=1.0)
                nc.vector.reciprocal(out=rstd, in_=rstd)
                nmr = stat_p.tile([P, 1], f32, tag="nmr")
                nc.vector.scalar_tensor_tensor(
                    out=nmr, in0=mv[:, 0:1], scalar=rstd, in1=neg1,
                    op0=ALU.mult, op1=ALU.mult)
                if gb is None:
                    nc.scalar.activation(out=out_sl, in_=xpre,
                                         func=ACT_F.Identity,
                                         bias=nmr, scale=rstd)
                else:
                    nc.scalar.activation(out=xpre, in_=xpre,
                                         func=ACT_F.Identity,
                                         bias=nmr, scale=rstd)
                    nc.gpsimd.tensor_tensor(out=out_sl, in0=xpre,
                                            in1=gb, op=ALU.mult)

            # ---- AV1 + residual + LN1 per qt ----
            PT = pt_p.tile([P, ST, S], bf16, tag="pt")
            x1b = xb_p.tile([P, ST, D], bf16, tag="xb", name="x1b")
            wp_pre = {}
            bp_pre = {}
            # cross-attn V2 blocks are interleaved below as PE filler while
            # the VEC-bound AV1/LN1 chain drains
            V2t = v_p.tile([P, NIT, D], bf16, tag="v")
            nc.gpsimd.memset(V2t, 0.0)

            def v2_block(a, nh):
                pa = P if a == 0 else NI - P
                pm = ps.tile([P, 512], f32, tag="ps")
                for k in range(DIT):
                    nc.tensor.matmul(
                        pm[:pa, :],
                        lhsT=img_sb[:, k * NI + a * P:k * NI + a * P + pa],
                        rhs=wv2_sb[:, k * D + nh * 512:k * D + (nh + 1) * 512],
                        start=(k == 0), stop=(k == DIT - 1))
                nc.vector.tensor_tensor(
                    out=V2t[:pa, a, nh * 512:(nh + 1) * 512], in0=pm[:pa, :],
                    in1=bv2b[:pa, nh * 512:(nh + 1) * 512], op=ALU.add)

            for qt in range(ST):
                for kt in range(qt + 1):
                    tp = ps.tile([P, 512], bf16, tag="ps", name="tp")
                    nc.tensor.transpose(out=tp[:, :P],
                                        in_=Pbs[qt][:, kt * P:(kt + 1) * P],
                                        identity=ident)
                    nc.vector.tensor_copy(out=PT[:, kt, qt * P:(qt + 1) * P],
                                          in_=tp[:, :P])
                xpre = xpre_p.tile([P, D], f32, tag="xpre")
                for nh in range(2):
                    sl = slice(nh * 512, (nh + 1) * 512)
                    pm = ps.tile([P, 512], f32, tag="ps")
                    for kt in range(qt + 1):
                        nc.tensor.matmul(pm, lhsT=PT[:, kt, qt * P:(qt + 1) * P],
                                         rhs=Vt[:, kt, nh * 512:(nh + 1) * 512],
                                         start=(kt == 0), stop=(kt == qt))
                    # residual split: SCA rescale, then GPS add (keeps VEC free)
                    nc.scalar.activation(out=xpre[:, sl], in_=pm,
                                         func=ACT_F.Identity,
                                         scale=rinv1[:, qt:qt + 1])
                    nc.gpsimd.tensor_tensor(
                        out=xpre[:, sl], in0=xpre[:, sl],
                        in1=x0b[:, qt * D + nh * 512:qt * D + (nh + 1) * 512],
                        op=ALU.add)
                v2_block(qt % NIT, qt // NIT)  # PE filler during stt/LN1
                layernorm(xpre, x1b[:, qt, :], g1b)
                if qt == 0:
                    # prefetch the first vocab chunks + bias strip now that
                    # the startup DMA window has drained (gpsimd stream pos)
                    for c in range(3):
                        t = wp_p.tile([P, DT * CN], bf16, tag="wp",
                                      name=f"wp_pre{c}")
                        nc.gpsimd.dma_start(out=t, in_=h_wp[c])
                        wp_pre[c] = t
                    bp0 = bp_p.tile([P, GRP * CN], bf16, tag="bp",
                                    name="bp_pre0")
                    nc.gpsimd.dma_start(out=bp0, in_=bcast(h_bp, GRP * CN))
                    bp_pre[0] = bp0

            # ---- layer 2, pipelined in qt-pair halves ----
            def transpose_cols(dst, src_b, a_list, tag):
                """transpose x[P, a, db*P:(db+1)*P] -> dst[:, db, a*P:(a+1)*P]."""
                for a in a_list:
                    for db in range(DT):
                        tp = ps.tile([P, 512], bf16, tag="ps", name=tag)
                        nc.tensor.transpose(
                            out=tp[:, :P],
                            in_=src_b[:, a, db * P:(db + 1) * P],
                            identity=ident)
                        nc.scalar.copy(
                            out=dst[:, db, a * P:(a + 1) * P], in_=tp[:, :P])

            def proj_T_into(o, w_sb, b_sb, rhsT, c0, c1, kt=DT):
                w = c1 - c0
                for m in range(DT):
                    pm = ps.tile([P, 512], f32, tag="ps", name="pm")
                    for k in range(kt):
                        nc.tensor.matmul(
                            pm[:, :w],
                            lhsT=w_sb[:, k * D + m * P:k * D + (m + 1) * P],
                            rhs=rhsT[:, k, c0:c1],
                            start=(k == 0), stop=(k == kt - 1))
                    nc.scalar.activation(out=o[:, m, c0:c1], in_=pm[:, :w],
                                         func=ACT_F.Identity,
                                         bias=b_sb[:, m:m + 1], scale=1.0)

            x1T = xt_p.tile([P, DT, S], bf16, tag="xt", name="x1t")
            Q2T = qk_p.tile([P, DT, S], bf16, tag="qk", name="q2t")
            P2bs = []
            rinv2 = stat_p.tile([P, ST], f32, tag="rinv2")

            def scores2_softmax(qt):
                pm = ps.tile([P, 512], f32, tag="ps")
                for k in range(DT):
                    nc.tensor.matmul(pm[:, :NI],
                                     lhsT=Q2T[:, k, qt * P:(qt + 1) * P],
                                     rhs=K2T[:, k, :NI],
                                     start=(k == 0), stop=(k == DT - 1))
                nmax = stat_p.tile([P, 1], f32, tag="nmax")
                nc.vector.reduce_max(nmax, pm[:, :NI], axis=X, negate=True)
                P2b = pb_p.tile([P, NI_PAD], bf16, tag="pb2", name=f"p2b{qt}")
                nc.gpsimd.memset(P2b[:, NI:], 0.0)
                rsum = stat_p.tile([P, 1], f32, tag="rsum")
                nc.scalar.activation(out=P2b[:, :NI], in_=pm[:, :NI],
                                     func=ACT_F.Exp, bias=nmax, scale=1.0,
                                     accum_out=rsum)
                nc.vector.reciprocal(out=rinv2[:, qt:qt + 1], in_=rsum)
                P2bs.append(P2b)

            # first half: qt 0,1
            transpose_cols(x1T, x1b, (0, 1), "x1t_tp")
            proj_T_into(Q2T, wq2_sb, bq2s, x1T, 0, 256)
            scores2_softmax(0)
            scores2_softmax(1)
            # second half: qt 2,3
            transpose_cols(x1T, x1b, (2, 3), "x1t_tp")
            proj_T_into(Q2T, wq2_sb, bq2s, x1T, 256, 512)
            scores2_softmax(2)
            scores2_softmax(3)

            # ---- AV2 + residual + LN2 per qt (normalized out; affine folded
            # into Wp/bp) ----
            PT2 = pt_p.tile([P, NIT, S], bf16, tag="pt2")
            x2b = xb_p.tile([P, ST, D], bf16, tag="xb", name="x2b")

            def av2_ln2(qt):
                for kt in range(NIT):
                    tp = ps.tile([P, 512], bf16, tag="ps", name="tp2")
                    nc.tensor.transpose(out=tp[:, :P],
                                        in_=P2bs[qt][:, kt * P:(kt + 1) * P],
                                        identity=ident)
                    nc.vector.tensor_copy(out=PT2[:, kt, qt * P:(qt + 1) * P],
                                          in_=tp[:, :P])
                xpre = xpre_p.tile([P, D], f32, tag="xpre")
                for nh in range(2):
                    sl = slice(nh * 512, (nh + 1) * 512)
                    pm = ps.tile([P, 512], f32, tag="ps")
                    for kt in range(NIT):
                        nc.tensor.matmul(pm, lhsT=PT2[:, kt, qt * P:(qt + 1) * P],
                                         rhs=V2t[:, kt, nh * 512:(nh + 1) * 512],
                                         start=(kt == 0), stop=(kt == NIT - 1))
                    nc.scalar.activation(out=xpre[:, sl], in_=pm,
                                         func=ACT_F.Identity,
                                         scale=rinv2[:, qt:qt + 1])
                    nc.gpsimd.tensor_tensor(out=xpre[:, sl], in0=xpre[:, sl],
                                            in1=x1b[:, qt, sl], op=ALU.add)
                layernorm(xpre, x2b[:, qt, :], None)

            x2T = xt_p.tile([P, DT, S], bf16, tag="xt", name="x2t")

            for qt in range(ST):
                av2_ln2(qt)
            transpose_cols(x2T, x2b, (0, 1), "x2t_tp")

            # ---- vocab projection, streamed in CN-column chunks ----
            def vocab_chunks(chunks, qts, dma_par, pre=()):
                """Process wp[chunks] x qts; chunks must align to GRP groups."""
                for gi in range(0, len(chunks), GRP):
                    g = chunks[gi] // GRP
                    if g in bp_pre:
                        bp_bc = bp_pre.pop(g)
                    else:
                        bp_bc = bp_p.tile([P, GRP * CN], bf16, tag="bp")
                        nc.gpsimd.dma_start(out=bp_bc,
                                            in_=bcast(h_bp, GRP * CN,
                                                      offset=g * GRP * CN))
                    osb = {q: osb_p.tile([P, GRP * CN], bf16, tag="osb",
                                         name=f"osb_{g}_{q}")
                           for q in qts}
                    for cc in range(GRP):
                        c = chunks[gi + cc]
                        if c in pre and c in wp_pre:
                            wp_sb = wp_pre.pop(c)
                        else:
                            wp_sb = wp_p.tile([P, DT * CN], bf16, tag="wp")
                            dma_eng = (nc.sync if (c + dma_par) % 2 == 0
                                       else nc.scalar)
                            dma_eng.dma_start(out=wp_sb, in_=h_wp[c])
                        for qt in qts:
                            pm = ps.tile([P, 512], f32, tag="ps")
                            for k in range(DT):
                                nc.tensor.matmul(
                                    pm, lhsT=x2T[:, k, qt * P:(qt + 1) * P],
                                    rhs=wp_sb[:, k * CN:(k + 1) * CN],
                                    start=(k == 0), stop=(k == DT - 1))
                            nc.vector.tensor_tensor(
                                out=osb[qt][:, cc * CN:(cc + 1) * CN], in0=pm,
                                in1=bp_bc[:, cc * CN:(cc + 1) * CN], op=ALU.add)
                            if cc == GRP - 1:
                                # fire each strip as soon as it completes
                                out_eng = nc.sync if qt < 2 else nc.scalar
                                out_eng.dma_start(
                                    out=h_out[qt * P:(qt + 1) * P,
                                              g * GRP * CN:(g + 1) * GRP * CN],
                                    in_=osb[qt])

            # early pass: first NE chunks for qt {0,1} while LN2(2,3) drains
            vocab_chunks(list(range(NE)), (0, 1), 0, pre=(0, 1, 2))
            transpose_cols(x2T, x2b, (2, 3), "x2t_tp")
            # late pass for those chunks' qt {2,3} (re-streamed), then the rest
            vocab_chunks(list(range(NE)), (2, 3), 1)
            vocab_chunks(list(range(NE, NCHUNK)), (0, 1, 2, 3), 0)

    nc.compile()
    return nc


def _tile_sq(w, kt):
    """[K, N] -> [128, K//128, N] contiguous."""
    k, n = w.shape
    assert k == kt * P
    return np.ascontiguousarray(
        w.reshape(kt, P, n).transpose(1, 0, 2)).astype(BF16)


def _pos_enc():
    posn = np.arange(S)[:, None].astype(np.float32)
    i = np.arange(0, D, 2).astype(np.float32)
    ang = posn / np.power(10000.0, i / D)
    pos = np.zeros((S, D), dtype=np.float32)
    pos[:, 0::2] = np.sin(ang)
    pos[:, 1::2] = np.cos(ang)
    return pos


def _prep_inputs(inputs):
    g = lambda name: np.asarray(inputs[name], dtype=np.float32)
    tokens = np.asarray(inputs["tokens"]).astype(np.int64)
    img = g("img_emb")
    table = g("emb_table")
    pos = _pos_enc()

    b1 = g("b1")
    g2 = g("g2")
    b2 = g("b2")
    wp = g("Wp") * g2[:, None]          # fold LN2 gamma
    wp_pad = np.zeros((D, VP), dtype=np.float32)
    wp_pad[:, :V] = wp
    wp_t = np.ascontiguousarray(
        wp_pad.reshape(DT, P, NCHUNK, CN).transpose(2, 1, 0, 3)).astype(BF16)
    bp_pad = np.zeros((VP,), dtype=np.float32)
    bp_pad[:V] = g("bp") + b2 @ g("Wp")  # fold LN2 beta
    bp_pad = bp_pad.astype(BF16)

    def bias_tiled(b):
        return np.ascontiguousarray(b.reshape(DT, P).T).astype(np.float32)

    shared = {
        "wq1": _tile_sq(g("Wq1") * SCALE, DT).reshape(P, -1),
        "wk1": _tile_sq(g("Wk1"), DT).reshape(P, -1),
        "wv1": _tile_sq(g("Wv1"), DT).reshape(P, -1),
        "wq2": _tile_sq(g("Wq2") * SCALE, DT).reshape(P, -1),
        "wk2": _tile_sq(g("Wk2"), DIT).reshape(P, -1),
        "wv2": _tile_sq(g("Wv2"), DIT).reshape(P, -1),
        "wp": wp_t.reshape(NCHUNK, P, -1),
        "bq1": bias_tiled(g("bq1") * SCALE),
        "bk1": bias_tiled(g("bk1")),
        # fold LN1 beta into the cross-attn query bias and value bias
        "bq2": bias_tiled((g("bq2") + b1 @ g("Wq2")) * SCALE),
        "bk2": bias_tiled(g("bk2")),
        "bv2": g("bv2") + b1,
        "g1": g("g1"),
        "bp": bp_pad,
    }
    in_maps = []
    for c in range(N_CORES):
        m = dict(shared)
        x0 = table[tokens[c]] + pos                      # [S, D] f32
        # bv1 folded into the self-attn residual (A1 rows sum to 1)
        x0r = x0 + g("bv1")
        x0b = np.ascontiguousarray(
            x0r.reshape(ST, P, D).transpose(1, 0, 2)).astype(BF16)
        x0T = np.ascontiguousarray(
            x0.T.reshape(DT, P, S).transpose(1, 0, 2)).astype(BF16)
        m["x0b"] = x0b.reshape(P, -1)
        m["x0t"] = x0T.reshape(P, -1)
        m["img_t"] = np.ascontiguousarray(
            img[c].T.reshape(DIT, P, NI).transpose(1, 0, 2)).astype(
                BF16).reshape(P, -1)
        in_maps.append(m)
    return in_maps


def _ensure_axon_hooks():
    """bass_utils imports antenv.axon_hooks when BASS_TRACE is set; stub it
    if the module is absent so tracing degrades instead of crashing."""
    try:
        import antenv.axon_hooks  # noqa: F401
    except ImportError:
        import types
        mod = types.ModuleType("antenv.axon_hooks")
        mod.get_axon_ntff_profile_hook = lambda: None
        mod.set_axon_ntff_profile_hook = lambda h: None
        sys.modules["antenv.axon_hooks"] = mod


def kernel(**inputs):
    global LAST_RESULTS
    _ensure_axon_hooks()
    from concourse.bass_utils import run_bass_kernel_spmd

    if "nc" not in _CACHE:
        _CACHE["nc"] = _build_program()
    nc = _CACHE["nc"]

    in_maps = _prep_inputs(inputs)
    res = run_bass_kernel_spmd(nc, in_maps, core_ids=list(range(N_CORES)))
    LAST_RESULTS = res
    out = np.stack([res.results[c]["out"][:, :V].astype(np.float32)
                    for c in range(N_CORES)])
    return out


# revision 55
# speedup vs baseline: 1.0584x; 1.0011x over previous
"""Trainium2 Bass kernel for an 8-batch image-conditioned decoder layer.

Strategy: pure data-parallel over the batch — core c computes batch element c
end-to-end (causal self-attention, cross-attention over the image tokens, both
layernorms, vocab projection). No collectives.

Schedule notes:
- Embedding gather + positional encoding are host-prepped (pure data movement);
  the device receives x0 in both seq-partition and d-partition layouts.
- Every DMA-touched tensor is laid out 2D ([P, free]) so each transfer lowers
  to a single DIRECT2D descriptor (3D APs cost one issue slot per outer index,
  ~0.6us of engine time each).
- Dummy warmup matmuls run during the initial DMA window so the PE HAM clock
  gate is at full rate when QT starts.
- Q/K weights stream as per-m-group chunks just-in-time, paced by the compute
  stream, so early HBM bandwidth stays focused on the critical path; larger
  secondary tensors are released behind gates keyed on QT/KT progress.
- b1 is folded into bq2/bv2, bv1 into the residual copy of x0, and g2/b2 into
  Wp/bp, which trims the layernorm critical path.
- Layer 2 runs per-qt-pipelined (Q2T in two 256-col halves) and the first
  vocab chunks are computed early for qt {0,1}; those chunks are re-streamed
  later for qt {2,3}.
- PSUM->SBUF moves run on Scalar; SBUF-only elementwise work on GpSimd; VEC
  keeps softmax/bn_stats and the vocab bias adds.

All matmuls run in bf16 with fp32 PSUM accumulation.
"""

import os
import sys

for _p in ("/opt/trn_rl_repo", "/root/.axon_site/_ro/trn_rl_repo"):
    if os.path.isdir(_p) and _p not in sys.path:
        sys.path.append(_p)

import numpy as np
import ml_dtypes

BF16 = ml_dtypes.bfloat16

# Problem dims (hardcoded per spec)
V, D, DI, S, B, NI = 32000, 1024, 768, 512, 8, 197
EPS = 1e-5
P = 128
ST = S // P          # 4 seq tiles
DT = D // P          # 8 model-dim tiles
DIT = DI // P        # 6 image-dim tiles
NIT = 2              # image tokens: 197 -> 2 partition tiles (128 + 69)
NI_PAD = 256
VP = 32768           # vocab padded to 64 chunks of 512
CN = 512             # vocab chunk width
NCHUNK = VP // CN    # 64
GRP = 2              # chunks per output strip
NGRP = NCHUNK // GRP
NE = 4               # chunks computed early for qt {0,1} (re-streamed later)
N_CORES = 8
SCALE = 1.0 / float(np.sqrt(np.float32(D)))

_CACHE = {}
LAST_RESULTS = None


def _build_program():
    import concourse.bacc as bacc
    import concourse.bass as bass
    import concourse.mybir as mybir
    from concourse.masks import make_identity
    from concourse.tile import TileContext

    f32 = mybir.dt.float32
    bf16 = mybir.dt.bfloat16
    X = mybir.AxisListType.X
    ALU = mybir.AluOpType
    ACT_F = mybir.ActivationFunctionType

    nc = bacc.Bacc("TRN2", target_bir_lowering=False, debug=False,
                   num_devices=N_CORES)

    # ---- I/O (all 2D so every DMA is a single DIRECT2D) ----
    h_x0b = nc.dram_tensor("x0b", [P, ST * D], bf16, kind="ExternalInput")
    h_x0T = nc.dram_tensor("x0t", [P, DT * S], bf16, kind="ExternalInput")
    h_img = nc.dram_tensor("img_t", [P, DIT * NI], bf16, kind="ExternalInput")
    h_wq1 = nc.dram_tensor("wq1", [P, DT * D], bf16, kind="ExternalInput")
    h_wk1 = nc.dram_tensor("wk1", [P, DT * D], bf16, kind="ExternalInput")
    h_wv1 = nc.dram_tensor("wv1", [P, DT * D], bf16, kind="ExternalInput")
    h_wq2 = nc.dram_tensor("wq2", [P, DT * D], bf16, kind="ExternalInput")
    h_wk2 = nc.dram_tensor("wk2", [P, DIT * D], bf16, kind="ExternalInput")
    h_wv2 = nc.dram_tensor("wv2", [P, DIT * D], bf16, kind="ExternalInput")
    h_wp = nc.dram_tensor("wp", [NCHUNK, P, DT * CN], bf16,
                          kind="ExternalInput")
    h_bq1 = nc.dram_tensor("bq1", [P, DT], f32, kind="ExternalInput")
    h_bk1 = nc.dram_tensor("bk1", [P, DT], f32, kind="ExternalInput")
    h_bq2 = nc.dram_tensor("bq2", [P, DT], f32, kind="ExternalInput")
    h_bk2 = nc.dram_tensor("bk2", [P, DT], f32, kind="ExternalInput")
    h_bv2 = nc.dram_tensor("bv2", [D], f32, kind="ExternalInput")
    h_g1 = nc.dram_tensor("g1", [D], f32, kind="ExternalInput")
    h_bp = nc.dram_tensor("bp", [VP], bf16, kind="ExternalInput")
    h_out = nc.dram_tensor("out", [S, VP], bf16, kind="ExternalOutput")

    def bcast(handle, n, offset=0):
        ap = handle[:]
        return bass.AP(tensor=ap.tensor, offset=offset, ap=[[0, P], [1, n]])

    with TileContext(nc) as tc:
        import contextlib
        ctx = contextlib.ExitStack()
        with ctx:
            const = ctx.enter_context(tc.tile_pool(name="const", bufs=1))
            xb_p = ctx.enter_context(tc.tile_pool(name="xb", bufs=3))
            xt_p = ctx.enter_context(tc.tile_pool(name="xt", bufs=2))
            qk_p = ctx.enter_context(tc.tile_pool(name="qk", bufs=2))
            v_p = ctx.enter_context(tc.tile_pool(name="vp", bufs=2))
            k2t_p = ctx.enter_context(tc.tile_pool(name="k2t", bufs=1))
            pb_p = ctx.enter_context(tc.tile_pool(name="pb", bufs=4))
            pt_p = ctx.enter_context(tc.tile_pool(name="pt", bufs=1))
            xpre_p = ctx.enter_context(tc.tile_pool(name="xpre", bufs=2))
            stat_p = ctx.enter_context(tc.tile_pool(name="stat", bufs=4))
            wts_p = ctx.enter_context(tc.tile_pool(name="wts", bufs=2))
            wv2_p = ctx.enter_context(tc.tile_pool(name="wv2p", bufs=1))
            wp_p = ctx.enter_context(tc.tile_pool(name="wpp", bufs=3))
            bp_p = ctx.enter_context(tc.tile_pool(name="bpp", bufs=2))
            osb_p = ctx.enter_context(tc.tile_pool(name="osb", bufs=6))
            ps = ctx.enter_context(tc.tile_pool(name="ps", bufs=8, space="PSUM"))

            # ---- constants / warmup ----
            ident = const.tile([P, P], bf16)
            make_identity(nc, ident)
            trimask = const.tile([P, P], f32)
            nc.gpsimd.memset(trimask, 0.0)
            nc.gpsimd.affine_select(
                out=trimask, in_=trimask, compare_op=ALU.is_ge, fill=-1e10,
                base=0, pattern=[[-1, P]], channel_multiplier=1)
            warm_src = const.tile([P, 256], bf16)
            nc.vector.memset(warm_src, 0.0)
            epst = const.tile([P, 1], f32)
            nc.vector.memset(epst, EPS)
            neg1 = const.tile([P, 1], f32)
            nc.vector.memset(neg1, -1.0)

            # HAM warmup: keep the PE busy while the first weights stream in
            for w in range(28):
                pw = ps.tile([P, 512], f32, tag="ps", name=f"warm{w}")
                nc.tensor.matmul(pw[:, :256], lhsT=ident, rhs=warm_src,
                                 start=True, stop=True)

            # ---- early DMAs ----
            x0T = xt_p.tile([P, DT * S], bf16, tag="xt", name="x0t")
            nc.sync.dma_start(out=x0T, in_=h_x0T[:])
            bq1s = const.tile([P, DT], f32)
            bk1s = const.tile([P, DT], f32)
            bq2s = const.tile([P, DT], f32)
            bk2s = const.tile([P, DT], f32)
            for t, h in ((bq1s, h_bq1), (bk1s, h_bk1), (bq2s, h_bq2),
                         (bk2s, h_bk2)):
                nc.gpsimd.dma_start(out=t, in_=h[:])

            # Weight loads: whole tensors through a 2-slot ring. The ring-slot
            # waits stagger the transfers automatically (wv1 fires only once
            # QT has consumed wq1, wq2 once KT is done), so the early HBM
            # window belongs to x0T+wq1+wk1 alone. The blocked DMAs sit on the
            # otherwise-idle sync stream.
            wq1_sb = wts_p.tile([P, DT * D], bf16, tag="wts")
            nc.scalar.dma_start(out=wq1_sb, in_=h_wq1[:])

            def proj_T(w_sb, b_sb, name):
                """out[P, DT, S] bf16 = (W.T @ x0.T) + b, d-partition."""
                o = qk_p.tile([P, DT, S], bf16, tag="qk", name=name)
                for m in range(DT):
                    pm = ps.tile([P, 512], f32, tag="ps", name="pm")
                    for k in range(DT):
                        nc.tensor.matmul(
                            pm,
                            lhsT=w_sb[:, k * D + m * P:k * D + (m + 1) * P],
                            rhs=x0T[:, k * S:(k + 1) * S],
                            start=(k == 0), stop=(k == DT - 1))
                    nc.scalar.activation(out=o[:, m, :], in_=pm,
                                         func=ACT_F.Identity,
                                         bias=b_sb[:, m:m + 1], scale=1.0)
                return o

            QT = proj_T(wq1_sb, bq1s, "qt")

            # gpsimd-gated DMAs: released once QT compute is underway so they
            # don't steal HBM bandwidth from the critical startup path; only
            # x0T+wq1 (3MB) occupy the first window
            gate_t = const.tile([P, 1], bf16)
            nc.gpsimd.tensor_copy(out=gate_t, in_=QT[:, 0, 0:1])
            wk1_sb = wts_p.tile([P, DT * D], bf16, tag="wts")
            nc.gpsimd.dma_start(out=wk1_sb, in_=h_wk1[:])
            # wv1/wq2 are ring-slot gated behind QT/KT consumption of wq1/wk1
            wv1_sb = wts_p.tile([P, DT * D], bf16, tag="wts")
            nc.sync.dma_start(out=wv1_sb, in_=h_wv1[:])
            wq2_sb = wts_p.tile([P, DT * D], bf16, tag="wts")
            nc.sync.dma_start(out=wq2_sb, in_=h_wq2[:])

            KT = proj_T(wk1_sb, bk1s, "kt")

            gate_t2 = const.tile([P, 1], bf16)
            nc.gpsimd.tensor_copy(out=gate_t2, in_=KT[:, 0, 0:1])
            img_sb = const.tile([P, DIT * NI], bf16)
            nc.gpsimd.dma_start(out=img_sb, in_=h_img[:])
            wk2_sb = wv2_p.tile([P, DIT * D], bf16, tag="wk2")
            nc.gpsimd.dma_start(out=wk2_sb, in_=h_wk2[:])
            x0b = xb_p.tile([P, ST * D], bf16, tag="xb", name="x0b")
            nc.gpsimd.dma_start(out=x0b, in_=h_x0b[:])
            wv2_sb = wv2_p.tile([P, DIT * D], bf16, tag="wv2")
            nc.gpsimd.dma_start(out=wv2_sb, in_=h_wv2[:])
            g1b = const.tile([P, D], f32)
            bv2b = const.tile([P, D], f32)
            for t, h in ((g1b, h_g1), (bv2b, h_bv2)):
                nc.gpsimd.dma_start(out=t, in_=bcast(h, D))

            # value projection (bv1 folded into x0b host-side; attention rows
            # are convex combinations so the V-bias passes through unchanged)
            Vt = v_p.tile([P, ST, D], bf16, tag="v")
            for a in range(ST):
                for nh in range(2):
                    pm = ps.tile([P, 512], f32, tag="ps")
                    for k in range(DT):
                        nc.tensor.matmul(
                            pm,
                            lhsT=x0T[:, k * S + a * P:k * S + (a + 1) * P],
                            rhs=wv1_sb[:, k * D + nh * 512:
                                       k * D + (nh + 1) * 512],
                            start=(k == 0), stop=(k == DT - 1))
                    nc.scalar.copy(out=Vt[:, a, nh * 512:(nh + 1) * 512],
                                   in_=pm)

            # ---- causal self-attention: scores + softmax (all qt) ----
            Pbs = []
            rinv1 = stat_p.tile([P, ST], f32, tag="rinv")
            for qt in range(ST):
                width = (qt + 1) * P
                pm = ps.tile([P, 512], f32, tag="ps")
                for k in range(DT):
                    nc.tensor.matmul(pm[:, :width],
                                     lhsT=QT[:, k, qt * P:(qt + 1) * P],
                                     rhs=KT[:, k, :width],
                                     start=(k == 0), stop=(k == DT - 1))
                # mask the diagonal block in place (PSUM RMW)
                nc.vector.tensor_tensor(out=pm[:, qt * P:width],
                                        in0=pm[:, qt * P:width], in1=trimask,
                                        op=ALU.add)
                nmax = stat_p.tile([P, 1], f32, tag="nmax")
                nc.vector.reduce_max(nmax, pm[:, :width], axis=X, negate=True)
                Pb = pb_p.tile([P, 512], bf16, tag="pb", name=f"pb{qt}")
                rsum = stat_p.tile([P, 1], f32, tag="rsum")
                nc.scalar.activation(out=Pb[:, :width], in_=pm[:, :width],
                                     func=ACT_F.Exp, bias=nmax, scale=1.0,
                                     accum_out=rsum)
                nc.vector.reciprocal(out=rinv1[:, qt:qt + 1], in_=rsum)
                Pbs.append(Pb)

            # ---- cross-attn K2 (fills the softmax1 pipeline shadow) ----
            K2T = k2t_p.tile([P, DT, NI_PAD], bf16, tag="k2t")
            for m in range(DT):
                pm = ps.tile([P, 512], f32, tag="ps")
                for k in range(DIT):
                    nc.tensor.matmul(
                        pm[:, :NI],
                        lhsT=wk2_sb[:, k * D + m * P:k * D + (m + 1) * P],
                        rhs=img_sb[:, k * NI:(k + 1) * NI],
                        start=(k == 0), stop=(k == DIT - 1))
                nc.scalar.activation(out=K2T[:, m, :NI], in_=pm[:, :NI],
                                     func=ACT_F.Identity,
                                     bias=bk2s[:, m:m + 1], scale=1.0)

            def layernorm(xpre, out_sl, gb):
                """xpre [P, D] f32 -> out_sl [P, D] bf16.

                Writes the normalized rows times gb (or raw normalized rows if
                gb is None — affine folded into the consumers)."""
                stats = stat_p.tile([P, 2, 6], f32, tag="bnst")
                for sg in range(2):
                    nc.vector.bn_stats(out=stats[:, sg, :],
                                       in_=xpre[:, sg * 512:(sg + 1) * 512])
                mv = stat_p.tile([P, 2], f32, tag="bnmv")
                nc.vector.bn_aggr(out=mv, in_=stats)
                rstd = stat_p.tile([P, 1], f32, tag="rstd")
                nc.scalar.activation(out=rstd, in_=mv[:, 1:2], func=ACT_F.Sqrt,
                                     bias=epst, scale=1.0)
                nc.vector.reciprocal(out=rstd, in_=rstd)
                nmr = stat_p.tile([P, 1], f32, tag="nmr")
                nc.vector.scalar_tensor_tensor(
                    out=nmr, in0=mv[:, 0:1], scalar=rstd, in1=neg1,
                    op0=ALU.mult, op1=ALU.mult)
                if gb is None:
                    nc.scalar.activation(out=out_sl, in_=xpre,
                                         func=ACT_F.Identity,
                                         bias=nmr, scale=rstd)
                else:
                    nc.scalar.activation(out=xpre, in_=xpre,
                                         func=ACT_F.Identity,
                                         bias=nmr, scale=rstd)
                    nc.vector.tensor_tensor(out=out_sl[:, :512],
                                            in0=xpre[:, :512],
                                            in1=gb[:, :512], op=ALU.mult)
                    nc.gpsimd.tensor_tensor(out=out_sl[:, 512:],
                                            in0=xpre[:, 512:],
                                            in1=gb[:, 512:], op=ALU.mult)

            # ---- AV1 + residual + LN1 per qt ----
            PT = pt_p.tile([P, ST, S], bf16, tag="pt")
            x1b = xb_p.tile([P, ST, D], bf16, tag="xb", name="x1b")
            wp_pre = {}
            bp_pre = {}
            # cross-attn V2 blocks are interleaved below as PE filler while
            # the VEC-bound AV1/LN1 chain drains
            V2t = v_p.tile([P, NIT, D], bf16, tag="v")
            nc.gpsimd.memset(V2t, 0.0)

            def v2_block(a, nh):
                pa = P if a == 0 else NI - P
                pm = ps.tile([P, 512], f32, tag="ps")
                for k in range(DIT):
                    nc.tensor.matmul(
                        pm[:pa, :],
                        lhsT=img_sb[:, k * NI + a * P:k * NI + a * P + pa],
                        rhs=wv2_sb[:, k * D + nh * 512:k * D + (nh + 1) * 512],
                        start=(k == 0), stop=(k == DIT - 1))
                nc.vector.tensor_tensor(
                    out=V2t[:pa, a, nh * 512:(nh + 1) * 512], in0=pm[:pa, :],
                    in1=bv2b[:pa, nh * 512:(nh + 1) * 512], op=ALU.add)

            for qt in range(ST):
                for kt in range(qt + 1):
                    tp = ps.tile([P, 512], bf16, tag="ps", name="tp")
                    nc.tensor.transpose(out=tp[:, :P],
                                        in_=Pbs[qt][:, kt * P:(kt + 1) * P],
                                        identity=ident)
                    nc.vector.tensor_copy(out=PT[:, kt, qt * P:(qt + 1) * P],
                                          in_=tp[:, :P])
                xpre = xpre_p.tile([P, D], f32, tag="xpre")
                for nh in range(2):
                    sl = slice(nh * 512, (nh + 1) * 512)
                    pm = ps.tile([P, 512], f32, tag="ps")
                    for kt in range(qt + 1):
                        nc.tensor.matmul(pm, lhsT=PT[:, kt, qt * P:(qt + 1) * P],
                                         rhs=Vt[:, kt, nh * 512:(nh + 1) * 512],
                                         start=(kt == 0), stop=(kt == qt))
                    # residual split: SCA rescale, then VEC/GPS halves add
                    nc.scalar.activation(out=xpre[:, sl], in_=pm,
                                         func=ACT_F.Identity,
                                         scale=rinv1[:, qt:qt + 1])
                    eng = nc.vector if nh == 0 else nc.gpsimd
                    eng.tensor_tensor(
                        out=xpre[:, sl], in0=xpre[:, sl],
                        in1=x0b[:, qt * D + nh * 512:qt * D + (nh + 1) * 512],
                        op=ALU.add)
                v2_block(qt % NIT, qt // NIT)  # PE filler during stt/LN1
                layernorm(xpre, x1b[:, qt, :], g1b)
                if qt == 0:
                    # prefetch the first vocab chunks + bias strip now that
                    # the startup DMA window has drained (gpsimd stream pos)
                    for c in range(3):
                        t = wp_p.tile([P, DT * CN], bf16, tag="wp",
                                      name=f"wp_pre{c}")
                        nc.gpsimd.dma_start(out=t, in_=h_wp[c])
                        wp_pre[c] = t
                    bp0 = bp_p.tile([P, GRP * CN], bf16, tag="bp",
                                    name="bp_pre0")
                    nc.gpsimd.dma_start(out=bp0, in_=bcast(h_bp, GRP * CN))
                    bp_pre[0] = bp0

            # ---- layer 2, pipelined in qt-pair halves ----
            def transpose_cols(dst, src_b, a_list, tag):
                """transpose x[P, a, db*P:(db+1)*P] -> dst[:, db, a*P:(a+1)*P]."""
                for a in a_list:
                    for db in range(DT):
                        tp = ps.tile([P, 512], bf16, tag="ps", name=tag)
                        nc.tensor.transpose(
                            out=tp[:, :P],
                            in_=src_b[:, a, db * P:(db + 1) * P],
                            identity=ident)
                        nc.scalar.copy(
                            out=dst[:, db, a * P:(a + 1) * P], in_=tp[:, :P])

            def proj_T_into(o, w_sb, b_sb, rhsT, c0, c1, kt=DT):
                w = c1 - c0
                for m in range(DT):
                    pm = ps.tile([P, 512], f32, tag="ps", name="pm")
                    for k in range(kt):
                        nc.tensor.matmul(
                            pm[:, :w],
                            lhsT=w_sb[:, k * D + m * P:k * D + (m + 1) * P],
                            rhs=rhsT[:, k, c0:c1],
                            start=(k == 0), stop=(k == kt - 1))
                    nc.scalar.activation(out=o[:, m, c0:c1], in_=pm[:, :w],
                                         func=ACT_F.Identity,
                                         bias=b_sb[:, m:m + 1], scale=1.0)

            x1T = xt_p.tile([P, DT, S], bf16, tag="xt", name="x1t")
            Q2T = qk_p.tile([P, DT, S], bf16, tag="qk", name="q2t")
            P2bs = []
            rinv2 = stat_p.tile([P, ST], f32, tag="rinv2")

            def scores2_softmax(qt):
                pm = ps.tile([P, 512], f32, tag="ps")
                for k in range(DT):
                    nc.tensor.matmul(pm[:, :NI],
                                     lhsT=Q2T[:, k, qt * P:(qt + 1) * P],
                                     rhs=K2T[:, k, :NI],
                                     start=(k == 0), stop=(k == DT - 1))
                nmax = stat_p.tile([P, 1], f32, tag="nmax")
                nc.vector.reduce_max(nmax, pm[:, :NI], axis=X, negate=True)
                P2b = pb_p.tile([P, NI_PAD], bf16, tag="pb2", name=f"p2b{qt}")
                nc.gpsimd.memset(P2b[:, NI:], 0.0)
                rsum = stat_p.tile([P, 1], f32, tag="rsum")
                nc.scalar.activation(out=P2b[:, :NI], in_=pm[:, :NI],
                                     func=ACT_F.Exp, bias=nmax, scale=1.0,
                                     accum_out=rsum)
                nc.vector.reciprocal(out=rinv2[:, qt:qt + 1], in_=rsum)
                P2bs.append(P2b)

            # first half: qt 0,1
            transpose_cols(x1T, x1b, (0, 1), "x1t_tp")
            proj_T_into(Q2T, wq2_sb, bq2s, x1T, 0, 256)
            scores2_softmax(0)
            scores2_softmax(1)
            # second half: qt 2,3
            transpose_cols(x1T, x1b, (2, 3), "x1t_tp")
            proj_T_into(Q2T, wq2_sb, bq2s, x1T, 256, 512)
            scores2_softmax(2)
            scores2_softmax(3)

            # ---- AV2 + residual + LN2 per qt (normalized out; affine folded
            # into Wp/bp) ----
            PT2 = pt_p.tile([P, NIT, S], bf16, tag="pt2")
            x2b = xb_p.tile([P, ST, D], bf16, tag="xb", name="x2b")

            def av2_ln2(qt):
                for kt in range(NIT):
                    tp = ps.tile([P, 512], bf16, tag="ps", name="tp2")
                    nc.tensor.transpose(out=tp[:, :P],
                                        in_=P2bs[qt][:, kt * P:(kt + 1) * P],
                                        identity=ident)
                    nc.vector.tensor_copy(out=PT2[:, kt, qt * P:(qt + 1) * P],
                                          in_=tp[:, :P])
                xpre = xpre_p.tile([P, D], f32, tag="xpre")
                for nh in range(2):
                    sl = slice(nh * 512, (nh + 1) * 512)
                    pm = ps.tile([P, 512], f32, tag="ps")
                    for kt in range(NIT):
                        nc.tensor.matmul(pm, lhsT=PT2[:, kt, qt * P:(qt + 1) * P],
                                         rhs=V2t[:, kt, nh * 512:(nh + 1) * 512],
                                         start=(kt == 0), stop=(kt == NIT - 1))
                    nc.scalar.activation(out=xpre[:, sl], in_=pm,
                                         func=ACT_F.Identity,
                                         scale=rinv2[:, qt:qt + 1])
                    eng = nc.vector if nh == 0 else nc.gpsimd
                    eng.tensor_tensor(out=xpre[:, sl], in0=xpre[:, sl],
                                      in1=x1b[:, qt, sl], op=ALU.add)
                layernorm(xpre, x2b[:, qt, :], None)

            x2T = xt_p.tile([P, DT, S], bf16, tag="xt", name="x2t")

            for qt in range(ST):
                av2_ln2(qt)
            transpose_cols(x2T, x2b, (0, 1), "x2t_tp")

            # ---- vocab projection, streamed in CN-column chunks ----
            def vocab_chunks(chunks, qts, dma_par, pre=()):
                """Process wp[chunks] x qts; chunks must align to GRP groups."""
                for gi in range(0, len(chunks), GRP):
                    g = chunks[gi] // GRP
                    if g in bp_pre:
                        bp_bc = bp_pre.pop(g)
                    else:
                        bp_bc = bp_p.tile([P, GRP * CN], bf16, tag="bp")
                        nc.gpsimd.dma_start(out=bp_bc,
                                            in_=bcast(h_bp, GRP * CN,
                                                      offset=g * GRP * CN))
                    osb = {q: osb_p.tile([P, GRP * CN], bf16, tag="osb",
                                         name=f"osb_{g}_{q}")
                           for q in qts}
                    for cc in range(GRP):
                        c = chunks[gi + cc]
                        if c in pre and c in wp_pre:
                            wp_sb = wp_pre.pop(c)
                        else:
                            wp_sb = wp_p.tile([P, DT * CN], bf16, tag="wp")
                            dma_eng = (nc.sync if (c + dma_par) % 2 == 0
                                       else nc.scalar)
                            dma_eng.dma_start(out=wp_sb, in_=h_wp[c])
                        for qt in qts:
                            pm = ps.tile([P, 512], f32, tag="ps")
                            for k in range(DT):
                                nc.tensor.matmul(
                                    pm, lhsT=x2T[:, k, qt * P:(qt + 1) * P],
                                    rhs=wp_sb[:, k * CN:(k + 1) * CN],
                                    start=(k == 0), stop=(k == DT - 1))
                            nc.vector.tensor_tensor(
                                out=osb[qt][:, cc * CN:(cc + 1) * CN], in0=pm,
                                in1=bp_bc[:, cc * CN:(cc + 1) * CN], op=ALU.add)
                            if cc == GRP - 1:
                                # fire each strip as soon as it completes
                                out_eng = nc.sync if qt < 2 else nc.scalar
                                out_eng.dma_start(
                                    out=h_out[qt * P:(qt + 1) * P,
                                              g * GRP * CN:(g + 1) * GRP * CN],
                                    in_=osb[qt])

            # early pass: first NE chunks for qt {0,1} while LN2(2,3) drains
            vocab_chunks(list(range(NE)), (0, 1), 0, pre=(0, 1, 2))
            transpose_cols(x2T, x2b, (2, 3), "x2t_tp")
            # late pass for those chunks' qt {2,3} (re-streamed), then the rest
            vocab_chunks(list(range(NE)), (2, 3), 1)
            vocab_chunks(list(range(NE, NCHUNK)), (0, 1, 2, 3), 0)

    nc.compile()
    return nc


def _tile_sq(w, kt):
    """[K, N] -> [128, K//128, N] contiguous."""
    k, n = w.shape
    assert k == kt * P
    return np.ascontiguousarray(
        w.reshape(kt, P, n).transpose(1, 0, 2)).astype(BF16)


def _pos_enc():
    posn = np.arange(S)[:, None].astype(np.float32)
    i = np.arange(0, D, 2).astype(np.float32)
    ang = posn / np.power(10000.0, i / D)
    pos = np.zeros((S, D), dtype=np.float32)
    pos[:, 0::2] = np.sin(ang)
    pos[:, 1::2] = np.cos(ang)
    return pos


def _prep_inputs(inputs):
    g = lambda name: np.asarray(inputs[name], dtype=np.float32)
    tokens = np.asarray(inputs["tokens"]).astype(np.int64)
    img = g("img_emb")
    table = g("emb_table")
    pos = _pos_enc()

    b1 = g("b1")
    g2 = g("g2")
    b2 = g("b2")
    wp = g("Wp") * g2[:, None]          # fold LN2 gamma
    wp_pad = np.zeros((D, VP), dtype=np.float32)
    wp_pad[:, :V] = wp
    wp_t = np.ascontiguousarray(
        wp_pad.reshape(DT, P, NCHUNK, CN).transpose(2, 1, 0, 3)).astype(BF16)
    bp_pad = np.zeros((VP,), dtype=np.float32)
    bp_pad[:V] = g("bp") + b2 @ g("Wp")  # fold LN2 beta
    bp_pad = bp_pad.astype(BF16)

    def bias_tiled(b):
        return np.ascontiguousarray(b.reshape(DT, P).T).astype(np.float32)

    shared = {
        "wq1": _tile_sq(g("Wq1") * SCALE, DT).reshape(P, -1),
        "wk1": _tile_sq(g("Wk1"), DT).reshape(P, -1),
        "wv1": _tile_sq(g("Wv1"), DT).reshape(P, -1),
        "wq2": _tile_sq(g("Wq2") * SCALE, DT).reshape(P, -1),
        "wk2": _tile_sq(g("Wk2"), DIT).reshape(P, -1),
        "wv2": _tile_sq(g("Wv2"), DIT).reshape(P, -1),
        "wp": wp_t.reshape(NCHUNK, P, -1),
        "bq1": bias_tiled(g("bq1") * SCALE),
        "bk1": bias_tiled(g("bk1")),
        # fold LN1 beta into the cross-attn query bias and value bias
        "bq2": bias_tiled((g("bq2") + b1 @ g("Wq2")) * SCALE),
        "bk2": bias_tiled(g("bk2")),
        "bv2": g("bv2") + b1,
        "g1": g("g1"),
        "bp": bp_pad,
    }
    in_maps = []
    for c in range(N_CORES):
        m = dict(shared)
        x0 = table[tokens[c]] + pos                      # [S, D] f32
        # bv1 folded into the self-attn residual (A1 rows sum to 1)
        x0r = x0 + g("bv1")
        x0b = np.ascontiguousarray(
            x0r.reshape(ST, P, D).transpose(1, 0, 2)).astype(BF16)
        x0T = np.ascontiguousarray(
            x0.T.reshape(DT, P, S).transpose(1, 0, 2)).astype(BF16)
        m["x0b"] = x0b.reshape(P, -1)
        m["x0t"] = x0T.reshape(P, -1)
        m["img_t"] = np.ascontiguousarray(
            img[c].T.reshape(DIT, P, NI).transpose(1, 0, 2)).astype(
                BF16).reshape(P, -1)
        in_maps.append(m)
    return in_maps


def _ensure_axon_hooks():
    """bass_utils imports antenv.axon_hooks when BASS_TRACE is set; stub it
    if the module is absent so tracing degrades instead of crashing."""
    try:
        import antenv.axon_hooks  # noqa: F401
    except ImportError:
        import types
        mod = types.ModuleType("antenv.axon_hooks")
        mod.get_axon_ntff_profile_hook = lambda: None
        mod.set_axon_ntff_profile_hook = lambda h: None
        sys.modules["antenv.axon_hooks"] = mod


def kernel(**inputs):
    global LAST_RESULTS
    _ensure_axon_hooks()
    from concourse.bass_utils import run_bass_kernel_spmd

    if "nc" not in _CACHE:
        _CACHE["nc"] = _build_program()
    nc = _CACHE["nc"]

    in_maps = _prep_inputs(inputs)
    res = run_bass_kernel_spmd(nc, in_maps, core_ids=list(range(N_CORES)))
    LAST_RESULTS = res
    out = np.stack([res.results[c]["out"][:, :V].astype(np.float32)
                    for c in range(N_CORES)])
    return out


# revision 64
# speedup vs baseline: 1.0703x; 1.0112x over previous
"""Trainium2 Bass kernel for an 8-batch image-conditioned decoder layer.

Strategy: pure data-parallel over the batch — core c computes batch element c
end-to-end (causal self-attention, cross-attention over the image tokens, both
layernorms, vocab projection). No collectives.

Schedule notes:
- Embedding gather + positional encoding are host-prepped (pure data movement);
  the device receives x0 in both seq-partition and d-partition layouts.
- Every DMA-touched tensor is laid out 2D ([P, free]) so each transfer lowers
  to a single DIRECT2D descriptor (3D APs cost one issue slot per outer index,
  ~0.6us of engine time each).
- Dummy warmup matmuls run during the initial DMA window so the PE HAM clock
  gate is at full rate when QT starts.
- Q/K weights stream as per-m-group chunks just-in-time, paced by the compute
  stream, so early HBM bandwidth stays focused on the critical path; larger
  secondary tensors are released behind gates keyed on QT/KT progress.
- b1 is folded into bq2/bv2, bv1 into the residual copy of x0, and g2/b2 into
  Wp/bp, which trims the layernorm critical path.
- Layer 2 runs per-qt-pipelined (Q2T in two 256-col halves) and the first
  vocab chunks are computed early for qt {0,1}; those chunks are re-streamed
  later for qt {2,3}.
- PSUM->SBUF moves run on Scalar; SBUF-only elementwise work on GpSimd; VEC
  keeps softmax/bn_stats and the vocab bias adds.

All matmuls run in bf16 with fp32 PSUM accumulation.
"""

import os
import sys

for _p in ("/opt/trn_rl_repo", "/root/.axon_site/_ro/trn_rl_repo"):
    if os.path.isdir(_p) and _p not in sys.path:
        sys.path.append(_p)

import numpy as np
import ml_dtypes

BF16 = ml_dtypes.bfloat16

# Problem dims (hardcoded per spec)
V, D, DI, S, B, NI = 32000, 1024, 768, 512, 8, 197
EPS = 1e-5
P = 128
ST = S // P          # 4 seq tiles
DT = D // P          # 8 model-dim tiles
DIT = DI // P        # 6 image-dim tiles
NIT = 2              # image tokens: 197 -> 2 partition tiles (128 + 69)
NI_PAD = 256
VP = 32768           # vocab padded to 64 chunks of 512
CN = 512             # vocab chunk width
NCHUNK = VP // CN    # 64
GRP = 2              # chunks per output strip
NGRP = NCHUNK // GRP
NE = 4               # chunks computed early for qt {0,1} (re-streamed later)
N_CORES = 8
SCALE = 1.0 / float(np.sqrt(np.float32(D)))

_CACHE = {}
LAST_RESULTS = None


def _build_program():
    import concourse.bacc as bacc
    import concourse.bass as bass
    import concourse.mybir as mybir
    from concourse.masks import make_identity
    from concourse.tile import TileContext

    f32 = mybir.dt.float32
    bf16 = mybir.dt.bfloat16
    X = mybir.AxisListType.X
    ALU = mybir.AluOpType
    ACT_F = mybir.ActivationFunctionType

    nc = bacc.Bacc("TRN2", target_bir_lowering=False, debug=False,
                   num_devices=N_CORES)

    # ---- I/O (all 2D so every DMA is a single DIRECT2D) ----
    h_x0b = nc.dram_tensor("x0b", [P, ST * D], bf16, kind="ExternalInput")
    h_x0T = nc.dram_tensor("x0t", [P, DT * S], bf16, kind="ExternalInput")
    h_img = nc.dram_tensor("img_t", [P, DIT * NI], bf16, kind="ExternalInput")
    h_wq1 = nc.dram_tensor("wq1", [P, DT * D], bf16, kind="ExternalInput")
    h_wk1 = nc.dram_tensor("wk1", [P, DT * D], bf16, kind="ExternalInput")
    h_wv1 = nc.dram_tensor("wv1", [P, DT * D], bf16, kind="ExternalInput")
    h_wq2 = nc.dram_tensor("wq2", [P, DT * D], bf16, kind="ExternalInput")
    h_wk2 = nc.dram_tensor("wk2", [P, DIT * D], bf16, kind="ExternalInput")
    h_wv2 = nc.dram_tensor("wv2", [P, DIT * D], bf16, kind="ExternalInput")
    h_wp = nc.dram_tensor("wp", [NCHUNK, P, DT * CN], bf16,
                          kind="ExternalInput")
    h_bq1 = nc.dram_tensor("bq1", [P, DT], f32, kind="ExternalInput")
    h_bk1 = nc.dram_tensor("bk1", [P, DT], f32, kind="ExternalInput")
    h_bq2 = nc.dram_tensor("bq2", [P, DT], f32, kind="ExternalInput")
    h_bk2 = nc.dram_tensor("bk2", [P, DT], f32, kind="ExternalInput")
    h_bv2 = nc.dram_tensor("bv2", [D], f32, kind="ExternalInput")
    h_g1 = nc.dram_tensor("g1", [D], f32, kind="ExternalInput")
    h_bp = nc.dram_tensor("bp", [VP], bf16, kind="ExternalInput")
    h_out = nc.dram_tensor("out", [S, VP], bf16, kind="ExternalOutput")

    def bcast(handle, n, offset=0):
        ap = handle[:]
        return bass.AP(tensor=ap.tensor, offset=offset, ap=[[0, P], [1, n]])

    with TileContext(nc) as tc:
        import contextlib
        ctx = contextlib.ExitStack()
        with ctx:
            const = ctx.enter_context(tc.tile_pool(name="const", bufs=1))
            xb_p = ctx.enter_context(tc.tile_pool(name="xb", bufs=3))
            xt_p = ctx.enter_context(tc.tile_pool(name="xt", bufs=2))
            qk_p = ctx.enter_context(tc.tile_pool(name="qk", bufs=2))
            v_p = ctx.enter_context(tc.tile_pool(name="vp", bufs=2))
            k2t_p = ctx.enter_context(tc.tile_pool(name="k2t", bufs=1))
            pb_p = ctx.enter_context(tc.tile_pool(name="pb", bufs=4))
            pt_p = ctx.enter_context(tc.tile_pool(name="pt", bufs=1))
            xpre_p = ctx.enter_context(tc.tile_pool(name="xpre", bufs=2))
            stat_p = ctx.enter_context(tc.tile_pool(name="stat", bufs=4))
            wts_p = ctx.enter_context(tc.tile_pool(name="wts", bufs=2))
            wv2_p = ctx.enter_context(tc.tile_pool(name="wv2p", bufs=1))
            wp_p = ctx.enter_context(tc.tile_pool(name="wpp", bufs=3))
            bp_p = ctx.enter_context(tc.tile_pool(name="bpp", bufs=2))
            osb_p = ctx.enter_context(tc.tile_pool(name="osb", bufs=6))
            ps = ctx.enter_context(tc.tile_pool(name="ps", bufs=8, space="PSUM"))

            # ---- constants / warmup ----
            ident = const.tile([P, P], bf16)
            make_identity(nc, ident)
            trimask = const.tile([P, P], f32)
            nc.gpsimd.memset(trimask, 0.0)
            nc.gpsimd.affine_select(
                out=trimask, in_=trimask, compare_op=ALU.is_ge, fill=-1e10,
                base=0, pattern=[[-1, P]], channel_multiplier=1)
            warm_src = const.tile([P, 256], bf16)
            nc.vector.memset(warm_src, 0.0)
            epst = const.tile([P, 1], f32)
            nc.vector.memset(epst, EPS)
            neg1 = const.tile([P, 1], f32)
            nc.vector.memset(neg1, -1.0)

            # HAM warmup: keep the PE busy while the first weights stream in
            for w in range(28):
                pw = ps.tile([P, 512], f32, tag="ps", name=f"warm{w}")
                nc.tensor.matmul(pw[:, :256], lhsT=ident, rhs=warm_src,
                                 start=True, stop=True)

            # ---- early DMAs ----
            x0T = xt_p.tile([P, DT * S], bf16, tag="xt", name="x0t")
            nc.sync.dma_start(out=x0T, in_=h_x0T[:])
            bq1s = const.tile([P, DT], f32)
            bk1s = const.tile([P, DT], f32)
            bq2s = const.tile([P, DT], f32)
            bk2s = const.tile([P, DT], f32)
            for t, h in ((bq1s, h_bq1), (bk1s, h_bk1), (bq2s, h_bq2),
                         (bk2s, h_bk2)):
                nc.gpsimd.dma_start(out=t, in_=h[:])

            # K2T's inputs (img 0.3MB + wk2 1.5MB) are the smallest load that
            # unlocks real PE work, so they go first alongside x0T+wq1; the
            # rest is gated so the early HBM window stays uncongested.
            img_sb = const.tile([P, DIT * NI], bf16)
            nc.gpsimd.dma_start(out=img_sb, in_=h_img[:])
            wk2_sb = wv2_p.tile([P, DIT * D], bf16, tag="wk2")
            nc.gpsimd.dma_start(out=wk2_sb, in_=h_wk2[:])
            wq1_sb = wts_p.tile([P, DT * D], bf16, tag="wts")
            nc.scalar.dma_start(out=wq1_sb, in_=h_wq1[:])

            # ---- cross-attn K2 (first real PE work; minimal DMA deps) ----
            K2T = k2t_p.tile([P, DT, NI_PAD], bf16, tag="k2t")
            for m in range(DT):
                pm = ps.tile([P, 512], f32, tag="ps")
                for k in range(DIT):
                    nc.tensor.matmul(
                        pm[:, :NI],
                        lhsT=wk2_sb[:, k * D + m * P:k * D + (m + 1) * P],
                        rhs=img_sb[:, k * NI:(k + 1) * NI],
                        start=(k == 0), stop=(k == DIT - 1))
                nc.scalar.activation(out=K2T[:, m, :NI], in_=pm[:, :NI],
                                     func=ACT_F.Identity,
                                     bias=bk2s[:, m:m + 1], scale=1.0)

            def proj_T(w_sb, b_sb, name):
                """out[P, DT, S] bf16 = (W.T @ x0.T) + b, d-partition."""
                o = qk_p.tile([P, DT, S], bf16, tag="qk", name=name)
                for m in range(DT):
                    pm = ps.tile([P, 512], f32, tag="ps", name="pm")
                    for k in range(DT):
                        nc.tensor.matmul(
                            pm,
                            lhsT=w_sb[:, k * D + m * P:k * D + (m + 1) * P],
                            rhs=x0T[:, k * S:(k + 1) * S],
                            start=(k == 0), stop=(k == DT - 1))
                    nc.scalar.activation(out=o[:, m, :], in_=pm,
                                         func=ACT_F.Identity,
                                         bias=b_sb[:, m:m + 1], scale=1.0)
                return o

            QT = proj_T(wq1_sb, bq1s, "qt")

            # gpsimd-gated DMAs: released once QT compute is underway so they
            # don't steal HBM bandwidth from the critical startup path
            gate_t = const.tile([P, 1], bf16)
            nc.gpsimd.tensor_copy(out=gate_t, in_=QT[:, 0, 0:1])
            wk1_sb = wts_p.tile([P, DT * D], bf16, tag="wts")
            nc.gpsimd.dma_start(out=wk1_sb, in_=h_wk1[:])
            # wv1/wq2 are ring-slot gated behind QT/KT consumption of wq1/wk1
            wv1_sb = wts_p.tile([P, DT * D], bf16, tag="wts")
            nc.sync.dma_start(out=wv1_sb, in_=h_wv1[:])
            wq2_sb = wts_p.tile([P, DT * D], bf16, tag="wts")
            nc.sync.dma_start(out=wq2_sb, in_=h_wq2[:])

            KT = proj_T(wk1_sb, bk1s, "kt")

            gate_t2 = const.tile([P, 1], bf16)
            nc.gpsimd.tensor_copy(out=gate_t2, in_=KT[:, 0, 0:1])
            x0b = xb_p.tile([P, ST * D], bf16, tag="xb", name="x0b")
            nc.gpsimd.dma_start(out=x0b, in_=h_x0b[:])
            wv2_sb = wv2_p.tile([P, DIT * D], bf16, tag="wv2")
            nc.gpsimd.dma_start(out=wv2_sb, in_=h_wv2[:])
            g1b = const.tile([P, D], f32)
            bv2b = const.tile([P, D], f32)
            for t, h in ((g1b, h_g1), (bv2b, h_bv2)):
                nc.gpsimd.dma_start(out=t, in_=bcast(h, D))

            # value projection (bv1 folded into x0b host-side; attention rows
            # are convex combinations so the V-bias passes through unchanged)
            Vt = v_p.tile([P, ST, D], bf16, tag="v")

            def vt_block(a):
                for nh in range(2):
                    pm = ps.tile([P, 512], f32, tag="ps")
                    for k in range(DT):
                        nc.tensor.matmul(
                            pm,
                            lhsT=x0T[:, k * S + a * P:k * S + (a + 1) * P],
                            rhs=wv1_sb[:, k * D + nh * 512:
                                       k * D + (nh + 1) * 512],
                            start=(k == 0), stop=(k == DT - 1))
                    nc.scalar.copy(out=Vt[:, a, nh * 512:(nh + 1) * 512],
                                   in_=pm)

            # Vt a=0,1 now; a=2,3 deferred into the softmax1/AV1 shadow
            vt_block(0)
            vt_block(1)

            # ---- causal self-attention: scores + softmax (all qt) ----
            Pbs = []
            rinv1 = stat_p.tile([P, ST], f32, tag="rinv")
            for qt in range(ST):
                width = (qt + 1) * P
                pm = ps.tile([P, 512], f32, tag="ps")
                for k in range(DT):
                    nc.tensor.matmul(pm[:, :width],
                                     lhsT=QT[:, k, qt * P:(qt + 1) * P],
                                     rhs=KT[:, k, :width],
                                     start=(k == 0), stop=(k == DT - 1))
                # mask the diagonal block in place (PSUM RMW)
                nc.vector.tensor_tensor(out=pm[:, qt * P:width],
                                        in0=pm[:, qt * P:width], in1=trimask,
                                        op=ALU.add)
                nmax = stat_p.tile([P, 1], f32, tag="nmax")
                nc.vector.reduce_max(nmax, pm[:, :width], axis=X, negate=True)
                Pb = pb_p.tile([P, 512], bf16, tag="pb", name=f"pb{qt}")
                rsum = stat_p.tile([P, 1], f32, tag="rsum")
                nc.scalar.activation(out=Pb[:, :width], in_=pm[:, :width],
                                     func=ACT_F.Exp, bias=nmax, scale=1.0,
                                     accum_out=rsum)
                nc.vector.reciprocal(out=rinv1[:, qt:qt + 1], in_=rsum)
                Pbs.append(Pb)

            vt_block(2)  # fills the softmax1 shadow

            def layernorm(xpre, out_sl, gb):
                """xpre [P, D] f32 -> out_sl [P, D] bf16.

                Writes the normalized rows times gb (or raw normalized rows if
                gb is None — affine folded into the consumers)."""
                stats = stat_p.tile([P, 2, 6], f32, tag="bnst")
                for sg in range(2):
                    nc.vector.bn_stats(out=stats[:, sg, :],
                                       in_=xpre[:, sg * 512:(sg + 1) * 512])
                mv = stat_p.tile([P, 2], f32, tag="bnmv")
                nc.vector.bn_aggr(out=mv, in_=stats)
                rstd = stat_p.tile([P, 1], f32, tag="rstd")
                nc.scalar.activation(out=rstd, in_=mv[:, 1:2], func=ACT_F.Sqrt,
                                     bias=epst, scale=1.0)
                nc.vector.reciprocal(out=rstd, in_=rstd)
                nmr = stat_p.tile([P, 1], f32, tag="nmr")
                nc.vector.scalar_tensor_tensor(
                    out=nmr, in0=mv[:, 0:1], scalar=rstd, in1=neg1,
                    op0=ALU.mult, op1=ALU.mult)
                if gb is None:
                    nc.scalar.activation(out=out_sl, in_=xpre,
                                         func=ACT_F.Identity,
                                         bias=nmr, scale=rstd)
                else:
                    nc.scalar.activation(out=xpre, in_=xpre,
                                         func=ACT_F.Identity,
                                         bias=nmr, scale=rstd)
                    nc.vector.tensor_tensor(out=out_sl[:, :512],
                                            in0=xpre[:, :512],
                                            in1=gb[:, :512], op=ALU.mult)
                    nc.gpsimd.tensor_tensor(out=out_sl[:, 512:],
                                            in0=xpre[:, 512:],
                                            in1=gb[:, 512:], op=ALU.mult)

            # ---- AV1 + residual + LN1 per qt ----
            PT = pt_p.tile([P, ST, S], bf16, tag="pt")
            x1b = xb_p.tile([P, ST, D], bf16, tag="xb", name="x1b")
            wp_pre = {}
            bp_pre = {}
            # cross-attn V2 blocks are interleaved below as PE filler while
            # the VEC-bound AV1/LN1 chain drains
            V2t = v_p.tile([P, NIT, D], bf16, tag="v")
            nc.gpsimd.memset(V2t, 0.0)

            def v2_block(a, nh):
                pa = P if a == 0 else NI - P
                pm = ps.tile([P, 512], f32, tag="ps")
                for k in range(DIT):
                    nc.tensor.matmul(
                        pm[:pa, :],
                        lhsT=img_sb[:, k * NI + a * P:k * NI + a * P + pa],
                        rhs=wv2_sb[:, k * D + nh * 512:k * D + (nh + 1) * 512],
                        start=(k == 0), stop=(k == DIT - 1))
                nc.vector.tensor_tensor(
                    out=V2t[:pa, a, nh * 512:(nh + 1) * 512], in0=pm[:pa, :],
                    in1=bv2b[:pa, nh * 512:(nh + 1) * 512], op=ALU.add)

            for qt in range(ST):
                for kt in range(qt + 1):
                    tp = ps.tile([P, 512], bf16, tag="ps", name="tp")
                    nc.tensor.transpose(out=tp[:, :P],
                                        in_=Pbs[qt][:, kt * P:(kt + 1) * P],
                                        identity=ident)
                    nc.vector.tensor_copy(out=PT[:, kt, qt * P:(qt + 1) * P],
                                          in_=tp[:, :P])
                xpre = xpre_p.tile([P, D], f32, tag="xpre")
                for nh in range(2):
                    sl = slice(nh * 512, (nh + 1) * 512)
                    pm = ps.tile([P, 512], f32, tag="ps")
                    for kt in range(qt + 1):
                        nc.tensor.matmul(pm, lhsT=PT[:, kt, qt * P:(qt + 1) * P],
                                         rhs=Vt[:, kt, nh * 512:(nh + 1) * 512],
                                         start=(kt == 0), stop=(kt == qt))
                    # residual split: SCA rescale, then VEC/GPS halves add
                    nc.scalar.activation(out=xpre[:, sl], in_=pm,
                                         func=ACT_F.Identity,
                                         scale=rinv1[:, qt:qt + 1])
                    eng = nc.vector if nh == 0 else nc.gpsimd
                    eng.tensor_tensor(
                        out=xpre[:, sl], in0=xpre[:, sl],
                        in1=x0b[:, qt * D + nh * 512:qt * D + (nh + 1) * 512],
                        op=ALU.add)
                if qt == 0:
                    vt_block(3)  # PE filler during stt/LN1
                v2_block(qt % NIT, qt // NIT)  # PE filler during stt/LN1
                layernorm(xpre, x1b[:, qt, :], g1b)
                if qt == 0:
                    # prefetch the first vocab chunks + bias strip now that
                    # the startup DMA window has drained (gpsimd stream pos)
                    for c in range(3):
                        t = wp_p.tile([P, DT * CN], bf16, tag="wp",
                                      name=f"wp_pre{c}")
                        nc.gpsimd.dma_start(out=t, in_=h_wp[c])
                        wp_pre[c] = t
                    bp0 = bp_p.tile([P, GRP * CN], bf16, tag="bp",
                                    name="bp_pre0")
                    nc.gpsimd.dma_start(out=bp0, in_=bcast(h_bp, GRP * CN))
                    bp_pre[0] = bp0

            # ---- layer 2, pipelined in qt-pair halves ----
            def transpose_cols(dst, src_b, a_list, tag):
                """transpose x[P, a, db*P:(db+1)*P] -> dst[:, db, a*P:(a+1)*P]."""
                for a in a_list:
                    for db in range(DT):
                        tp = ps.tile([P, 512], bf16, tag="ps", name=tag)
                        nc.tensor.transpose(
                            out=tp[:, :P],
                            in_=src_b[:, a, db * P:(db + 1) * P],
                            identity=ident)
                        nc.scalar.copy(
                            out=dst[:, db, a * P:(a + 1) * P], in_=tp[:, :P])

            def proj_T_into(o, w_sb, b_sb, rhsT, c0, c1, kt=DT):
                w = c1 - c0
                for m in range(DT):
                    pm = ps.tile([P, 512], f32, tag="ps", name="pm")
                    for k in range(kt):
                        nc.tensor.matmul(
                            pm[:, :w],
                            lhsT=w_sb[:, k * D + m * P:k * D + (m + 1) * P],
                            rhs=rhsT[:, k, c0:c1],
                            start=(k == 0), stop=(k == kt - 1))
                    nc.scalar.activation(out=o[:, m, c0:c1], in_=pm[:, :w],
                                         func=ACT_F.Identity,
                                         bias=b_sb[:, m:m + 1], scale=1.0)

            x1T = xt_p.tile([P, DT, S], bf16, tag="xt", name="x1t")
            Q2T = qk_p.tile([P, DT, S], bf16, tag="qk", name="q2t")
            P2bs = []
            rinv2 = stat_p.tile([P, ST], f32, tag="rinv2")

            def scores2_softmax(qt):
                pm = ps.tile([P, 512], f32, tag="ps")
                for k in range(DT):
                    nc.tensor.matmul(pm[:, :NI],
                                     lhsT=Q2T[:, k, qt * P:(qt + 1) * P],
                                     rhs=K2T[:, k, :NI],
                                     start=(k == 0), stop=(k == DT - 1))
                nmax = stat_p.tile([P, 1], f32, tag="nmax")
                nc.vector.reduce_max(nmax, pm[:, :NI], axis=X, negate=True)
                P2b = pb_p.tile([P, NI_PAD], bf16, tag="pb2", name=f"p2b{qt}")
                nc.gpsimd.memset(P2b[:, NI:], 0.0)
                rsum = stat_p.tile([P, 1], f32, tag="rsum")
                nc.scalar.activation(out=P2b[:, :NI], in_=pm[:, :NI],
                                     func=ACT_F.Exp, bias=nmax, scale=1.0,
                                     accum_out=rsum)
                nc.vector.reciprocal(out=rinv2[:, qt:qt + 1], in_=rsum)
                P2bs.append(P2b)

            # ---- AV2 + residual + LN2 per qt (normalized out; affine folded
            # into Wp/bp) ----
            PT2 = pt_p.tile([P, NIT, S], bf16, tag="pt2")
            x2b = xb_p.tile([P, ST, D], bf16, tag="xb", name="x2b")

            def av2_ln2(qt):
                for kt in range(NIT):
                    tp = ps.tile([P, 512], bf16, tag="ps", name="tp2")
                    nc.tensor.transpose(out=tp[:, :P],
                                        in_=P2bs[qt][:, kt * P:(kt + 1) * P],
                                        identity=ident)
                    nc.vector.tensor_copy(out=PT2[:, kt, qt * P:(qt + 1) * P],
                                          in_=tp[:, :P])
                xpre = xpre_p.tile([P, D], f32, tag="xpre")
                for nh in range(2):
                    sl = slice(nh * 512, (nh + 1) * 512)
                    pm = ps.tile([P, 512], f32, tag="ps")
                    for kt in range(NIT):
                        nc.tensor.matmul(pm, lhsT=PT2[:, kt, qt * P:(qt + 1) * P],
                                         rhs=V2t[:, kt, nh * 512:(nh + 1) * 512],
                                         start=(kt == 0), stop=(kt == NIT - 1))
                    nc.scalar.activation(out=xpre[:, sl], in_=pm,
                                         func=ACT_F.Identity,
                                         scale=rinv2[:, qt:qt + 1])
                    eng = nc.vector if nh == 0 else nc.gpsimd
                    eng.tensor_tensor(out=xpre[:, sl], in0=xpre[:, sl],
                                      in1=x1b[:, qt, sl], op=ALU.add)
                layernorm(xpre, x2b[:, qt, :], None)

            x2T = xt_p.tile([P, DT, S], bf16, tag="xt", name="x2t")

            # Layer-2 pipeline: second-half projections and transposes fill
            # the LN shadows of the first half; vocab A fills the tail.
            transpose_cols(x1T, x1b, (0, 1), "x1t_tp")
            proj_T_into(Q2T, wq2_sb, bq2s, x1T, 0, 256)
            scores2_softmax(0)
            scores2_softmax(1)
            av2_ln2(0)
            transpose_cols(x1T, x1b, (2, 3), "x1t_tp")
            av2_ln2(1)
            proj_T_into(Q2T, wq2_sb, bq2s, x1T, 256, 512)
            transpose_cols(x2T, x2b, (0,), "x2t_tp")
            scores2_softmax(2)
            scores2_softmax(3)
            transpose_cols(x2T, x2b, (1,), "x2t_tp")
            av2_ln2(2)
            av2_ln2(3)

            # ---- vocab projection, streamed in CN-column chunks ----
            def vocab_chunks(chunks, qts, dma_par, pre=()):
                """Process wp[chunks] x qts; chunks must align to GRP groups."""
                for gi in range(0, len(chunks), GRP):
                    g = chunks[gi] // GRP
                    if g in bp_pre:
                        bp_bc = bp_pre.pop(g)
                    else:
                        bp_bc = bp_p.tile([P, GRP * CN], bf16, tag="bp")
                        nc.gpsimd.dma_start(out=bp_bc,
                                            in_=bcast(h_bp, GRP * CN,
                                                      offset=g * GRP * CN))
                    osb = {q: osb_p.tile([P, GRP * CN], bf16, tag="osb",
                                         name=f"osb_{g}_{q}")
                           for q in qts}
                    for cc in range(GRP):
                        c = chunks[gi + cc]
                        if c in pre and c in wp_pre:
                            wp_sb = wp_pre.pop(c)
                        else:
                            wp_sb = wp_p.tile([P, DT * CN], bf16, tag="wp")
                            dma_eng = (nc.sync if (c + dma_par) % 2 == 0
                                       else nc.scalar)
                            dma_eng.dma_start(out=wp_sb, in_=h_wp[c])
                        for qt in qts:
                            pm = ps.tile([P, 512], f32, tag="ps")
                            for k in range(DT):
                                nc.tensor.matmul(
                                    pm, lhsT=x2T[:, k, qt * P:(qt + 1) * P],
                                    rhs=wp_sb[:, k * CN:(k + 1) * CN],
                                    start=(k == 0), stop=(k == DT - 1))
                            nc.vector.tensor_tensor(
                                out=osb[qt][:, cc * CN:(cc + 1) * CN], in0=pm,
                                in1=bp_bc[:, cc * CN:(cc + 1) * CN], op=ALU.add)
                            if cc == GRP - 1:
                                # fire each strip as soon as it completes
                                out_eng = nc.sync if qt < 2 else nc.scalar
                                out_eng.dma_start(
                                    out=h_out[qt * P:(qt + 1) * P,
                                              g * GRP * CN:(g + 1) * GRP * CN],
                                    in_=osb[qt])

            # early pass: first NE chunks for qt {0,1} while LN2(2,3) drain
            vocab_chunks(list(range(NE)), (0, 1), 0, pre=(0, 1, 2))
            transpose_cols(x2T, x2b, (2, 3), "x2t_tp")
            # late pass for those chunks' qt {2,3} (re-streamed), then the rest
            vocab_chunks(list(range(NE)), (2, 3), 1)
            vocab_chunks(list(range(NE, NCHUNK)), (0, 1, 2, 3), 0)

    nc.compile()
    return nc


def _tile_sq(w, kt):
    """[K, N] -> [128, K//128, N] contiguous."""
    k, n = w.shape
    assert k == kt * P
    return np.ascontiguousarray(
        w.reshape(kt, P, n).transpose(1, 0, 2)).astype(BF16)


def _pos_enc():
    posn = np.arange(S)[:, None].astype(np.float32)
    i = np.arange(0, D, 2).astype(np.float32)
    ang = posn / np.power(10000.0, i / D)
    pos = np.zeros((S, D), dtype=np.float32)
    pos[:, 0::2] = np.sin(ang)
    pos[:, 1::2] = np.cos(ang)
    return pos


def _prep_inputs(inputs):
    g = lambda name: np.asarray(inputs[name], dtype=np.float32)
    tokens = np.asarray(inputs["tokens"]).astype(np.int64)
    img = g("img_emb")
    table = g("emb_table")
    pos = _pos_enc()

    b1 = g("b1")
    g2 = g("g2")
    b2 = g("b2")
    wp = g("Wp") * g2[:, None]          # fold LN2 gamma
    wp_pad = np.zeros((D, VP), dtype=np.float32)
    wp_pad[:, :V] = wp
    wp_t = np.ascontiguousarray(
        wp_pad.reshape(DT, P, NCHUNK, CN).transpose(2, 1, 0, 3)).astype(BF16)
    bp_pad = np.zeros((VP,), dtype=np.float32)
    bp_pad[:V] = g("bp") + b2 @ g("Wp")  # fold LN2 beta
    bp_pad = bp_pad.astype(BF16)

    def bias_tiled(b):
        return np.ascontiguousarray(b.reshape(DT, P).T).astype(np.float32)

    shared = {
        "wq1": _tile_sq(g("Wq1") * SCALE, DT).reshape(P, -1),
        "wk1": _tile_sq(g("Wk1"), DT).reshape(P, -1),
        "wv1": _tile_sq(g("Wv1"), DT).reshape(P, -1),
        "wq2": _tile_sq(g("Wq2") * SCALE, DT).reshape(P, -1),
        "wk2": _tile_sq(g("Wk2"), DIT).reshape(P, -1),
        "wv2": _tile_sq(g("Wv2"), DIT).reshape(P, -1),
        "wp": wp_t.reshape(NCHUNK, P, -1),
        "bq1": bias_tiled(g("bq1") * SCALE),
        "bk1": bias_tiled(g("bk1")),
        # fold LN1 beta into the cross-attn query bias and value bias
        "bq2": bias_tiled((g("bq2") + b1 @ g("Wq2")) * SCALE),
        "bk2": bias_tiled(g("bk2")),
        "bv2": g("bv2") + b1,
        "g1": g("g1"),
        "bp": bp_pad,
    }
    in_maps = []
    for c in range(N_CORES):
        m = dict(shared)
        x0 = table[tokens[c]] + pos                      # [S, D] f32
        # bv1 folded into the self-attn residual (A1 rows sum to 1)
        x0r = x0 + g("bv1")
        x0b = np.ascontiguousarray(
            x0r.reshape(ST, P, D).transpose(1, 0, 2)).astype(BF16)
        x0T = np.ascontiguousarray(
            x0.T.reshape(DT, P, S).transpose(1, 0, 2)).astype(BF16)
        m["x0b"] = x0b.reshape(P, -1)
        m["x0t"] = x0T.reshape(P, -1)
        m["img_t"] = np.ascontiguousarray(
            img[c].T.reshape(DIT, P, NI).transpose(1, 0, 2)).astype(
                BF16).reshape(P, -1)
        in_maps.append(m)
    return in_maps


def _ensure_axon_hooks():
    """bass_utils imports antenv.axon_hooks when BASS_TRACE is set; stub it
    if the module is absent so tracing degrades instead of crashing."""
    try:
        import antenv.axon_hooks  # noqa: F401
    except ImportError:
        import types
        mod = types.ModuleType("antenv.axon_hooks")
        mod.get_axon_ntff_profile_hook = lambda: None
        mod.set_axon_ntff_profile_hook = lambda h: None
        sys.modules["antenv.axon_hooks"] = mod


def kernel(**inputs):
    global LAST_RESULTS
    _ensure_axon_hooks()
    from concourse.bass_utils import run_bass_kernel_spmd

    if "nc" not in _CACHE:
        _CACHE["nc"] = _build_program()
    nc = _CACHE["nc"]

    in_maps = _prep_inputs(inputs)
    res = run_bass_kernel_spmd(nc, in_maps, core_ids=list(range(N_CORES)))
    LAST_RESULTS = res
    out = np.stack([res.results[c]["out"][:, :V].astype(np.float32)
                    for c in range(N_CORES)])
    return out


# revision 71
# speedup vs baseline: 1.0795x; 1.0086x over previous
"""Trainium2 Bass kernel for an 8-batch image-conditioned decoder layer.

Strategy: pure data-parallel over the batch — core c computes batch element c
end-to-end (causal self-attention, cross-attention over the image tokens, both
layernorms, vocab projection). No collectives.

Schedule notes:
- Embedding gather + positional encoding are host-prepped (pure data movement);
  the device receives x0 in both seq-partition and d-partition layouts.
- Every DMA-touched tensor is laid out 2D ([P, free]) so each transfer lowers
  to a single DIRECT2D descriptor (3D APs cost one issue slot per outer index,
  ~0.6us of engine time each).
- Dummy warmup matmuls run during the initial DMA window so the PE HAM clock
  gate is at full rate when QT starts.
- Q/K weights stream as per-m-group chunks just-in-time, paced by the compute
  stream, so early HBM bandwidth stays focused on the critical path; larger
  secondary tensors are released behind gates keyed on QT/KT progress.
- b1 is folded into bq2/bv2, bv1 into the residual copy of x0, and g2/b2 into
  Wp/bp, which trims the layernorm critical path.
- Layer 2 runs per-qt-pipelined (Q2T in two 256-col halves) and the first
  vocab chunks are computed early for qt {0,1}; those chunks are re-streamed
  later for qt {2,3}.
- PSUM->SBUF moves run on Scalar; SBUF-only elementwise work on GpSimd; VEC
  keeps softmax/bn_stats and the vocab bias adds.

All matmuls run in bf16 with fp32 PSUM accumulation.
"""

import os
import sys

for _p in ("/opt/trn_rl_repo", "/root/.axon_site/_ro/trn_rl_repo"):
    if os.path.isdir(_p) and _p not in sys.path:
        sys.path.append(_p)

import numpy as np
import ml_dtypes

BF16 = ml_dtypes.bfloat16

# Problem dims (hardcoded per spec)
V, D, DI, S, B, NI = 32000, 1024, 768, 512, 8, 197
EPS = 1e-5
P = 128
ST = S // P          # 4 seq tiles
DT = D // P          # 8 model-dim tiles
DIT = DI // P        # 6 image-dim tiles
NIT = 2              # image tokens: 197 -> 2 partition tiles (128 + 69)
NI_PAD = 256
VP = 32768           # vocab padded to 64 chunks of 512
CN = 512             # vocab chunk width
NCHUNK = VP // CN    # 64
GRP = 2              # chunks per output strip
NGRP = NCHUNK // GRP
NE = 4               # chunks computed early for qt {0,1} (re-streamed later)
N_CORES = 8
SCALE = 1.0 / float(np.sqrt(np.float32(D)))

_CACHE = {}
LAST_RESULTS = None


def _build_program():
    import concourse.bacc as bacc
    import concourse.bass as bass
    import concourse.mybir as mybir
    from concourse.masks import make_identity
    from concourse.tile import TileContext

    f32 = mybir.dt.float32
    bf16 = mybir.dt.bfloat16
    X = mybir.AxisListType.X
    ALU = mybir.AluOpType
    ACT_F = mybir.ActivationFunctionType

    nc = bacc.Bacc("TRN2", target_bir_lowering=False, debug=False,
                   num_devices=N_CORES)

    # ---- I/O (all 2D so every DMA is a single DIRECT2D) ----
    h_x0b = nc.dram_tensor("x0b", [P, ST * D], bf16, kind="ExternalInput")
    h_x0T = nc.dram_tensor("x0t", [P, DT * S], bf16, kind="ExternalInput")
    h_img = nc.dram_tensor("img_t", [P, DIT * NI], bf16, kind="ExternalInput")
    h_wq1 = nc.dram_tensor("wq1", [P, DT * D], bf16, kind="ExternalInput")
    h_wk1 = nc.dram_tensor("wk1", [P, DT * D], bf16, kind="ExternalInput")
    h_wv1 = nc.dram_tensor("wv1", [P, DT * D], bf16, kind="ExternalInput")
    h_wq2 = nc.dram_tensor("wq2", [P, DT * D], bf16, kind="ExternalInput")
    h_wk2 = nc.dram_tensor("wk2", [P, DIT * D], bf16, kind="ExternalInput")
    h_wv2 = nc.dram_tensor("wv2", [P, DIT * D], bf16, kind="ExternalInput")
    h_wp = nc.dram_tensor("wp", [NCHUNK, P, DT * CN], bf16,
                          kind="ExternalInput")
    h_bq1 = nc.dram_tensor("bq1", [P, DT], f32, kind="ExternalInput")
    h_bk1 = nc.dram_tensor("bk1", [P, DT], f32, kind="ExternalInput")
    h_bq2 = nc.dram_tensor("bq2", [P, DT], f32, kind="ExternalInput")
    h_bk2 = nc.dram_tensor("bk2", [P, DT], f32, kind="ExternalInput")
    h_bv2 = nc.dram_tensor("bv2", [D], bf16, kind="ExternalInput")
    h_g1 = nc.dram_tensor("g1", [D], bf16, kind="ExternalInput")
    h_bp = nc.dram_tensor("bp", [VP], bf16, kind="ExternalInput")
    h_out = nc.dram_tensor("out", [S, VP], bf16, kind="ExternalOutput")

    def bcast(handle, n, offset=0):
        ap = handle[:]
        return bass.AP(tensor=ap.tensor, offset=offset, ap=[[0, P], [1, n]])

    with TileContext(nc) as tc:
        import contextlib
        ctx = contextlib.ExitStack()
        with ctx:
            const = ctx.enter_context(tc.tile_pool(name="const", bufs=1))
            xb_p = ctx.enter_context(tc.tile_pool(name="xb", bufs=3))
            xt_p = ctx.enter_context(tc.tile_pool(name="xt", bufs=2))
            qk_p = ctx.enter_context(tc.tile_pool(name="qk", bufs=2))
            v_p = ctx.enter_context(tc.tile_pool(name="vp", bufs=2))
            k2t_p = ctx.enter_context(tc.tile_pool(name="k2t", bufs=1))
            pb_p = ctx.enter_context(tc.tile_pool(name="pb", bufs=4))
            pt_p = ctx.enter_context(tc.tile_pool(name="pt", bufs=1))
            xpre_p = ctx.enter_context(tc.tile_pool(name="xpre", bufs=2))
            stat_p = ctx.enter_context(tc.tile_pool(name="stat", bufs=4))
            wts_p = ctx.enter_context(tc.tile_pool(name="wts", bufs=2))
            wv2_p = ctx.enter_context(tc.tile_pool(name="wv2p", bufs=1))
            wp_p = ctx.enter_context(tc.tile_pool(name="wpp", bufs=4))
            bp_p = ctx.enter_context(tc.tile_pool(name="bpp", bufs=2))
            osb_p = ctx.enter_context(tc.tile_pool(name="osb", bufs=5))
            ps = ctx.enter_context(tc.tile_pool(name="ps", bufs=8, space="PSUM"))

            # ---- constants / warmup ----
            ident = const.tile([P, P], bf16)
            make_identity(nc, ident)
            trimask = const.tile([P, P], f32)
            nc.gpsimd.memset(trimask, 0.0)
            nc.gpsimd.affine_select(
                out=trimask, in_=trimask, compare_op=ALU.is_ge, fill=-1e10,
                base=0, pattern=[[-1, P]], channel_multiplier=1)
            warm_src = const.tile([P, 256], bf16)
            nc.vector.memset(warm_src, 0.0)
            epst = const.tile([P, 1], f32)
            nc.vector.memset(epst, EPS)
            neg1 = const.tile([P, 1], f32)
            nc.vector.memset(neg1, -1.0)

            # HAM warmup: keep the PE busy while the first weights stream in
            for w in range(28):
                pw = ps.tile([P, 512], f32, tag="ps", name=f"warm{w}")
                nc.tensor.matmul(pw[:, :256], lhsT=ident, rhs=warm_src,
                                 start=True, stop=True)

            # ---- early DMAs ----
            x0T = xt_p.tile([P, DT * S], bf16, tag="xt", name="x0t")
            nc.sync.dma_start(out=x0T, in_=h_x0T[:])
            bq1s = const.tile([P, DT], f32)
            bk1s = const.tile([P, DT], f32)
            bq2s = const.tile([P, DT], f32)
            bk2s = const.tile([P, DT], f32)
            for t, h in ((bq1s, h_bq1), (bk1s, h_bk1), (bq2s, h_bq2),
                         (bk2s, h_bk2)):
                nc.gpsimd.dma_start(out=t, in_=h[:])

            # K2T's inputs (img 0.3MB + wk2 1.5MB) are the smallest load that
            # unlocks real PE work, so they go first alongside x0T+wq1; the
            # rest is gated so the early HBM window stays uncongested.
            img_sb = const.tile([P, DIT * NI], bf16)
            nc.gpsimd.dma_start(out=img_sb, in_=h_img[:])
            wk2_sb = wv2_p.tile([P, DIT * D], bf16, tag="wk2")
            nc.gpsimd.dma_start(out=wk2_sb, in_=h_wk2[:])
            wq1_sb = wts_p.tile([P, DT * D], bf16, tag="wts")
            nc.scalar.dma_start(out=wq1_sb, in_=h_wq1[:])

            # ---- cross-attn K2 (first real PE work; minimal DMA deps) ----
            K2T = k2t_p.tile([P, DT, NI_PAD], bf16, tag="k2t")
            for m in range(DT):
                pm = ps.tile([P, 512], f32, tag="ps")
                for k in range(DIT):
                    nc.tensor.matmul(
                        pm[:, :NI],
                        lhsT=wk2_sb[:, k * D + m * P:k * D + (m + 1) * P],
                        rhs=img_sb[:, k * NI:(k + 1) * NI],
                        start=(k == 0), stop=(k == DIT - 1))
                nc.scalar.activation(out=K2T[:, m, :NI], in_=pm[:, :NI],
                                     func=ACT_F.Identity,
                                     bias=bk2s[:, m:m + 1], scale=1.0)

            def proj_T(w_sb, b_sb, name):
                """out[P, DT, S] bf16 = (W.T @ x0.T) + b, d-partition."""
                o = qk_p.tile([P, DT, S], bf16, tag="qk", name=name)
                for m in range(DT):
                    pm = ps.tile([P, 512], f32, tag="ps", name="pm")
                    for k in range(DT):
                        nc.tensor.matmul(
                            pm,
                            lhsT=w_sb[:, k * D + m * P:k * D + (m + 1) * P],
                            rhs=x0T[:, k * S:(k + 1) * S],
                            start=(k == 0), stop=(k == DT - 1))
                    nc.scalar.activation(out=o[:, m, :], in_=pm,
                                         func=ACT_F.Identity,
                                         bias=b_sb[:, m:m + 1], scale=1.0)
                return o

            QT = proj_T(wq1_sb, bq1s, "qt")

            # gpsimd-gated DMAs: released once QT compute is underway so they
            # don't steal HBM bandwidth from the critical startup path
            gate_t = const.tile([P, 1], bf16)
            nc.gpsimd.tensor_copy(out=gate_t, in_=QT[:, 0, 0:1])
            wk1_sb = wts_p.tile([P, DT * D], bf16, tag="wts")
            nc.gpsimd.dma_start(out=wk1_sb, in_=h_wk1[:])
            # wv1/wq2 are ring-slot gated behind QT/KT consumption of wq1/wk1
            wv1_sb = wts_p.tile([P, DT * D], bf16, tag="wts")
            nc.sync.dma_start(out=wv1_sb, in_=h_wv1[:])
            wq2_sb = wts_p.tile([P, DT * D], bf16, tag="wts")
            nc.sync.dma_start(out=wq2_sb, in_=h_wq2[:])

            KT = proj_T(wk1_sb, bk1s, "kt")

            gate_t2 = const.tile([P, 1], bf16)
            nc.gpsimd.tensor_copy(out=gate_t2, in_=KT[:, 0, 0:1])
            x0b = xb_p.tile([P, ST * D], bf16, tag="xb", name="x0b")
            nc.gpsimd.dma_start(out=x0b, in_=h_x0b[:])
            wv2_sb = wv2_p.tile([P, DIT * D], bf16, tag="wv2")
            nc.gpsimd.dma_start(out=wv2_sb, in_=h_wv2[:])
            g1b = const.tile([P, D], bf16)
            bv2b = const.tile([P, D], bf16)
            for t, h in ((g1b, h_g1), (bv2b, h_bv2)):
                nc.gpsimd.dma_start(out=t, in_=bcast(h, D))

            # value projection (bv1 folded into x0b host-side; attention rows
            # are convex combinations so the V-bias passes through unchanged)
            Vt = v_p.tile([P, ST, D], bf16, tag="v")

            def vt_block(a):
                for nh in range(2):
                    pm = ps.tile([P, 512], f32, tag="ps")
                    for k in range(DT):
                        nc.tensor.matmul(
                            pm,
                            lhsT=x0T[:, k * S + a * P:k * S + (a + 1) * P],
                            rhs=wv1_sb[:, k * D + nh * 512:
                                       k * D + (nh + 1) * 512],
                            start=(k == 0), stop=(k == DT - 1))
                    nc.scalar.copy(out=Vt[:, a, nh * 512:(nh + 1) * 512],
                                   in_=pm)

            # Vt a=0,1 now; a=2,3 deferred into the softmax1/AV1 shadow
            vt_block(0)
            vt_block(1)

            # ---- causal self-attention: scores + softmax (all qt) ----
            Pbs = []
            rinv1 = stat_p.tile([P, ST], f32, tag="rinv")
            for qt in range(ST):
                width = (qt + 1) * P
                pm = ps.tile([P, 512], f32, tag="ps")
                for k in range(DT):
                    nc.tensor.matmul(pm[:, :width],
                                     lhsT=QT[:, k, qt * P:(qt + 1) * P],
                                     rhs=KT[:, k, :width],
                                     start=(k == 0), stop=(k == DT - 1))
                # mask the diagonal block in place (PSUM RMW)
                nc.vector.tensor_tensor(out=pm[:, qt * P:width],
                                        in0=pm[:, qt * P:width], in1=trimask,
                                        op=ALU.add)
                nmax = stat_p.tile([P, 1], f32, tag="nmax")
                nc.vector.reduce_max(nmax, pm[:, :width], axis=X, negate=True)
                Pb = pb_p.tile([P, 512], bf16, tag="pb", name=f"pb{qt}")
                rsum = stat_p.tile([P, 1], f32, tag="rsum")
                nc.scalar.activation(out=Pb[:, :width], in_=pm[:, :width],
                                     func=ACT_F.Exp, bias=nmax, scale=1.0,
                                     accum_out=rsum)
                nc.vector.reciprocal(out=rinv1[:, qt:qt + 1], in_=rsum)
                Pbs.append(Pb)

            vt_block(2)  # fills the softmax1 shadow

            def layernorm(xpre, out_sl, gb):
                """xpre [P, D] f32 -> out_sl [P, D] bf16.

                Writes the normalized rows times gb (or raw normalized rows if
                gb is None — affine folded into the consumers)."""
                stats = stat_p.tile([P, 2, 6], f32, tag="bnst")
                for sg in range(2):
                    nc.vector.bn_stats(out=stats[:, sg, :],
                                       in_=xpre[:, sg * 512:(sg + 1) * 512])
                mv = stat_p.tile([P, 2], f32, tag="bnmv")
                nc.vector.bn_aggr(out=mv, in_=stats)
                rstd = stat_p.tile([P, 1], f32, tag="rstd")
                nc.scalar.activation(out=rstd, in_=mv[:, 1:2], func=ACT_F.Sqrt,
                                     bias=epst, scale=1.0)
                nc.vector.reciprocal(out=rstd, in_=rstd)
                nmr = stat_p.tile([P, 1], f32, tag="nmr")
                nc.vector.scalar_tensor_tensor(
                    out=nmr, in0=mv[:, 0:1], scalar=rstd, in1=neg1,
                    op0=ALU.mult, op1=ALU.mult)
                if gb is None:
                    nc.scalar.activation(out=out_sl, in_=xpre,
                                         func=ACT_F.Identity,
                                         bias=nmr, scale=rstd)
                else:
                    nc.scalar.activation(out=xpre, in_=xpre,
                                         func=ACT_F.Identity,
                                         bias=nmr, scale=rstd)
                    nc.vector.tensor_tensor(out=out_sl[:, :512],
                                            in0=xpre[:, :512],
                                            in1=gb[:, :512], op=ALU.mult)
                    nc.gpsimd.tensor_tensor(out=out_sl[:, 512:],
                                            in0=xpre[:, 512:],
                                            in1=gb[:, 512:], op=ALU.mult)

            # ---- AV1 + residual + LN1 per qt ----
            PT = pt_p.tile([P, ST, S], bf16, tag="pt")
            x1b = xb_p.tile([P, ST, D], bf16, tag="xb", name="x1b")
            wp_pre = {}
            bp_pre = {}
            # cross-attn V2 blocks are interleaved below as PE filler while
            # the VEC-bound AV1/LN1 chain drains
            V2t = v_p.tile([P, NIT, D], bf16, tag="v")
            nc.gpsimd.memset(V2t, 0.0)

            def v2_block(a, nh):
                pa = P if a == 0 else NI - P
                pm = ps.tile([P, 512], f32, tag="ps")
                for k in range(DIT):
                    nc.tensor.matmul(
                        pm[:pa, :],
                        lhsT=img_sb[:, k * NI + a * P:k * NI + a * P + pa],
                        rhs=wv2_sb[:, k * D + nh * 512:k * D + (nh + 1) * 512],
                        start=(k == 0), stop=(k == DIT - 1))
                nc.vector.tensor_tensor(
                    out=V2t[:pa, a, nh * 512:(nh + 1) * 512], in0=pm[:pa, :],
                    in1=bv2b[:pa, nh * 512:(nh + 1) * 512], op=ALU.add)

            for qt in range(ST):
                for kt in range(qt + 1):
                    tp = ps.tile([P, 512], bf16, tag="ps", name="tp")
                    nc.tensor.transpose(out=tp[:, :P],
                                        in_=Pbs[qt][:, kt * P:(kt + 1) * P],
                                        identity=ident)
                    nc.vector.tensor_copy(out=PT[:, kt, qt * P:(qt + 1) * P],
                                          in_=tp[:, :P])
                xpre = xpre_p.tile([P, D], f32, tag="xpre")
                for nh in range(2):
                    sl = slice(nh * 512, (nh + 1) * 512)
                    pm = ps.tile([P, 512], f32, tag="ps")
                    for kt in range(qt + 1):
                        nc.tensor.matmul(pm, lhsT=PT[:, kt, qt * P:(qt + 1) * P],
                                         rhs=Vt[:, kt, nh * 512:(nh + 1) * 512],
                                         start=(kt == 0), stop=(kt == qt))
                    # residual split: SCA rescale, then VEC/GPS halves add
                    nc.scalar.activation(out=xpre[:, sl], in_=pm,
                                         func=ACT_F.Identity,
                                         scale=rinv1[:, qt:qt + 1])
                    eng = nc.vector if nh == 0 else nc.gpsimd
                    eng.tensor_tensor(
                        out=xpre[:, sl], in0=xpre[:, sl],
                        in1=x0b[:, qt * D + nh * 512:qt * D + (nh + 1) * 512],
                        op=ALU.add)
                if qt == 0:
                    vt_block(3)  # PE filler during stt/LN1
                v2_block(qt % NIT, qt // NIT)  # PE filler during stt/LN1
                layernorm(xpre, x1b[:, qt, :], g1b)
                if qt == 0:
                    # prefetch the first vocab chunks + bias strips now that
                    # the startup DMA window has drained (gpsimd stream pos)
                    for c in range(NE):
                        t = wp_p.tile([P, DT * CN], bf16, tag="wp",
                                      name=f"wp_pre{c}")
                        nc.gpsimd.dma_start(out=t, in_=h_wp[c])
                        wp_pre[c] = t
                    for gg in range(NE // GRP):
                        bpt = bp_p.tile([P, GRP * CN], bf16, tag="bp",
                                        name=f"bp_pre{gg}")
                        nc.gpsimd.dma_start(
                            out=bpt, in_=bcast(h_bp, GRP * CN,
                                               offset=gg * GRP * CN))
                        bp_pre[gg] = bpt

            # ---- layer 2, pipelined in qt-pair halves ----
            def transpose_cols(dst, src_b, a_list, tag):
                """transpose x[P, a, db*P:(db+1)*P] -> dst[:, db, a*P:(a+1)*P]."""
                for a in a_list:
                    for db in range(DT):
                        tp = ps.tile([P, 512], bf16, tag="ps", name=tag)
                        nc.tensor.transpose(
                            out=tp[:, :P],
                            in_=src_b[:, a, db * P:(db + 1) * P],
                            identity=ident)
                        nc.scalar.copy(
                            out=dst[:, db, a * P:(a + 1) * P], in_=tp[:, :P])

            def proj_T_into(o, w_sb, b_sb, rhsT, c0, c1, kt=DT):
                w = c1 - c0
                for m in range(DT):
                    pm = ps.tile([P, 512], f32, tag="ps", name="pm")
                    for k in range(kt):
                        nc.tensor.matmul(
                            pm[:, :w],
                            lhsT=w_sb[:, k * D + m * P:k * D + (m + 1) * P],
                            rhs=rhsT[:, k, c0:c1],
                            start=(k == 0), stop=(k == kt - 1))
                    nc.scalar.activation(out=o[:, m, c0:c1], in_=pm[:, :w],
                                         func=ACT_F.Identity,
                                         bias=b_sb[:, m:m + 1], scale=1.0)

            x1T = xt_p.tile([P, DT, S], bf16, tag="xt", name="x1t")
            Q2T = qk_p.tile([P, DT, S], bf16, tag="qk", name="q2t")
            P2bs = []
            rinv2 = stat_p.tile([P, ST], f32, tag="rinv2")

            def scores2_softmax(qt):
                pm = ps.tile([P, 512], f32, tag="ps")
                for k in range(DT):
                    nc.tensor.matmul(pm[:, :NI],
                                     lhsT=Q2T[:, k, qt * P:(qt + 1) * P],
                                     rhs=K2T[:, k, :NI],
                                     start=(k == 0), stop=(k == DT - 1))
                nmax = stat_p.tile([P, 1], f32, tag="nmax")
                nc.vector.reduce_max(nmax, pm[:, :NI], axis=X, negate=True)
                P2b = pb_p.tile([P, NI_PAD], bf16, tag="pb2", name=f"p2b{qt}")
                nc.gpsimd.memset(P2b[:, NI:], 0.0)
                rsum = stat_p.tile([P, 1], f32, tag="rsum")
                nc.scalar.activation(out=P2b[:, :NI], in_=pm[:, :NI],
                                     func=ACT_F.Exp, bias=nmax, scale=1.0,
                                     accum_out=rsum)
                nc.vector.reciprocal(out=rinv2[:, qt:qt + 1], in_=rsum)
                P2bs.append(P2b)

            # ---- AV2 + residual + LN2 per qt (normalized out; affine folded
            # into Wp/bp) ----
            PT2 = pt_p.tile([P, NIT, S], bf16, tag="pt2")
            x2b = xb_p.tile([P, ST, D], bf16, tag="xb", name="x2b")

            def av2_ln2(qt):
                for kt in range(NIT):
                    tp = ps.tile([P, 512], bf16, tag="ps", name="tp2")
                    nc.tensor.transpose(out=tp[:, :P],
                                        in_=P2bs[qt][:, kt * P:(kt + 1) * P],
                                        identity=ident)
                    nc.vector.tensor_copy(out=PT2[:, kt, qt * P:(qt + 1) * P],
                                          in_=tp[:, :P])
                xpre = xpre_p.tile([P, D], f32, tag="xpre")
                for nh in range(2):
                    sl = slice(nh * 512, (nh + 1) * 512)
                    pm = ps.tile([P, 512], f32, tag="ps")
                    for kt in range(NIT):
                        nc.tensor.matmul(pm, lhsT=PT2[:, kt, qt * P:(qt + 1) * P],
                                         rhs=V2t[:, kt, nh * 512:(nh + 1) * 512],
                                         start=(kt == 0), stop=(kt == NIT - 1))
                    nc.scalar.activation(out=xpre[:, sl], in_=pm,
                                         func=ACT_F.Identity,
                                         scale=rinv2[:, qt:qt + 1])
                    eng = nc.vector if nh == 0 else nc.gpsimd
                    eng.tensor_tensor(out=xpre[:, sl], in0=xpre[:, sl],
                                      in1=x1b[:, qt, sl], op=ALU.add)
                layernorm(xpre, x2b[:, qt, :], None)

            x2T = xt_p.tile([P, DT, S], bf16, tag="xt", name="x2t")

            # Layer-2 pipeline: second-half projections and transposes fill
            # the LN shadows of the first half; vocab A fills the tail.
            transpose_cols(x1T, x1b, (0, 1), "x1t_tp")
            proj_T_into(Q2T, wq2_sb, bq2s, x1T, 0, 256)
            scores2_softmax(0)
            scores2_softmax(1)
            av2_ln2(0)
            transpose_cols(x1T, x1b, (2, 3), "x1t_tp")
            av2_ln2(1)
            proj_T_into(Q2T, wq2_sb, bq2s, x1T, 256, 512)
            transpose_cols(x2T, x2b, (0,), "x2t_tp")
            scores2_softmax(2)
            scores2_softmax(3)
            transpose_cols(x2T, x2b, (1,), "x2t_tp")
            av2_ln2(2)
            av2_ln2(3)

            # ---- vocab projection, streamed in CN-column chunks ----
            def vocab_chunks(chunks, qts, dma_par, pre=()):
                """Process wp[chunks] x qts; chunks must align to GRP groups."""
                for gi in range(0, len(chunks), GRP):
                    g = chunks[gi] // GRP
                    if g in bp_pre:
                        bp_bc = bp_pre.pop(g)
                    else:
                        bp_bc = bp_p.tile([P, GRP * CN], bf16, tag="bp")
                        nc.gpsimd.dma_start(out=bp_bc,
                                            in_=bcast(h_bp, GRP * CN,
                                                      offset=g * GRP * CN))
                    osb = {q: osb_p.tile([P, GRP * CN], bf16, tag="osb",
                                         name=f"osb_{g}_{q}")
                           for q in qts}
                    for cc in range(GRP):
                        c = chunks[gi + cc]
                        if c in pre and c in wp_pre:
                            wp_sb = wp_pre.pop(c)
                        else:
                            wp_sb = wp_p.tile([P, DT * CN], bf16, tag="wp")
                            dma_eng = (nc.sync if (c + dma_par) % 2 == 0
                                       else nc.scalar)
                            dma_eng.dma_start(out=wp_sb, in_=h_wp[c])
                        for qt in qts:
                            pm = ps.tile([P, 512], f32, tag="ps")
                            for k in range(DT):
                                nc.tensor.matmul(
                                    pm, lhsT=x2T[:, k, qt * P:(qt + 1) * P],
                                    rhs=wp_sb[:, k * CN:(k + 1) * CN],
                                    start=(k == 0), stop=(k == DT - 1))
                            nc.vector.tensor_tensor(
                                out=osb[qt][:, cc * CN:(cc + 1) * CN], in0=pm,
                                in1=bp_bc[:, cc * CN:(cc + 1) * CN], op=ALU.add)
                            if cc == GRP - 1:
                                # fire each strip as soon as it completes
                                out_eng = nc.sync if qt < 2 else nc.scalar
                                out_eng.dma_start(
                                    out=h_out[qt * P:(qt + 1) * P,
                                              g * GRP * CN:(g + 1) * GRP * CN],
                                    in_=osb[qt])

            # early pass: first NE chunks for qt {0,1} while LN2(2,3) drain
            vocab_chunks(list(range(NE)), (0, 1), 0, pre=tuple(range(NE)))
            transpose_cols(x2T, x2b, (2, 3), "x2t_tp")
            # late pass for those chunks' qt {2,3} (re-streamed), then the rest
            vocab_chunks(list(range(NE)), (2, 3), 1)
            vocab_chunks(list(range(NE, NCHUNK)), (0, 1, 2, 3), 0)

    nc.compile()
    return nc


def _tile_sq(w, kt):
    """[K, N] -> [128, K//128, N] contiguous."""
    k, n = w.shape
    assert k == kt * P
    return np.ascontiguousarray(
        w.reshape(kt, P, n).transpose(1, 0, 2)).astype(BF16)


def _pos_enc():
    posn = np.arange(S)[:, None].astype(np.float32)
    i = np.arange(0, D, 2).astype(np.float32)
    ang = posn / np.power(10000.0, i / D)
    pos = np.zeros((S, D), dtype=np.float32)
    pos[:, 0::2] = np.sin(ang)
    pos[:, 1::2] = np.cos(ang)
    return pos


def _prep_inputs(inputs):
    g = lambda name: np.asarray(inputs[name], dtype=np.float32)
    tokens = np.asarray(inputs["tokens"]).astype(np.int64)
    img = g("img_emb")
    table = g("emb_table")
    pos = _pos_enc()

    b1 = g("b1")
    g2 = g("g2")
    b2 = g("b2")
    wp = g("Wp") * g2[:, None]          # fold LN2 gamma
    wp_pad = np.zeros((D, VP), dtype=np.float32)
    wp_pad[:, :V] = wp
    wp_t = np.ascontiguousarray(
        wp_pad.reshape(DT, P, NCHUNK, CN).transpose(2, 1, 0, 3)).astype(BF16)
    bp_pad = np.zeros((VP,), dtype=np.float32)
    bp_pad[:V] = g("bp") + b2 @ g("Wp")  # fold LN2 beta
    bp_pad = bp_pad.astype(BF16)

    def bias_tiled(b):
        return np.ascontiguousarray(b.reshape(DT, P).T).astype(np.float32)

    shared = {
        "wq1": _tile_sq(g("Wq1") * SCALE, DT).reshape(P, -1),
        "wk1": _tile_sq(g("Wk1"), DT).reshape(P, -1),
        "wv1": _tile_sq(g("Wv1"), DT).reshape(P, -1),
        "wq2": _tile_sq(g("Wq2") * SCALE, DT).reshape(P, -1),
        "wk2": _tile_sq(g("Wk2"), DIT).reshape(P, -1),
        "wv2": _tile_sq(g("Wv2"), DIT).reshape(P, -1),
        "wp": wp_t.reshape(NCHUNK, P, -1),
        "bq1": bias_tiled(g("bq1") * SCALE),
        "bk1": bias_tiled(g("bk1")),
        # fold LN1 beta into the cross-attn query bias and value bias
        "bq2": bias_tiled((g("bq2") + b1 @ g("Wq2")) * SCALE),
        "bk2": bias_tiled(g("bk2")),
        "bv2": (g("bv2") + b1).astype(BF16),
        "g1": g("g1").astype(BF16),
        "bp": bp_pad,
    }
    in_maps = []
    for c in range(N_CORES):
        m = dict(shared)
        x0 = table[tokens[c]] + pos                      # [S, D] f32
        # bv1 folded into the self-attn residual (A1 rows sum to 1)
        x0r = x0 + g("bv1")
        x0b = np.ascontiguousarray(
            x0r.reshape(ST, P, D).transpose(1, 0, 2)).astype(BF16)
        x0T = np.ascontiguousarray(
            x0.T.reshape(DT, P, S).transpose(1, 0, 2)).astype(BF16)
        m["x0b"] = x0b.reshape(P, -1)
        m["x0t"] = x0T.reshape(P, -1)
        m["img_t"] = np.ascontiguousarray(
            img[c].T.reshape(DIT, P, NI).transpose(1, 0, 2)).astype(
                BF16).reshape(P, -1)
        in_maps.append(m)
    return in_maps


def _ensure_axon_hooks():
    """bass_utils imports antenv.axon_hooks when BASS_TRACE is set; stub it
    if the module is absent so tracing degrades instead of crashing."""
    try:
        import antenv.axon_hooks  # noqa: F401
    except ImportError:
        import types
        mod = types.ModuleType("antenv.axon_hooks")
        mod.get_axon_ntff_profile_hook = lambda: None
        mod.set_axon_ntff_profile_hook = lambda h: None
        sys.modules["antenv.axon_hooks"] = mod


def kernel(**inputs):
    global LAST_RESULTS
    _ensure_axon_hooks()
    from concourse.bass_utils import run_bass_kernel_spmd

    if "nc" not in _CACHE:
        _CACHE["nc"] = _build_program()
    nc = _CACHE["nc"]

    in_maps = _prep_inputs(inputs)
    res = run_bass_kernel_spmd(nc, in_maps, core_ids=list(range(N_CORES)))
    LAST_RESULTS = res
    out = np.stack([res.results[c]["out"][:, :V].astype(np.float32)
                    for c in range(N_CORES)])
    return out
